# revision 7
# baseline (speedup 1.0000x reference)
"""Trainium2 Bass kernel v2: fp8 DoubleRow attention + bf16 MLP.

Sharding: sequence-parallel over 8 cores (512 tokens each, batch = core//4).
One 4-rank AllGather carries fp8 K (feature-major) + fp8 V (token-major).

Precision plan (validated in numpy, max_rel ~1.0e-2 vs 2e-2 gate):
  - weights qkv/proj: e4m3 x32 host-scaled; fc1/fc2: bf16 (MLP dominates error)
  - h1/q/k/v/pt/ctx: e4m3 (q,k,v at sigma~16 via 0.5 consume scale)
  - softmax exp: constant shift C=4 (cancels in normalize); split between
    Act (native Exp -> fp8) and DVE (Schraudolph: psum*a+b -> uint8 whose
    bit pattern IS e4m3 2^x; floor-vs-round ambiguity is a constant factor
    that cancels in the softmax normalize)
  - x residual fp32r; LN stats via ones-matmul (fp8 DoubleRow for LN1 on
    host-provided x8, fp32r for LN2)
DoubleRow pair slots: chunk pairs for QKV/ctx/proj; (k, zeros) for scores
(d=64 contraction cannot pair; zero slot makes the 0.5 cyc/row rate legal).
"""
import sys

sys.path.insert(0, "/opt/trn_rl_repo")
import numpy as np
import ml_dtypes
import concourse.bass as bass
import concourse.mybir as mybir
import concourse.tile as tile
from concourse import bacc
from concourse.bass_utils import run_bass_kernel_spmd

B, N, D = 2, 2048, 1024
H, DH = 16, 64
HID = 4096
NCORES = 8
TOK = (B * N) // NCORES  # 512
EPS = 1e-5
SCALE = DH**-0.5
P = 128
CH = D // P  # 8
KC = N // P  # 16
HCH = HID // P  # 32
RANKS = 4
CSH = 4.0  # exp arg shift, cancels in softmax
LN2_ = float(np.log(2.0))
# Schraudolph uint8-as-e4m3: y = psum * SA + SB
SA = 8.0 * (2.0**-11) / LN2_
SB = 56.5 - 8.0 * CSH / LN2_

F32 = mybir.dt.float32
F32R = mybir.dt.float32r
F8 = mybir.dt.float8e4
BF = mybir.dt.bfloat16
U8 = mybir.dt.uint8
AF = mybir.ActivationFunctionType
OP = mybir.AluOpType
DR = mybir.MatmulPerfMode.DoubleRow

REPLICA_GROUPS = [[0, 1, 2, 3], [4, 5, 6, 7]]

KV_K = D * TOK  # bytes of K region (fp8 feature-major [1024, 512])
DHP = DH + 16  # per-head stride in V region: 64 v + 1 ones + 15 pad
# (dual-fp8 LdWeights requires 16B-aligned weight base addresses)
KV_V = TOK * (H * DHP)  # V region [512, 1280]
KV_SZ = KV_K + KV_V


def round_fp32r(x: np.ndarray) -> np.ndarray:
    u = np.ascontiguousarray(x, dtype=np.float32).view(np.uint32)
    u = (u + 0x7FF + ((u >> 12) & 1)) & np.uint32(0xFFFFF000)
    return u.view(np.float32)


def _stripe(v: np.ndarray) -> np.ndarray:
    """[M] -> [P, M//P] with col m, part p = v[m*128+p]."""
    return np.ascontiguousarray(np.asarray(v, np.float32).reshape(-1, P).T)


def build_program(do_compile=True):
    nc = bacc.Bacc("TRN2", target_bir_lowering=False, debug=False, num_devices=NCORES)

    xT = nc.dram_tensor("xT", [D, TOK], F32, kind="ExternalInput").ap()
    x8T = nc.dram_tensor("x8T", [D, TOK], F8, kind="ExternalInput").ap()
    # weight tiles, DMA-contiguous per partition
    wqk8 = nc.dram_tensor("wqk8", [16, P, CH * P], F8, kind="ExternalInput").ap()
    wv8 = nc.dram_tensor("wv8", [2, P, CH * TOK], F8, kind="ExternalInput").ap()
    wp8 = nc.dram_tensor("wp8", [CH, P, CH * P], F8, kind="ExternalInput").ap()
    w1bf = nc.dram_tensor("w1bf", [CH, P, 4 * CH * P], BF, kind="ExternalInput").ap()
    w2bf = nc.dram_tensor("w2bf", [CH, P, HCH * P], BF, kind="ExternalInput").ap()
    biasqk = nc.dram_tensor("biasqk", [P, 16], F32, kind="ExternalInput").ap()
    biaspf = nc.dram_tensor("biaspf", [P, 16], F32, kind="ExternalInput").ap()
    fc1b = nc.dram_tensor("fc1b", [P, HCH], F32, kind="ExternalInput").ap()
    outT = nc.dram_tensor("outT", [D, TOK], F32, kind="ExternalOutput").ap()

    with tile.TileContext(nc) as tc:
        with (
            tc.tile_pool(name="consts", bufs=1) as consts,
            tc.tile_pool(name="bigs", bufs=1) as bigs,
            tc.tile_pool(name="work", bufs=3) as work,
            tc.tile_pool(name="wpool", bufs=2) as wpool,
            tc.tile_pool(name="kvz", bufs=2) as kvz,
            tc.tile_pool(name="pp", bufs=2) as ppool,
            tc.tile_pool(name="rows", bufs=3) as rows,
            tc.tile_pool(name="bc", bufs=2) as bcpool,
            tc.tile_pool(name="stg", bufs=3) as stg,
            tc.tile_pool(name="dram", bufs=1, space="DRAM") as dram,
        ):
            ones8 = consts.tile([P, 1], F8)
            nc.vector.memset(ones8[:].bitcast(U8), 0x38)  # e4m3 1.0
            ones_r = consts.tile([P, 1], F32R)
            nc.vector.memset(ones_r[:].bitcast(F32), 1.0)
            eps_row = consts.tile([1, 1], F32, tag="eps")
            nc.vector.memset(eps_row[:], EPS)
            negc_row = consts.tile([P, 1], F32, tag="negc")
            nc.vector.memset(negc_row[:], -CSH)
            onesv = consts.tile([P, 4, H], F8, tag="onesv")
            nc.vector.memset(onesv[:].bitcast(U8), 0x38)
            biasqk_sb = consts.tile([P, 16], F32, tag="bqk")
            biaspf_sb = consts.tile([P, 16], F32, tag="bpf")
            fc1b_sb = consts.tile([P, HCH], F32, tag="b1")
            nc.sync.dma_start(biasqk_sb[:], biasqk[:])
            nc.sync.dma_start(biaspf_sb[:], biaspf[:])
            nc.sync.dma_start(fc1b_sb[:], fc1b[:])

            kv_in = dram.tile([KV_SZ], F8, tag="kvin")
            kv_out = dram.tile([RANKS * KV_SZ], F8, tag="kvout")
            vk_in = kv_in[0:KV_K].rearrange("(f t) -> f t", t=TOK)
            vv_in = kv_in[KV_K:KV_SZ].rearrange(
                "(t v) -> t v", v=H * DHP
            )
            kv_or = kv_out[:].rearrange("(r x) -> r x", r=RANKS)
            # ones column of the V region: written up-front (no data deps)
            for tc_ in range(RANKS):
                ones_dst = vv_in[tc_ * P : (tc_ + 1) * P, :].rearrange(
                    "p (h c) -> p h c", c=DHP
                )[:, :, DH : DH + 1]
                nc.sync.dma_start(
                    ones_dst, onesv[:, tc_, :].rearrange("p (h c) -> p h c", c=1)
                )

            # ---- LN1 ---- (x8 first: stats depend on it; xr arrives later)
            x8_sb = bigs.tile([P, CH, TOK], F8, tag="x8")
            nc.sync.dma_start(
                x8_sb[:], x8T.rearrange("(ch p) t -> p ch t", p=P)
            )
            xr_sb = bigs.tile([P, CH, TOK], F32, tag="xr")
            nc.sync.dma_start(
                xr_sb[:], xT.rearrange("(ch p) t -> p ch t", p=P)
            )
            sq8 = bigs.tile([P, CH, TOK], F8, tag="h1")  # released before h1

            def ln_rows(psum_mu, psum_s2, name):
                mu = rows.tile([1, TOK], F32, tag="r", name=f"mu{name}")
                nc.vector.tensor_scalar_mul(mu[:], psum_mu[:], 1.0 / D)
                var = rows.tile([1, TOK], F32, tag="r", name=f"va{name}")
                nc.vector.tensor_tensor(var[:], mu[:], mu[:], OP.mult)
                ex2 = rows.tile([1, TOK], F32, tag="r", name=f"e2{name}")
                nc.vector.tensor_scalar_mul(ex2[:], psum_s2[:], 1.0 / D)
                nc.vector.tensor_sub(var[:], ex2[:], var[:])
                rstd = rows.tile([1, TOK], F32, tag="r", name=f"rs{name}")
                nc.scalar.activation(
                    out=rstd[:], in_=var[:], func=AF.Sqrt, bias=eps_row[:]
                )
                nc.vector.reciprocal(rstd[:], rstd[:])
                cpos = rows.tile([1, TOK], F32, tag="r", name=f"cp{name}")
                nc.vector.tensor_tensor(cpos[:], mu[:], rstd[:], OP.mult)
                rstd_b = bcpool.tile([P, TOK], F32, tag="bc", name=f"rb{name}")
                nc.gpsimd.partition_broadcast(rstd_b[:], rstd[:])
                c_b = bcpool.tile([P, TOK], F32, tag="bc", name=f"cb{name}")
                nc.gpsimd.partition_broadcast(c_b[:], cpos[:])
                return rstd_b, c_b

            with tc.tile_pool(name="ps_row1", bufs=2, space="PSUM") as prow:
                psum_mu = prow.tile([1, TOK], F32, tag="row")
                psum_s2 = prow.tile([1, TOK], F32, tag="row")
                for ch in range(CH):
                    eng = nc.vector if ch % 2 == 0 else nc.gpsimd
                    eng.tensor_tensor(
                        sq8[:, ch, :], x8_sb[:, ch, :], x8_sb[:, ch, :], OP.mult
                    )
                for ch in range(CH):
                    nc.tensor.matmul(
                        psum_mu[:],
                        ones8[:],
                        x8_sb[:, ch, :],
                        start=(ch == 0),
                        stop=(ch == CH - 1),
                    )
                for ch in range(CH):
                    nc.tensor.matmul(
                        psum_s2[:],
                        ones8[:],
                        sq8[:, ch, :],
                        start=(ch == 0),
                        stop=(ch == CH - 1),
                    )
                rstd1_b, c1_b = ln_rows(psum_mu, psum_s2, "1")

            h1 = bigs.tile([P, CH, TOK], F8, tag="h1")
            for ch in range(CH):
                eng = nc.vector if ch % 2 == 0 else nc.gpsimd
                t1 = work.tile([P, TOK], F32, tag="t1")
                eng.tensor_tensor(t1[:], xr_sb[:, ch, :], rstd1_b[:], OP.mult)
                eng.tensor_tensor(h1[:, ch, :], t1[:], c1_b[:], OP.subtract)

            # ---- QKV ----
            st2 = tc.tile_pool(name="ps_mm2", bufs=5, space="PSUM")
            ps_acc = st2.__enter__()

            def qkv_block(wt, i, m, consume):
                psum = ps_acc.tile([P, TOK], F32, tag="acc", name=f"ps_{m}")
                for j in range(CH // 2):
                    nc.tensor.matmul(
                        psum[:],
                        wt[:, i, 2 * j : 2 * j + 2, :],
                        h1[:, 2 * j : 2 * j + 2, :],
                        start=(j == 0),
                        stop=(j == CH // 2 - 1),
                        perf_mode=DR,
                    )
                consume(m, psum)

            def k_consume(m, psum):
                k8 = stg.tile([P, TOK], F8, tag="cp", name=f"k8_{m}")
                nc.scalar.activation(
                    out=k8[:],
                    in_=psum[:],
                    func=AF.Identity,
                    bias=biasqk_sb[:, 8 + m : 9 + m],
                    scale=0.5,
                )
                nc.sync.dma_start(vk_in[m * P : (m + 1) * P, :], k8[:])

            # K blocks (g1 folded into weights on host; cols D..2D of qkv_w)
            wk_t = []
            for g in range(2):
                wt = wpool.tile([P, 4, CH, P], F8, tag="wq", name=f"wk{g}")
                nc.sync.dma_start(
                    wt[:],
                    wqk8[8 + 4 * g : 12 + 4 * g].rearrange("i p (ch o) -> p i ch o", ch=CH),
                )
                wk_t.append(wt)
            for m in range(CH):
                qkv_block(wk_t[m // 4], m % 4, m, k_consume)

            # V (token-major): lhsT = h1 chunk-pair, rhs = wv columns
            for ph in range(2):
                wv_t = wpool.tile([P, CH, TOK], F8, tag="wq", name=f"wv{ph}")
                nc.sync.dma_start(
                    wv_t[:], wv8[ph].rearrange("p (ch v) -> p ch v", ch=CH)
                )
                for tt_ in range(TOK // P):
                    psum = ps_acc.tile([P, TOK], F32, tag="acc", name=f"pv{ph}_{tt_}")
                    for j in range(CH // 2):
                        nc.tensor.matmul(
                            psum[:],
                            h1[:, 2 * j : 2 * j + 2, tt_ * P : (tt_ + 1) * P],
                            wv_t[:, 2 * j : 2 * j + 2, :],
                            start=(j == 0),
                            stop=(j == CH // 2 - 1),
                            perf_mode=DR,
                        )
                    v8 = stg.tile([P, TOK], F8, tag="cp", name=f"v8_{ph}_{tt_}")
                    nc.scalar.activation(
                        out=v8[:], in_=psum[:], func=AF.Identity, scale=0.5
                    )
                    dst = vv_in[
                        tt_ * P : (tt_ + 1) * P,
                        ph * 8 * DHP : (ph + 1) * 8 * DHP,
                    ].rearrange("t (h c) -> t h c", c=DHP)[:, :, 0:DH]
                    nc.sync.dma_start(
                        dst, v8[:].rearrange("t (h d) -> t h d", d=DH)
                    )

            nc.gpsimd.collective_compute(
                "AllGather",
                OP.bypass,
                ins=[kv_in[:].opt()],
                outs=[kv_out[:].opt()],
                replica_groups=REPLICA_GROUPS,
            )

            # Q blocks -> SBUF (chunk 8 duplicates chunk 7 for the hp=7 rhs pair)
            qT = bigs.tile([P, CH + 1, TOK], F8, tag="qT")

            def q_consume(m, psum):
                nc.vector.tensor_scalar(
                    out=qT[:, m, :],
                    in0=psum[:],
                    scalar1=0.5,
                    scalar2=biasqk_sb[:, m : m + 1],
                    op0=OP.mult,
                    op1=OP.add,
                )

            for g in range(2):
                wt = wpool.tile([P, 4, CH, P], F8, tag="wq", name=f"wq{g}")
                nc.sync.dma_start(
                    wt[:],
                    wqk8[4 * g : 4 * g + 4].rearrange("i p (ch o) -> p i ch o", ch=CH),
                )
                for i in range(4):
                    qkv_block(wt, i, 4 * g + i, q_consume)
            nc.sync.dma_start(qT[:, CH, :], qT[:, CH - 1, :])
            st2.__exit__(None, None, None)

            # ---- attention ----
            # kp: [P(2 heads' d), 2, KC, P] fp8; slot0 = K data, slot1 = zeros
            kp_t = []
            for i in range(2):
                t = kvz.tile([P, 2, KC, P], F8, tag="kp", name=f"kp{i}")
                nc.vector.memset(t[:, 1, :, :].bitcast(U8), 0)
                kp_t.append(t)
            # vfull: [P(key-in-chunk), KC, 16*(64+1)] fp8, ones pre-gathered
            vfull = bigs.tile([P, KC, H * DHP], F8, tag="vfull")
            for r in range(RANKS):
                src = (
                    kv_or[r : r + 1, KV_K:KV_SZ]
                    .rearrange("o (t v) -> o t v", v=H * DHP)[0]
                    .rearrange("(tc p) v -> p tc v", p=P)
                )
                nc.sync.dma_start(vfull[:, 4 * r : 4 * r + 4, :], src)

            def load_kp(hp, t):
                src = (
                    kv_or[:, 0:KV_K]
                    .rearrange("r (f t) -> r f t", t=TOK)[
                        :, hp * P : (hp + 1) * P, :
                    ]
                    .rearrange("r p t -> p r t")
                )
                nc.sync.dma_start(t[:, 0, :, :], src)

            ctxT = bigs.tile([P, CH, TOK], F8, tag="x8")  # reuse x8 region
            groups = [(0, 3), (3, 3), (6, 3), (9, 3), (12, 2), (14, 2)]
            pairs_after = {1: [0, 1, 2], 2: [3], 3: [4, 5], 4: [6], 5: [7]}

            att_pools = (
                tc.tile_pool(name="ps_s", bufs=2, space="PSUM"),
                tc.tile_pool(name="ps_ctx", bufs=2, space="PSUM"),
            )
            ps_spool = att_pools[0].__enter__()
            ps_ctx = att_pools[1].__enter__()
            eidx = 0
            for hp in range(H // 2):
                kp = kp_t[hp % 2]
                load_kp(hp, kp)
                for hh in range(2):
                    h = 2 * hp + hh
                    half = slice(hh * DH, hh * DH + DH)
                    qpair = qT[half, hp : hp + 2, :]
                    pt = ppool.tile([P, KC, TOK], F8, tag="pt", name=f"pt{h}")
                    psum_c = ps_ctx.tile([DH + 1, TOK], F32, tag="ctx")
                    for gi, (kc0, nb) in enumerate(groups):
                        ps_s = ps_spool.tile([P, 3 * TOK], F32, tag="s")
                        for j in range(nb):
                            nc.tensor.matmul(
                                ps_s[:, j * TOK : (j + 1) * TOK],
                                kp[half, :, kc0 + j, :],
                                qpair,
                                start=True,
                                stop=True,
                                perf_mode=DR,
                            )
                        dst = pt[:, kc0 : kc0 + nb, :]
                        # alternate Act/DVE so consecutive groups of a head
                        # never serialize on one engine (GPSIMD can't read
                        # PSUM per the BIR verifier)
                        if (gi + h) % 2 == 0:
                            nc.scalar.activation(
                                out=dst,
                                in_=ps_s[:, : nb * TOK],
                                func=AF.Exp,
                                bias=negc_row[:],
                                scale=2.0**-11,
                            )
                        else:
                            nc.vector.tensor_scalar(
                                out=dst.bitcast(U8),
                                in0=ps_s[:, : nb * TOK],
                                scalar1=SA,
                                scalar2=SB,
                                op0=OP.mult,
                                op1=OP.add,
                            )
                        eidx += 1
                        for pj in pairs_after.get(gi, []):
                            nc.tensor.matmul(
                                psum_c[:],
                                vfull[
                                    :,
                                    2 * pj : 2 * pj + 2,
                                    h * DHP : h * DHP + DH + 1,
                                ],
                                pt[:, 2 * pj : 2 * pj + 2, :],
                                start=(pj == 0),
                                stop=(pj == KC // 2 - 1),
                                perf_mode=DR,
                            )
                    rrow = rows.tile([1, TOK], F32, tag="r", name=f"rr{h}")
                    nc.vector.reciprocal(rrow[:], psum_c[DH : DH + 1, :])
                    rb = bcpool.tile([DH, TOK], F32, tag="rb", name=f"rb{h}")
                    nc.gpsimd.partition_broadcast(rb[:], rrow[:])
                    nc.vector.tensor_tensor(
                        ctxT[half, hp, :], psum_c[0:DH, :], rb[:], OP.mult
                    )
            att_pools[1].__exit__(None, None, None)
            att_pools[0].__exit__(None, None, None)

            # ---- proj + residual ----
            st4 = tc.tile_pool(name="ps_mm4", bufs=5, space="PSUM")
            ps_mlp = st4.__enter__()
            x2 = bigs.tile([P, CH, TOK], F32R, tag="x2")
            for g in range(2):
                wt = wpool.tile([P, 4, CH, P], F8, tag="wq", name=f"wpj{g}")
                nc.sync.dma_start(
                    wt[:],
                    wp8[4 * g : 4 * g + 4].rearrange("i p (ch o) -> p i ch o", ch=CH),
                )
                for i in range(4):
                    m = 4 * g + i
                    psum = ps_mlp.tile([P, TOK], F32, tag="acc", name=f"pp_{m}")
                    for j in range(CH // 2):
                        nc.tensor.matmul(
                            psum[:],
                            wt[:, i, 2 * j : 2 * j + 2, :],
                            ctxT[:, 2 * j : 2 * j + 2, :],
                            start=(j == 0),
                            stop=(j == CH // 2 - 1),
                            perf_mode=DR,
                        )
                    attn_sb = stg.tile([P, TOK], F32, tag="stg", name=f"at_{m}")
                    nc.scalar.activation(
                        out=attn_sb[:],
                        in_=psum[:],
                        func=AF.Identity,
                        bias=biaspf_sb[:, m : m + 1],
                        scale=2.0**-9,
                    )
                    nc.gpsimd.tensor_tensor(
                        x2[:, m, :], attn_sb[:], xr_sb[:, m, :], OP.add
                    )  # f32r out: rounded on write for the LN2 stats matmul

            # ---- LN2 (fp32r stats on x2) ----
            with tc.tile_pool(name="ps_row2", bufs=2, space="PSUM") as prow:
                psum_mu2 = prow.tile([1, TOK], F32, tag="row")
                psum_s22 = prow.tile([1, TOK], F32, tag="row")
                for ch in range(CH):
                    nc.tensor.matmul(
                        psum_mu2[:],
                        ones_r[:],
                        x2[:, ch, :],
                        start=(ch == 0),
                        stop=(ch == CH - 1),
                    )
                    sq = work.tile([P, TOK], F32R, tag="t1", name=f"sq2_{ch}")
                    nc.gpsimd.tensor_tensor(
                        sq[:],
                        x2[:, ch, :].bitcast(F32),
                        x2[:, ch, :].bitcast(F32),
                        OP.mult,
                    )
                    nc.tensor.matmul(
                        psum_s22[:],
                        ones_r[:],
                        sq[:],
                        start=(ch == 0),
                        stop=(ch == CH - 1),
                    )
                rstd2_b, c2_b = ln_rows(psum_mu2, psum_s22, "2")

            xn = bigs.tile([P, CH, TOK], BF, tag="xn")
            for ch in range(CH):
                t1 = work.tile([P, TOK], F32, tag="t1", name=f"t2_{ch}")
                nc.gpsimd.tensor_tensor(
                    t1[:], x2[:, ch, :].bitcast(F32), rstd2_b[:], OP.mult
                )
                nc.vector.tensor_tensor(xn[:, ch, :], t1[:], c2_b[:], OP.subtract)

            # ---- MLP (bf16) ----
            gbf = bigs.tile([P, HCH, TOK], BF, tag="g")
            for g in range(CH):
                w1 = wpool.tile([P, 4, CH, P], BF, tag="w1", name=f"w1_{g}")
                nc.sync.dma_start(
                    w1[:], w1bf[g].rearrange("p (i ch o) -> p i ch o", i=4, ch=CH)
                )
                for i in range(4):
                    m = 4 * g + i
                    psum = ps_mlp.tile([P, TOK], F32, tag="acc", name=f"p1_{m}")
                    for ch in range(CH):
                        nc.tensor.matmul(
                            psum[:],
                            w1[:, i, ch, :],
                            xn[:, ch, :],
                            start=(ch == 0),
                            stop=(ch == CH - 1),
                        )
                    nc.scalar.activation(
                        out=gbf[:, m, :],
                        in_=psum[:],
                        func=AF.Gelu,
                        bias=fc1b_sb[:, m : m + 1],
                    )
            for m2 in range(CH):
                w2 = wpool.tile([P, HCH, P], BF, tag="w2", name=f"w2_{m2}")
                nc.sync.dma_start(
                    w2[:], w2bf[m2].rearrange("p (hc o) -> p hc o", hc=HCH)
                )
                psum = ps_mlp.tile([P, TOK], F32, tag="acc", name=f"p2_{m2}")
                for hc in range(HCH):
                    nc.tensor.matmul(
                        psum[:],
                        w2[:, hc, :],
                        gbf[:, hc, :],
                        start=(hc == 0),
                        stop=(hc == HCH - 1),
                    )
                o_sb = stg.tile([P, TOK], F32, tag="stg", name=f"o_{m2}")
                nc.scalar.activation(
                    out=o_sb[:],
                    in_=psum[:],
                    func=AF.Identity,
                    bias=biaspf_sb[:, 8 + m2 : 9 + m2],
                )
                o_f = stg.tile([P, TOK], F32, tag="of", bufs=2, name=f"of_{m2}")
                nc.vector.tensor_add(
                    out=o_f[:], in0=o_sb[:], in1=x2[:, m2, :].bitcast(F32)
                )
                nc.sync.dma_start(outT[m2 * P : (m2 + 1) * P, :], o_f[:])
            st4.__exit__(None, None, None)

    if do_compile:
        nc.compile()
    return nc


_CACHE = {}


def _get_program():
    if "nc" not in _CACHE:
        _CACHE["nc"] = build_program()
    return _CACHE["nc"]


def _prep_inputs(inputs):
    E4 = ml_dtypes.float8_e4m3
    x = np.asarray(inputs["x"], np.float32)
    g1 = np.asarray(inputs["ln1_g"], np.float32)
    b1 = np.asarray(inputs["ln1_b"], np.float32)
    g2 = np.asarray(inputs["ln2_g"], np.float32)
    b2 = np.asarray(inputs["ln2_b"], np.float32)
    qkv_w = np.asarray(inputs["qkv_w"], np.float32) * g1[None, :]
    proj_w = np.asarray(inputs["proj_w"], np.float32)
    fc1_w = np.asarray(inputs["fc1_w"], np.float32) * g2[None, :]
    fc2_w = np.asarray(inputs["fc2_w"], np.float32)

    qkv_bias = np.asarray(inputs["qkv_w"], np.float32) @ b1  # [3D]
    assert np.abs(qkv_bias[2 * D :]).max() == 0.0, "nonzero ln1_b v-bias unsupported"

    def wtile8(w, blocks):
        """w [O, D] -> [nb, P, CH*P] fp8 with [m, p, ch*128+o] = 32*w[m*128+o, ch*128+p]."""
        out = np.empty((len(blocks), P, CH * P), E4)
        for bi, m in enumerate(blocks):
            blk = w[m * P : (m + 1) * P, :] * 32.0  # [o 128, c 1024]
            out[bi] = (
                blk.reshape(P, CH, P).transpose(2, 1, 0).reshape(P, CH * P)
            ).astype(E4)
        return out

    def wtile_bf(w, nb, batch):
        """w [O, D] -> [nb//batch, P, batch*CH*P] bf16 tiles."""
        out = np.empty((nb // batch, P, batch * (w.shape[1] // P) * P), ml_dtypes.bfloat16)
        chn = w.shape[1] // P
        for g in range(nb // batch):
            t = np.empty((P, batch, chn, P), np.float32)
            for i in range(batch):
                m = g * batch + i
                blk = w[m * P : (m + 1) * P, :]  # [o, c]
                t[:, i] = blk.reshape(P, chn, P).transpose(2, 1, 0)
            out[g] = t.reshape(P, -1).astype(ml_dtypes.bfloat16)
        return out

    # V weights token-major: [ph, p, ch*512+vc] = 32*qkv_w'[2D+ph*512+vc, ch*128+p]
    wv = np.empty((2, P, CH * TOK), E4)
    for ph in range(2):
        blk = qkv_w[2 * D + ph * TOK : 2 * D + (ph + 1) * TOK, :] * 32.0  # [vc, c]
        wv[ph] = blk.reshape(TOK, CH, P).transpose(2, 1, 0).reshape(P, CH * TOK).astype(E4)

    bqk = np.zeros((P, 16), np.float32)
    bqk[:, 0:8] = _stripe(16.0 * qkv_bias[0:D])
    bqk[:, 8:16] = _stripe(16.0 * qkv_bias[D : 2 * D])
    bpf = np.zeros((P, 16), np.float32)
    bpf[:, 0:8] = _stripe(inputs["proj_b"])
    bpf[:, 8:16] = _stripe(inputs["fc2_b"])

    shared = {
        "wqk8": wtile8(qkv_w, list(range(16))),
        "wv8": wv,
        "wp8": wtile8(proj_w, list(range(CH))),
        "w1bf": wtile_bf(fc1_w, HCH, 4),
        "w2bf": wtile_bf(fc2_w, CH, 1),
        "biasqk": bqk,
        "biaspf": bpf,
        "fc1b": _stripe(
            np.asarray(inputs["fc1_b"], np.float32)
            + np.asarray(inputs["fc1_w"], np.float32) @ b2
        ),
    }
    in_maps = []
    for c in range(NCORES):
        b, blk = divmod(c, RANKS)
        xblk = x[b, blk * TOK : (blk + 1) * TOK, :]  # [TOK, D]
        xt = round_fp32r(np.ascontiguousarray(xblk.T))
        m = dict(shared)
        m["xT"] = xt
        m["x8T"] = xt.astype(E4)
        in_maps.append(m)
    return in_maps


def _assemble(results):
    out = np.empty((B, N, D), dtype=np.float32)
    for c in range(NCORES):
        b, blk = divmod(c, RANKS)
        out[b, blk * TOK : (blk + 1) * TOK, :] = results[c]["outT"].T
    return out


def run_device(inputs, **kwargs):
    nc = _get_program()
    in_maps = _prep_inputs(inputs)
    res = run_bass_kernel_spmd(nc, in_maps, core_ids=list(range(NCORES)), **kwargs)
    return _assemble(res.results), res


def kernel(**inputs) -> np.ndarray:
    out, _ = run_device(inputs)
    return out


# revision 8
# speedup vs baseline: 1.0032x; 1.0032x over previous
"""Trainium2 Bass kernel v2: fp8 DoubleRow attention + bf16 MLP.

Sharding: sequence-parallel over 8 cores (512 tokens each, batch = core//4).
One 4-rank AllGather carries fp8 K (feature-major) + fp8 V (token-major).

Precision plan (validated in numpy, max_rel ~1.0e-2 vs 2e-2 gate):
  - weights qkv/proj: e4m3 x32 host-scaled; fc1/fc2: bf16 (MLP dominates error)
  - h1/q/k/v/pt/ctx: e4m3 (q,k,v at sigma~16 via 0.5 consume scale)
  - softmax exp: constant shift C=4 (cancels in normalize); split between
    Act (native Exp -> fp8) and DVE (Schraudolph: psum*a+b -> uint8 whose
    bit pattern IS e4m3 2^x; floor-vs-round ambiguity is a constant factor
    that cancels in the softmax normalize)
  - x residual fp32r; LN stats via ones-matmul (fp8 DoubleRow for LN1 on
    host-provided x8, fp32r for LN2)
DoubleRow pair slots: chunk pairs for QKV/ctx/proj; (k, zeros) for scores
(d=64 contraction cannot pair; zero slot makes the 0.5 cyc/row rate legal).
"""
import sys

sys.path.insert(0, "/opt/trn_rl_repo")
import numpy as np
import ml_dtypes
import concourse.bass as bass
import concourse.mybir as mybir
import concourse.tile as tile
from concourse import bacc
from concourse.bass_utils import run_bass_kernel_spmd

B, N, D = 2, 2048, 1024
H, DH = 16, 64
HID = 4096
NCORES = 8
TOK = (B * N) // NCORES  # 512
EPS = 1e-5
SCALE = DH**-0.5
P = 128
CH = D // P  # 8
KC = N // P  # 16
HCH = HID // P  # 32
RANKS = 4
CSH = 4.0  # exp arg shift, cancels in softmax
LN2_ = float(np.log(2.0))
# Schraudolph uint8-as-e4m3: y = psum * SA + SB
SA = 8.0 * (2.0**-11) / LN2_
SB = 56.5 - 8.0 * CSH / LN2_

F32 = mybir.dt.float32
F32R = mybir.dt.float32r
F8 = mybir.dt.float8e4
BF = mybir.dt.bfloat16
U8 = mybir.dt.uint8
AF = mybir.ActivationFunctionType
OP = mybir.AluOpType
DR = mybir.MatmulPerfMode.DoubleRow

REPLICA_GROUPS = [[0, 1, 2, 3], [4, 5, 6, 7]]

KV_K = D * TOK  # bytes of K region (fp8 feature-major [1024, 512])
DHP = DH + 16  # per-head stride in V region: 64 v + 1 ones + 15 pad
# (dual-fp8 LdWeights requires 16B-aligned weight base addresses)
KV_V = TOK * (H * DHP)  # V region [512, 1280]
KV_SZ = KV_K + KV_V


def round_fp32r(x: np.ndarray) -> np.ndarray:
    u = np.ascontiguousarray(x, dtype=np.float32).view(np.uint32)
    u = (u + 0x7FF + ((u >> 12) & 1)) & np.uint32(0xFFFFF000)
    return u.view(np.float32)


def _stripe(v: np.ndarray) -> np.ndarray:
    """[M] -> [P, M//P] with col m, part p = v[m*128+p]."""
    return np.ascontiguousarray(np.asarray(v, np.float32).reshape(-1, P).T)


def build_program(do_compile=True):
    nc = bacc.Bacc("TRN2", target_bir_lowering=False, debug=False, num_devices=NCORES)

    xT = nc.dram_tensor("xT", [D, TOK], F32, kind="ExternalInput").ap()
    x8T = nc.dram_tensor("x8T", [D, TOK], F8, kind="ExternalInput").ap()
    # weight tiles, DMA-contiguous per partition
    wqk8 = nc.dram_tensor("wqk8", [16, P, CH * P], F8, kind="ExternalInput").ap()
    wv8 = nc.dram_tensor("wv8", [2, P, CH * TOK], F8, kind="ExternalInput").ap()
    wp8 = nc.dram_tensor("wp8", [CH, P, CH * P], F8, kind="ExternalInput").ap()
    w1bf = nc.dram_tensor("w1bf", [CH, P, 4 * CH * P], BF, kind="ExternalInput").ap()
    w2bf = nc.dram_tensor("w2bf", [CH, P, HCH * P], BF, kind="ExternalInput").ap()
    biasqk = nc.dram_tensor("biasqk", [P, 16], F32, kind="ExternalInput").ap()
    biaspf = nc.dram_tensor("biaspf", [P, 16], F32, kind="ExternalInput").ap()
    fc1b = nc.dram_tensor("fc1b", [P, HCH], F32, kind="ExternalInput").ap()
    outT = nc.dram_tensor("outT", [D, TOK], F32, kind="ExternalOutput").ap()

    with tile.TileContext(nc) as tc:
        with (
            tc.tile_pool(name="consts", bufs=1) as consts,
            tc.tile_pool(name="bigs", bufs=1) as bigs,
            tc.tile_pool(name="work", bufs=3) as work,
            tc.tile_pool(name="wpool", bufs=2) as wpool,
            tc.tile_pool(name="kvz", bufs=2) as kvz,
            tc.tile_pool(name="pp", bufs=2) as ppool,
            tc.tile_pool(name="rows", bufs=3) as rows,
            tc.tile_pool(name="bc", bufs=2) as bcpool,
            tc.tile_pool(name="stg", bufs=3) as stg,
            tc.tile_pool(name="dram", bufs=1, space="DRAM") as dram,
        ):
            ones8 = consts.tile([P, 1], F8)
            nc.vector.memset(ones8[:].bitcast(U8), 0x38)  # e4m3 1.0
            ones_r = consts.tile([P, 1], F32R)
            nc.vector.memset(ones_r[:].bitcast(F32), 1.0)
            eps_row = consts.tile([1, 1], F32, tag="eps")
            nc.vector.memset(eps_row[:], EPS)
            negc_row = consts.tile([P, 1], F32, tag="negc")
            nc.vector.memset(negc_row[:], -CSH)
            onesv = consts.tile([P, 4, H], F8, tag="onesv")
            nc.vector.memset(onesv[:].bitcast(U8), 0x38)
            biasqk_sb = consts.tile([P, 16], F32, tag="bqk")
            biaspf_sb = consts.tile([P, 16], F32, tag="bpf")
            fc1b_sb = consts.tile([P, HCH], F32, tag="b1")
            nc.sync.dma_start(biasqk_sb[:], biasqk[:])
            nc.sync.dma_start(biaspf_sb[:], biaspf[:])
            nc.sync.dma_start(fc1b_sb[:], fc1b[:])

            kv_in = dram.tile([KV_SZ], F8, tag="kvin")
            kv_out = dram.tile([RANKS * KV_SZ], F8, tag="kvout")
            vk_in = kv_in[0:KV_K].rearrange("(f t) -> f t", t=TOK)
            vv_in = kv_in[KV_K:KV_SZ].rearrange(
                "(t v) -> t v", v=H * DHP
            )
            kv_or = kv_out[:].rearrange("(r x) -> r x", r=RANKS)
            # ones column of the V region: written up-front (no data deps)
            for tc_ in range(RANKS):
                ones_dst = vv_in[tc_ * P : (tc_ + 1) * P, :].rearrange(
                    "p (h c) -> p h c", c=DHP
                )[:, :, DH : DH + 1]
                nc.sync.dma_start(
                    ones_dst, onesv[:, tc_, :].rearrange("p (h c) -> p h c", c=1)
                )

            # ---- LN1 ---- (x8 first: stats depend on it; xr arrives later)
            x8_sb = bigs.tile([P, CH, TOK], F8, tag="x8")
            nc.sync.dma_start(
                x8_sb[:], x8T.rearrange("(ch p) t -> p ch t", p=P)
            )
            xr_sb = bigs.tile([P, CH, TOK], F32, tag="xr")
            nc.sync.dma_start(
                xr_sb[:], xT.rearrange("(ch p) t -> p ch t", p=P)
            )
            sq8 = bigs.tile([P, CH, TOK], F8, tag="h1")  # released before h1

            def ln_rows(psum_mu, psum_s2, name):
                mu = rows.tile([1, TOK], F32, tag="r", name=f"mu{name}")
                nc.vector.tensor_scalar_mul(mu[:], psum_mu[:], 1.0 / D)
                var = rows.tile([1, TOK], F32, tag="r", name=f"va{name}")
                nc.vector.tensor_tensor(var[:], mu[:], mu[:], OP.mult)
                ex2 = rows.tile([1, TOK], F32, tag="r", name=f"e2{name}")
                nc.vector.tensor_scalar_mul(ex2[:], psum_s2[:], 1.0 / D)
                nc.vector.tensor_sub(var[:], ex2[:], var[:])
                rstd = rows.tile([1, TOK], F32, tag="r", name=f"rs{name}")
                nc.scalar.activation(
                    out=rstd[:], in_=var[:], func=AF.Sqrt, bias=eps_row[:]
                )
                nc.vector.reciprocal(rstd[:], rstd[:])
                cpos = rows.tile([1, TOK], F32, tag="r", name=f"cp{name}")
                nc.vector.tensor_tensor(cpos[:], mu[:], rstd[:], OP.mult)
                rstd_b = bcpool.tile([P, TOK], F32, tag="bc", name=f"rb{name}")
                nc.gpsimd.partition_broadcast(rstd_b[:], rstd[:])
                c_b = bcpool.tile([P, TOK], F32, tag="bc", name=f"cb{name}")
                nc.gpsimd.partition_broadcast(c_b[:], cpos[:])
                return rstd_b, c_b

            with tc.tile_pool(name="ps_row1", bufs=2, space="PSUM") as prow:
                psum_mu = prow.tile([1, TOK], F32, tag="row")
                psum_s2 = prow.tile([1, TOK], F32, tag="row")
                for ch in range(CH):
                    eng = nc.vector if ch % 2 == 0 else nc.gpsimd
                    eng.tensor_tensor(
                        sq8[:, ch, :], x8_sb[:, ch, :], x8_sb[:, ch, :], OP.mult
                    )
                for ch in range(CH):
                    nc.tensor.matmul(
                        psum_mu[:],
                        ones8[:],
                        x8_sb[:, ch, :],
                        start=(ch == 0),
                        stop=(ch == CH - 1),
                    )
                for ch in range(CH):
                    nc.tensor.matmul(
                        psum_s2[:],
                        ones8[:],
                        sq8[:, ch, :],
                        start=(ch == 0),
                        stop=(ch == CH - 1),
                    )
                rstd1_b, c1_b = ln_rows(psum_mu, psum_s2, "1")

            h1 = bigs.tile([P, CH, TOK], F8, tag="h1")
            for ch in range(CH):
                eng = nc.vector if ch % 2 == 0 else nc.gpsimd
                t1 = work.tile([P, TOK], F32, tag="t1")
                eng.tensor_tensor(t1[:], xr_sb[:, ch, :], rstd1_b[:], OP.mult)
                eng.tensor_tensor(h1[:, ch, :], t1[:], c1_b[:], OP.subtract)

            # ---- QKV ----
            st2 = tc.tile_pool(name="ps_mm2", bufs=5, space="PSUM")
            ps_acc = st2.__enter__()

            def qkv_block(wt, i, m, consume):
                psum = ps_acc.tile([P, TOK], F32, tag="acc", name=f"ps_{m}")
                for j in range(CH // 2):
                    nc.tensor.matmul(
                        psum[:],
                        wt[:, i, 2 * j : 2 * j + 2, :],
                        h1[:, 2 * j : 2 * j + 2, :],
                        start=(j == 0),
                        stop=(j == CH // 2 - 1),
                        perf_mode=DR,
                    )
                consume(m, psum)

            def k_consume(m, psum):
                k8 = stg.tile([P, TOK], F8, tag="cp", name=f"k8_{m}")
                nc.scalar.activation(
                    out=k8[:],
                    in_=psum[:],
                    func=AF.Identity,
                    bias=biasqk_sb[:, 8 + m : 9 + m],
                    scale=0.5,
                )
                nc.scalar.dma_start(vk_in[m * P : (m + 1) * P, :], k8[:])

            # K blocks (g1 folded into weights on host; cols D..2D of qkv_w)
            wk_t = []
            for g in range(2):
                wt = wpool.tile([P, 4, CH, P], F8, tag="wq", name=f"wk{g}")
                nc.sync.dma_start(
                    wt[:],
                    wqk8[8 + 4 * g : 12 + 4 * g].rearrange("i p (ch o) -> p i ch o", ch=CH),
                )
                wk_t.append(wt)
            for m in range(CH):
                qkv_block(wk_t[m // 4], m % 4, m, k_consume)

            # V (token-major): lhsT = h1 chunk-pair, rhs = wv columns
            for ph in range(2):
                wv_t = wpool.tile([P, CH, TOK], F8, tag="wq", name=f"wv{ph}")
                nc.sync.dma_start(
                    wv_t[:], wv8[ph].rearrange("p (ch v) -> p ch v", ch=CH)
                )
                for tt_ in range(TOK // P):
                    psum = ps_acc.tile([P, TOK], F32, tag="acc", name=f"pv{ph}_{tt_}")
                    for j in range(CH // 2):
                        nc.tensor.matmul(
                            psum[:],
                            h1[:, 2 * j : 2 * j + 2, tt_ * P : (tt_ + 1) * P],
                            wv_t[:, 2 * j : 2 * j + 2, :],
                            start=(j == 0),
                            stop=(j == CH // 2 - 1),
                            perf_mode=DR,
                        )
                    v8 = stg.tile([P, TOK], F8, tag="cp", name=f"v8_{ph}_{tt_}")
                    nc.scalar.activation(
                        out=v8[:], in_=psum[:], func=AF.Identity, scale=0.5
                    )
                    dst = vv_in[
                        tt_ * P : (tt_ + 1) * P,
                        ph * 8 * DHP : (ph + 1) * 8 * DHP,
                    ].rearrange("t (h c) -> t h c", c=DHP)[:, :, 0:DH]
                    nc.scalar.dma_start(
                        dst, v8[:].rearrange("t (h d) -> t h d", d=DH)
                    )

            nc.gpsimd.collective_compute(
                "AllGather",
                OP.bypass,
                ins=[kv_in[:].opt()],
                outs=[kv_out[:].opt()],
                replica_groups=REPLICA_GROUPS,
            )

            # Q blocks -> SBUF (chunk 8 duplicates chunk 7 for the hp=7 rhs pair)
            qT = bigs.tile([P, CH + 1, TOK], F8, tag="qT")

            def q_consume(m, psum):
                nc.vector.tensor_scalar(
                    out=qT[:, m, :],
                    in0=psum[:],
                    scalar1=0.5,
                    scalar2=biasqk_sb[:, m : m + 1],
                    op0=OP.mult,
                    op1=OP.add,
                )

            for g in range(2):
                wt = wpool.tile([P, 4, CH, P], F8, tag="wq", name=f"wq{g}")
                nc.sync.dma_start(
                    wt[:],
                    wqk8[4 * g : 4 * g + 4].rearrange("i p (ch o) -> p i ch o", ch=CH),
                )
                for i in range(4):
                    qkv_block(wt, i, 4 * g + i, q_consume)
            nc.sync.dma_start(qT[:, CH, :], qT[:, CH - 1, :])
            st2.__exit__(None, None, None)

            # ---- attention ----
            # kp: [P(2 heads' d), 2, KC, P] fp8; slot0 = K data, slot1 = zeros
            kp_t = []
            for i in range(2):
                t = kvz.tile([P, 2, KC, P], F8, tag="kp", name=f"kp{i}")
                nc.vector.memset(t[:, 1, :, :].bitcast(U8), 0)
                kp_t.append(t)
            # vfull: [P(key-in-chunk), KC, 16*(64+1)] fp8, ones pre-gathered
            vfull = bigs.tile([P, KC, H * DHP], F8, tag="vfull")
            for r in range(RANKS):
                src = (
                    kv_or[r : r + 1, KV_K:KV_SZ]
                    .rearrange("o (t v) -> o t v", v=H * DHP)[0]
                    .rearrange("(tc p) v -> p tc v", p=P)
                )
                nc.sync.dma_start(vfull[:, 4 * r : 4 * r + 4, :], src)

            def load_kp(hp, t):
                src = (
                    kv_or[:, 0:KV_K]
                    .rearrange("r (f t) -> r f t", t=TOK)[
                        :, hp * P : (hp + 1) * P, :
                    ]
                    .rearrange("r p t -> p r t")
                )
                nc.sync.dma_start(t[:, 0, :, :], src)

            ctxT = bigs.tile([P, CH, TOK], F8, tag="x8")  # reuse x8 region
            groups = [(0, 3), (3, 3), (6, 3), (9, 3), (12, 2), (14, 2)]
            pairs_after = {1: [0, 1, 2], 2: [3], 3: [4, 5], 4: [6], 5: [7]}

            att_pools = (
                tc.tile_pool(name="ps_s", bufs=2, space="PSUM"),
                tc.tile_pool(name="ps_ctx", bufs=2, space="PSUM"),
            )
            ps_spool = att_pools[0].__enter__()
            ps_ctx = att_pools[1].__enter__()
            eidx = 0
            for hp in range(H // 2):
                kp = kp_t[hp % 2]
                load_kp(hp, kp)
                for hh in range(2):
                    h = 2 * hp + hh
                    half = slice(hh * DH, hh * DH + DH)
                    qpair = qT[half, hp : hp + 2, :]
                    pt = ppool.tile([P, KC, TOK], F8, tag="pt", name=f"pt{h}")
                    psum_c = ps_ctx.tile([DH + 1, TOK], F32, tag="ctx")
                    for gi, (kc0, nb) in enumerate(groups):
                        ps_s = ps_spool.tile([P, 3 * TOK], F32, tag="s")
                        for j in range(nb):
                            nc.tensor.matmul(
                                ps_s[:, j * TOK : (j + 1) * TOK],
                                kp[half, :, kc0 + j, :],
                                qpair,
                                start=True,
                                stop=True,
                                perf_mode=DR,
                            )
                        dst = pt[:, kc0 : kc0 + nb, :]
                        # alternate Act/DVE so consecutive groups of a head
                        # never serialize on one engine (GPSIMD can't read
                        # PSUM per the BIR verifier); 4:2 toward Act since
                        # DVE also carries reciprocal + normalize
                        if (gi + h) % 2 == 0:
                            nc.scalar.activation(
                                out=dst,
                                in_=ps_s[:, : nb * TOK],
                                func=AF.Exp,
                                bias=negc_row[:],
                                scale=2.0**-11,
                            )
                        else:
                            nc.vector.tensor_scalar(
                                out=dst.bitcast(U8),
                                in0=ps_s[:, : nb * TOK],
                                scalar1=SA,
                                scalar2=SB,
                                op0=OP.mult,
                                op1=OP.add,
                            )
                        eidx += 1
                        for pj in pairs_after.get(gi, []):
                            nc.tensor.matmul(
                                psum_c[:],
                                vfull[
                                    :,
                                    2 * pj : 2 * pj + 2,
                                    h * DHP : h * DHP + DH + 1,
                                ],
                                pt[:, 2 * pj : 2 * pj + 2, :],
                                start=(pj == 0),
                                stop=(pj == KC // 2 - 1),
                                perf_mode=DR,
                            )
                    rrow = rows.tile([1, TOK], F32, tag="r", name=f"rr{h}")
                    nc.vector.reciprocal(rrow[:], psum_c[DH : DH + 1, :])
                    rb = bcpool.tile([DH, TOK], F32, tag="rb", name=f"rb{h}")
                    nc.gpsimd.partition_broadcast(rb[:], rrow[:])
                    nc.vector.tensor_tensor(
                        ctxT[half, hp, :], psum_c[0:DH, :], rb[:], OP.mult
                    )
            att_pools[1].__exit__(None, None, None)
            att_pools[0].__exit__(None, None, None)

            # ---- proj + residual ----
            st4 = tc.tile_pool(name="ps_mm4", bufs=5, space="PSUM")
            ps_mlp = st4.__enter__()
            x2 = bigs.tile([P, CH, TOK], F32R, tag="x2")
            for g in range(2):
                wt = wpool.tile([P, 4, CH, P], F8, tag="wq", name=f"wpj{g}")
                nc.sync.dma_start(
                    wt[:],
                    wp8[4 * g : 4 * g + 4].rearrange("i p (ch o) -> p i ch o", ch=CH),
                )
                for i in range(4):
                    m = 4 * g + i
                    psum = ps_mlp.tile([P, TOK], F32, tag="acc", name=f"pp_{m}")
                    for j in range(CH // 2):
                        nc.tensor.matmul(
                            psum[:],
                            wt[:, i, 2 * j : 2 * j + 2, :],
                            ctxT[:, 2 * j : 2 * j + 2, :],
                            start=(j == 0),
                            stop=(j == CH // 2 - 1),
                            perf_mode=DR,
                        )
                    attn_sb = stg.tile([P, TOK], F32, tag="stg", name=f"at_{m}")
                    nc.scalar.activation(
                        out=attn_sb[:],
                        in_=psum[:],
                        func=AF.Identity,
                        bias=biaspf_sb[:, m : m + 1],
                        scale=2.0**-9,
                    )
                    nc.gpsimd.tensor_tensor(
                        x2[:, m, :], attn_sb[:], xr_sb[:, m, :], OP.add
                    )  # f32r out: rounded on write for the LN2 stats matmul

            # ---- LN2 (fp32r stats on x2) ----
            with tc.tile_pool(name="ps_row2", bufs=2, space="PSUM") as prow:
                psum_mu2 = prow.tile([1, TOK], F32, tag="row")
                psum_s22 = prow.tile([1, TOK], F32, tag="row")
                for ch in range(CH):
                    nc.tensor.matmul(
                        psum_mu2[:],
                        ones_r[:],
                        x2[:, ch, :],
                        start=(ch == 0),
                        stop=(ch == CH - 1),
                    )
                    sq = work.tile([P, TOK], F32R, tag="t1", name=f"sq2_{ch}")
                    nc.gpsimd.tensor_tensor(
                        sq[:],
                        x2[:, ch, :].bitcast(F32),
                        x2[:, ch, :].bitcast(F32),
                        OP.mult,
                    )
                    nc.tensor.matmul(
                        psum_s22[:],
                        ones_r[:],
                        sq[:],
                        start=(ch == 0),
                        stop=(ch == CH - 1),
                    )
                rstd2_b, c2_b = ln_rows(psum_mu2, psum_s22, "2")

            xn = bigs.tile([P, CH, TOK], BF, tag="xn")
            for ch in range(CH):
                t1 = work.tile([P, TOK], F32, tag="t1", name=f"t2_{ch}")
                nc.gpsimd.tensor_tensor(
                    t1[:], x2[:, ch, :].bitcast(F32), rstd2_b[:], OP.mult
                )
                nc.vector.tensor_tensor(xn[:, ch, :], t1[:], c2_b[:], OP.subtract)

            # ---- MLP (bf16) ----
            gbf = bigs.tile([P, HCH, TOK], BF, tag="g")
            for g in range(CH):
                w1 = wpool.tile([P, 4, CH, P], BF, tag="w1", name=f"w1_{g}")
                nc.sync.dma_start(
                    w1[:], w1bf[g].rearrange("p (i ch o) -> p i ch o", i=4, ch=CH)
                )
                for i in range(4):
                    m = 4 * g + i
                    psum = ps_mlp.tile([P, TOK], F32, tag="acc", name=f"p1_{m}")
                    for ch in range(CH):
                        nc.tensor.matmul(
                            psum[:],
                            w1[:, i, ch, :],
                            xn[:, ch, :],
                            start=(ch == 0),
                            stop=(ch == CH - 1),
                        )
                    nc.scalar.activation(
                        out=gbf[:, m, :],
                        in_=psum[:],
                        func=AF.Gelu,
                        bias=fc1b_sb[:, m : m + 1],
                    )
            for m2 in range(CH):
                w2 = wpool.tile([P, HCH, P], BF, tag="w2", name=f"w2_{m2}")
                nc.sync.dma_start(
                    w2[:], w2bf[m2].rearrange("p (hc o) -> p hc o", hc=HCH)
                )
                psum = ps_mlp.tile([P, TOK], F32, tag="acc", name=f"p2_{m2}")
                for hc in range(HCH):
                    nc.tensor.matmul(
                        psum[:],
                        w2[:, hc, :],
                        gbf[:, hc, :],
                        start=(hc == 0),
                        stop=(hc == HCH - 1),
                    )
                o_sb = stg.tile([P, TOK], F32, tag="stg", name=f"o_{m2}")
                nc.scalar.activation(
                    out=o_sb[:],
                    in_=psum[:],
                    func=AF.Identity,
                    bias=biaspf_sb[:, 8 + m2 : 9 + m2],
                )
                o_f = stg.tile([P, TOK], F32, tag="of", bufs=2, name=f"of_{m2}")
                nc.vector.tensor_add(
                    out=o_f[:], in0=o_sb[:], in1=x2[:, m2, :].bitcast(F32)
                )
                nc.sync.dma_start(outT[m2 * P : (m2 + 1) * P, :], o_f[:])
            st4.__exit__(None, None, None)

    if do_compile:
        nc.compile()
    return nc


_CACHE = {}


def _get_program():
    if "nc" not in _CACHE:
        _CACHE["nc"] = build_program()
    return _CACHE["nc"]


def _prep_inputs(inputs):
    E4 = ml_dtypes.float8_e4m3
    x = np.asarray(inputs["x"], np.float32)
    g1 = np.asarray(inputs["ln1_g"], np.float32)
    b1 = np.asarray(inputs["ln1_b"], np.float32)
    g2 = np.asarray(inputs["ln2_g"], np.float32)
    b2 = np.asarray(inputs["ln2_b"], np.float32)
    qkv_w = np.asarray(inputs["qkv_w"], np.float32) * g1[None, :]
    proj_w = np.asarray(inputs["proj_w"], np.float32)
    fc1_w = np.asarray(inputs["fc1_w"], np.float32) * g2[None, :]
    fc2_w = np.asarray(inputs["fc2_w"], np.float32)

    qkv_bias = np.asarray(inputs["qkv_w"], np.float32) @ b1  # [3D]
    assert np.abs(qkv_bias[2 * D :]).max() == 0.0, "nonzero ln1_b v-bias unsupported"

    def wtile8(w, blocks):
        """w [O, D] -> [nb, P, CH*P] fp8 with [m, p, ch*128+o] = 32*w[m*128+o, ch*128+p]."""
        out = np.empty((len(blocks), P, CH * P), E4)
        for bi, m in enumerate(blocks):
            blk = w[m * P : (m + 1) * P, :] * 32.0  # [o 128, c 1024]
            out[bi] = (
                blk.reshape(P, CH, P).transpose(2, 1, 0).reshape(P, CH * P)
            ).astype(E4)
        return out

    def wtile_bf(w, nb, batch):
        """w [O, D] -> [nb//batch, P, batch*CH*P] bf16 tiles."""
        out = np.empty((nb // batch, P, batch * (w.shape[1] // P) * P), ml_dtypes.bfloat16)
        chn = w.shape[1] // P
        for g in range(nb // batch):
            t = np.empty((P, batch, chn, P), np.float32)
            for i in range(batch):
                m = g * batch + i
                blk = w[m * P : (m + 1) * P, :]  # [o, c]
                t[:, i] = blk.reshape(P, chn, P).transpose(2, 1, 0)
            out[g] = t.reshape(P, -1).astype(ml_dtypes.bfloat16)
        return out

    # V weights token-major: [ph, p, ch*512+vc] = 32*qkv_w'[2D+ph*512+vc, ch*128+p]
    wv = np.empty((2, P, CH * TOK), E4)
    for ph in range(2):
        blk = qkv_w[2 * D + ph * TOK : 2 * D + (ph + 1) * TOK, :] * 32.0  # [vc, c]
        wv[ph] = blk.reshape(TOK, CH, P).transpose(2, 1, 0).reshape(P, CH * TOK).astype(E4)

    bqk = np.zeros((P, 16), np.float32)
    bqk[:, 0:8] = _stripe(16.0 * qkv_bias[0:D])
    bqk[:, 8:16] = _stripe(16.0 * qkv_bias[D : 2 * D])
    bpf = np.zeros((P, 16), np.float32)
    bpf[:, 0:8] = _stripe(inputs["proj_b"])
    bpf[:, 8:16] = _stripe(inputs["fc2_b"])

    shared = {
        "wqk8": wtile8(qkv_w, list(range(16))),
        "wv8": wv,
        "wp8": wtile8(proj_w, list(range(CH))),
        "w1bf": wtile_bf(fc1_w, HCH, 4),
        "w2bf": wtile_bf(fc2_w, CH, 1),
        "biasqk": bqk,
        "biaspf": bpf,
        "fc1b": _stripe(
            np.asarray(inputs["fc1_b"], np.float32)
            + np.asarray(inputs["fc1_w"], np.float32) @ b2
        ),
    }
    in_maps = []
    for c in range(NCORES):
        b, blk = divmod(c, RANKS)
        xblk = x[b, blk * TOK : (blk + 1) * TOK, :]  # [TOK, D]
        xt = round_fp32r(np.ascontiguousarray(xblk.T))
        m = dict(shared)
        m["xT"] = xt
        m["x8T"] = xt.astype(E4)
        in_maps.append(m)
    return in_maps


def _assemble(results):
    out = np.empty((B, N, D), dtype=np.float32)
    for c in range(NCORES):
        b, blk = divmod(c, RANKS)
        out[b, blk * TOK : (blk + 1) * TOK, :] = results[c]["outT"].T
    return out


def run_device(inputs, **kwargs):
    nc = _get_program()
    in_maps = _prep_inputs(inputs)
    res = run_bass_kernel_spmd(nc, in_maps, core_ids=list(range(NCORES)), **kwargs)
    return _assemble(res.results), res


def kernel(**inputs) -> np.ndarray:
    out, _ = run_device(inputs)
    return out


# revision 9
# speedup vs baseline: 1.0033x; 1.0001x over previous
"""Trainium2 Bass kernel v2: fp8 DoubleRow attention + bf16 MLP.

Sharding: sequence-parallel over 8 cores (512 tokens each, batch = core//4).
One 4-rank AllGather carries fp8 K (feature-major) + fp8 V (token-major).

Precision plan (validated in numpy, max_rel ~1.0e-2 vs 2e-2 gate):
  - weights qkv/proj: e4m3 x32 host-scaled; fc1/fc2: bf16 (MLP dominates error)
  - h1/q/k/v/pt/ctx: e4m3 (q,k,v at sigma~16 via 0.5 consume scale)
  - softmax exp: constant shift C=4 (cancels in normalize); split between
    Act (native Exp -> fp8) and DVE (Schraudolph: psum*a+b -> uint8 whose
    bit pattern IS e4m3 2^x; floor-vs-round ambiguity is a constant factor
    that cancels in the softmax normalize)
  - x residual fp32r; LN stats via ones-matmul (fp8 DoubleRow for LN1 on
    host-provided x8, fp32r for LN2)
DoubleRow pair slots: chunk pairs for QKV/ctx/proj; (k, zeros) for scores
(d=64 contraction cannot pair; zero slot makes the 0.5 cyc/row rate legal).
"""
import sys

sys.path.insert(0, "/opt/trn_rl_repo")
import numpy as np
import ml_dtypes
import concourse.bass as bass
import concourse.mybir as mybir
import concourse.tile as tile
from concourse import bacc
from concourse.bass_utils import run_bass_kernel_spmd

B, N, D = 2, 2048, 1024
H, DH = 16, 64
HID = 4096
NCORES = 8
TOK = (B * N) // NCORES  # 512
EPS = 1e-5
SCALE = DH**-0.5
P = 128
CH = D // P  # 8
KC = N // P  # 16
HCH = HID // P  # 32
RANKS = 4
CSH = 4.0  # exp arg shift, cancels in softmax
LN2_ = float(np.log(2.0))
# Schraudolph uint8-as-e4m3: y = psum * SA + SB
SA = 8.0 * (2.0**-11) / LN2_
SB = 56.5 - 8.0 * CSH / LN2_

F32 = mybir.dt.float32
F32R = mybir.dt.float32r
F8 = mybir.dt.float8e4
BF = mybir.dt.bfloat16
U8 = mybir.dt.uint8
AF = mybir.ActivationFunctionType
OP = mybir.AluOpType
DR = mybir.MatmulPerfMode.DoubleRow

REPLICA_GROUPS = [[0, 1, 2, 3], [4, 5, 6, 7]]

KV_K = D * TOK  # bytes of K region (fp8 feature-major [1024, 512])
DHP = DH + 16  # per-head stride in V region: 64 v + 1 ones + 15 pad
# (dual-fp8 LdWeights requires 16B-aligned weight base addresses)
KV_V = TOK * (H * DHP)  # V region [512, 1280]
KV_SZ = KV_K + KV_V


def round_fp32r(x: np.ndarray) -> np.ndarray:
    u = np.ascontiguousarray(x, dtype=np.float32).view(np.uint32)
    u = (u + 0x7FF + ((u >> 12) & 1)) & np.uint32(0xFFFFF000)
    return u.view(np.float32)


def _stripe(v: np.ndarray) -> np.ndarray:
    """[M] -> [P, M//P] with col m, part p = v[m*128+p]."""
    return np.ascontiguousarray(np.asarray(v, np.float32).reshape(-1, P).T)


def build_program(do_compile=True):
    nc = bacc.Bacc("TRN2", target_bir_lowering=False, debug=False, num_devices=NCORES)

    xT = nc.dram_tensor("xT", [D, TOK], F32, kind="ExternalInput").ap()
    x8T = nc.dram_tensor("x8T", [D, TOK], F8, kind="ExternalInput").ap()
    # weight tiles, DMA-contiguous per partition
    wqk8 = nc.dram_tensor("wqk8", [16, P, CH * P], F8, kind="ExternalInput").ap()
    wv8 = nc.dram_tensor("wv8", [2, P, CH * TOK], F8, kind="ExternalInput").ap()
    wp8 = nc.dram_tensor("wp8", [CH, P, CH * P], F8, kind="ExternalInput").ap()
    w1bf = nc.dram_tensor("w1bf", [CH, P, 4 * CH * P], BF, kind="ExternalInput").ap()
    w2bf = nc.dram_tensor("w2bf", [CH, P, HCH * P], BF, kind="ExternalInput").ap()
    biasqk = nc.dram_tensor("biasqk", [P, 16], F32, kind="ExternalInput").ap()
    biaspf = nc.dram_tensor("biaspf", [P, 16], F32, kind="ExternalInput").ap()
    fc1b = nc.dram_tensor("fc1b", [P, HCH], F32, kind="ExternalInput").ap()
    outT = nc.dram_tensor("outT", [D, TOK], F32, kind="ExternalOutput").ap()

    with tile.TileContext(nc) as tc:
        with (
            tc.tile_pool(name="consts", bufs=1) as consts,
            tc.tile_pool(name="bigs", bufs=1) as bigs,
            tc.tile_pool(name="work", bufs=3) as work,
            tc.tile_pool(name="wpool", bufs=2) as wpool,
            tc.tile_pool(name="kvz", bufs=2) as kvz,
            tc.tile_pool(name="pp", bufs=2) as ppool,
            tc.tile_pool(name="rows", bufs=3) as rows,
            tc.tile_pool(name="bc", bufs=2) as bcpool,
            tc.tile_pool(name="stg", bufs=3) as stg,
            tc.tile_pool(name="dram", bufs=1, space="DRAM") as dram,
        ):
            ones8 = consts.tile([P, 1], F8)
            nc.vector.memset(ones8[:].bitcast(U8), 0x38)  # e4m3 1.0
            ones_r = consts.tile([P, 1], F32R)
            nc.vector.memset(ones_r[:].bitcast(F32), 1.0)
            eps_row = consts.tile([1, 1], F32, tag="eps")
            nc.vector.memset(eps_row[:], EPS)
            negc_row = consts.tile([P, 1], F32, tag="negc")
            nc.vector.memset(negc_row[:], -CSH)
            onesv = consts.tile([P, 4, H], F8, tag="onesv")
            nc.vector.memset(onesv[:].bitcast(U8), 0x38)
            biasqk_sb = consts.tile([P, 16], F32, tag="bqk")
            biaspf_sb = consts.tile([P, 16], F32, tag="bpf")
            fc1b_sb = consts.tile([P, HCH], F32, tag="b1")
            nc.sync.dma_start(biasqk_sb[:], biasqk[:])
            nc.sync.dma_start(biaspf_sb[:], biaspf[:])
            nc.sync.dma_start(fc1b_sb[:], fc1b[:])

            kv_in = dram.tile([KV_SZ], F8, tag="kvin")
            kv_out = dram.tile([RANKS * KV_SZ], F8, tag="kvout")
            vk_in = kv_in[0:KV_K].rearrange("(f t) -> f t", t=TOK)
            vv_in = kv_in[KV_K:KV_SZ].rearrange(
                "(t v) -> t v", v=H * DHP
            )
            kv_or = kv_out[:].rearrange("(r x) -> r x", r=RANKS)
            # ones column of the V region: written up-front (no data deps)
            for tc_ in range(RANKS):
                ones_dst = vv_in[tc_ * P : (tc_ + 1) * P, :].rearrange(
                    "p (h c) -> p h c", c=DHP
                )[:, :, DH : DH + 1]
                nc.sync.dma_start(
                    ones_dst, onesv[:, tc_, :].rearrange("p (h c) -> p h c", c=1)
                )

            # ---- LN1 ---- (x8 first: stats depend on it; xr arrives later)
            x8_sb = bigs.tile([P, CH, TOK], F8, tag="x8")
            nc.sync.dma_start(
                x8_sb[:], x8T.rearrange("(ch p) t -> p ch t", p=P)
            )
            xr_sb = bigs.tile([P, CH, TOK], F32, tag="xr")
            nc.sync.dma_start(
                xr_sb[:], xT.rearrange("(ch p) t -> p ch t", p=P)
            )
            sq8 = bigs.tile([P, CH, TOK], F8, tag="h1")  # released before h1

            def ln_rows(psum_mu, psum_s2, name):
                mu = rows.tile([1, TOK], F32, tag="r", name=f"mu{name}")
                nc.vector.tensor_scalar_mul(mu[:], psum_mu[:], 1.0 / D)
                var = rows.tile([1, TOK], F32, tag="r", name=f"va{name}")
                nc.vector.tensor_tensor(var[:], mu[:], mu[:], OP.mult)
                ex2 = rows.tile([1, TOK], F32, tag="r", name=f"e2{name}")
                nc.vector.tensor_scalar_mul(ex2[:], psum_s2[:], 1.0 / D)
                nc.vector.tensor_sub(var[:], ex2[:], var[:])
                rstd = rows.tile([1, TOK], F32, tag="r", name=f"rs{name}")
                nc.scalar.activation(
                    out=rstd[:], in_=var[:], func=AF.Sqrt, bias=eps_row[:]
                )
                nc.vector.reciprocal(rstd[:], rstd[:])
                cpos = rows.tile([1, TOK], F32, tag="r", name=f"cp{name}")
                nc.vector.tensor_tensor(cpos[:], mu[:], rstd[:], OP.mult)
                rstd_b = bcpool.tile([P, TOK], F32, tag="bc", name=f"rb{name}")
                nc.gpsimd.partition_broadcast(rstd_b[:], rstd[:])
                c_b = bcpool.tile([P, TOK], F32, tag="bc", name=f"cb{name}")
                nc.gpsimd.partition_broadcast(c_b[:], cpos[:])
                return rstd_b, c_b

            with tc.tile_pool(name="ps_row1", bufs=2, space="PSUM") as prow:
                psum_mu = prow.tile([1, TOK], F32, tag="row")
                psum_s2 = prow.tile([1, TOK], F32, tag="row")
                for ch in range(CH):
                    eng = nc.vector if ch % 2 == 0 else nc.gpsimd
                    eng.tensor_tensor(
                        sq8[:, ch, :], x8_sb[:, ch, :], x8_sb[:, ch, :], OP.mult
                    )
                for ch in range(CH):
                    nc.tensor.matmul(
                        psum_mu[:],
                        ones8[:],
                        x8_sb[:, ch, :],
                        start=(ch == 0),
                        stop=(ch == CH - 1),
                    )
                for ch in range(CH):
                    nc.tensor.matmul(
                        psum_s2[:],
                        ones8[:],
                        sq8[:, ch, :],
                        start=(ch == 0),
                        stop=(ch == CH - 1),
                    )
                rstd1_b, c1_b = ln_rows(psum_mu, psum_s2, "1")

            h1 = bigs.tile([P, CH, TOK], F8, tag="h1")
            for ch in range(CH):
                eng = nc.vector if ch % 2 == 0 else nc.gpsimd
                t1 = work.tile([P, TOK], F32, tag="t1")
                eng.tensor_tensor(t1[:], xr_sb[:, ch, :], rstd1_b[:], OP.mult)
                eng.tensor_tensor(h1[:, ch, :], t1[:], c1_b[:], OP.subtract)

            # ---- QKV ----
            st2 = tc.tile_pool(name="ps_mm2", bufs=5, space="PSUM")
            ps_acc = st2.__enter__()

            def qkv_block(wt, i, m, consume):
                psum = ps_acc.tile([P, TOK], F32, tag="acc", name=f"ps_{m}")
                for j in range(CH // 2):
                    nc.tensor.matmul(
                        psum[:],
                        wt[:, i, 2 * j : 2 * j + 2, :],
                        h1[:, 2 * j : 2 * j + 2, :],
                        start=(j == 0),
                        stop=(j == CH // 2 - 1),
                        perf_mode=DR,
                    )
                consume(m, psum)

            def k_consume(m, psum):
                k8 = stg.tile([P, TOK], F8, tag="cp", name=f"k8_{m}")
                nc.scalar.activation(
                    out=k8[:],
                    in_=psum[:],
                    func=AF.Identity,
                    bias=biasqk_sb[:, 8 + m : 9 + m],
                    scale=0.5,
                )
                nc.scalar.dma_start(vk_in[m * P : (m + 1) * P, :], k8[:])

            # K blocks (g1 folded into weights on host; cols D..2D of qkv_w)
            wk_t = []
            for g in range(2):
                wt = wpool.tile([P, 4, CH, P], F8, tag="wq", name=f"wk{g}")
                nc.sync.dma_start(
                    wt[:],
                    wqk8[8 + 4 * g : 12 + 4 * g].rearrange("i p (ch o) -> p i ch o", ch=CH),
                )
                wk_t.append(wt)
            for m in range(CH):
                qkv_block(wk_t[m // 4], m % 4, m, k_consume)

            # V (token-major): lhsT = h1 chunk-pair, rhs = wv columns
            for ph in range(2):
                wv_t = wpool.tile([P, CH, TOK], F8, tag="wq", name=f"wv{ph}")
                nc.sync.dma_start(
                    wv_t[:], wv8[ph].rearrange("p (ch v) -> p ch v", ch=CH)
                )
                for tt_ in range(TOK // P):
                    psum = ps_acc.tile([P, TOK], F32, tag="acc", name=f"pv{ph}_{tt_}")
                    for j in range(CH // 2):
                        nc.tensor.matmul(
                            psum[:],
                            h1[:, 2 * j : 2 * j + 2, tt_ * P : (tt_ + 1) * P],
                            wv_t[:, 2 * j : 2 * j + 2, :],
                            start=(j == 0),
                            stop=(j == CH // 2 - 1),
                            perf_mode=DR,
                        )
                    v8 = stg.tile([P, TOK], F8, tag="cp", name=f"v8_{ph}_{tt_}")
                    nc.vector.tensor_scalar_mul(v8[:], psum[:], 0.5)
                    dst = vv_in[
                        tt_ * P : (tt_ + 1) * P,
                        ph * 8 * DHP : (ph + 1) * 8 * DHP,
                    ].rearrange("t (h c) -> t h c", c=DHP)[:, :, 0:DH]
                    nc.gpsimd.dma_start(
                        dst, v8[:].rearrange("t (h d) -> t h d", d=DH)
                    )

            nc.gpsimd.collective_compute(
                "AllGather",
                OP.bypass,
                ins=[kv_in[:].opt()],
                outs=[kv_out[:].opt()],
                replica_groups=REPLICA_GROUPS,
            )

            # Q blocks -> SBUF (chunk 8 duplicates chunk 7 for the hp=7 rhs pair)
            qT = bigs.tile([P, CH + 1, TOK], F8, tag="qT")

            def q_consume(m, psum):
                nc.vector.tensor_scalar(
                    out=qT[:, m, :],
                    in0=psum[:],
                    scalar1=0.5,
                    scalar2=biasqk_sb[:, m : m + 1],
                    op0=OP.mult,
                    op1=OP.add,
                )

            for g in range(2):
                wt = wpool.tile([P, 4, CH, P], F8, tag="wq", name=f"wq{g}")
                nc.sync.dma_start(
                    wt[:],
                    wqk8[4 * g : 4 * g + 4].rearrange("i p (ch o) -> p i ch o", ch=CH),
                )
                for i in range(4):
                    qkv_block(wt, i, 4 * g + i, q_consume)
            nc.sync.dma_start(qT[:, CH, :], qT[:, CH - 1, :])
            st2.__exit__(None, None, None)

            # ---- attention ----
            # kp: [P(2 heads' d), 2, KC, P] fp8; slot0 = K data, slot1 = zeros
            kp_t = []
            for i in range(2):
                t = kvz.tile([P, 2, KC, P], F8, tag="kp", name=f"kp{i}")
                nc.vector.memset(t[:, 1, :, :].bitcast(U8), 0)
                kp_t.append(t)
            # vfull: [P(key-in-chunk), KC, 16*(64+1)] fp8, ones pre-gathered
            vfull = bigs.tile([P, KC, H * DHP], F8, tag="vfull")
            for r in range(RANKS):
                src = (
                    kv_or[r : r + 1, KV_K:KV_SZ]
                    .rearrange("o (t v) -> o t v", v=H * DHP)[0]
                    .rearrange("(tc p) v -> p tc v", p=P)
                )
                nc.sync.dma_start(vfull[:, 4 * r : 4 * r + 4, :], src)

            def load_kp(hp, t):
                src = (
                    kv_or[:, 0:KV_K]
                    .rearrange("r (f t) -> r f t", t=TOK)[
                        :, hp * P : (hp + 1) * P, :
                    ]
                    .rearrange("r p t -> p r t")
                )
                nc.sync.dma_start(t[:, 0, :, :], src)

            ctxT = bigs.tile([P, CH, TOK], F8, tag="x8")  # reuse x8 region
            groups = [(0, 3), (3, 3), (6, 3), (9, 3), (12, 2), (14, 2)]
            pairs_after = {1: [0, 1, 2], 2: [3], 3: [4, 5], 4: [6], 5: [7]}

            att_pools = (
                tc.tile_pool(name="ps_s", bufs=2, space="PSUM"),
                tc.tile_pool(name="ps_ctx", bufs=2, space="PSUM"),
            )
            ps_spool = att_pools[0].__enter__()
            ps_ctx = att_pools[1].__enter__()
            eidx = 0
            for hp in range(H // 2):
                kp = kp_t[hp % 2]
                load_kp(hp, kp)
                for hh in range(2):
                    h = 2 * hp + hh
                    half = slice(hh * DH, hh * DH + DH)
                    qpair = qT[half, hp : hp + 2, :]
                    pt = ppool.tile([P, KC, TOK], F8, tag="pt", name=f"pt{h}")
                    psum_c = ps_ctx.tile([DH + 1, TOK], F32, tag="ctx")
                    for gi, (kc0, nb) in enumerate(groups):
                        ps_s = ps_spool.tile([P, 3 * TOK], F32, tag="s")
                        for j in range(nb):
                            nc.tensor.matmul(
                                ps_s[:, j * TOK : (j + 1) * TOK],
                                kp[half, :, kc0 + j, :],
                                qpair,
                                start=True,
                                stop=True,
                                perf_mode=DR,
                            )
                        dst = pt[:, kc0 : kc0 + nb, :]
                        # alternate Act/DVE so consecutive groups of a head
                        # never serialize on one engine (GPSIMD can't read
                        # PSUM per the BIR verifier); 4:2 toward Act since
                        # DVE also carries reciprocal + normalize
                        if (gi + h) % 2 == 0:
                            nc.scalar.activation(
                                out=dst,
                                in_=ps_s[:, : nb * TOK],
                                func=AF.Exp,
                                bias=negc_row[:],
                                scale=2.0**-11,
                            )
                        else:
                            nc.vector.tensor_scalar(
                                out=dst.bitcast(U8),
                                in0=ps_s[:, : nb * TOK],
                                scalar1=SA,
                                scalar2=SB,
                                op0=OP.mult,
                                op1=OP.add,
                            )
                        eidx += 1
                        for pj in pairs_after.get(gi, []):
                            nc.tensor.matmul(
                                psum_c[:],
                                vfull[
                                    :,
                                    2 * pj : 2 * pj + 2,
                                    h * DHP : h * DHP + DH + 1,
                                ],
                                pt[:, 2 * pj : 2 * pj + 2, :],
                                start=(pj == 0),
                                stop=(pj == KC // 2 - 1),
                                perf_mode=DR,
                            )
                    rrow = rows.tile([1, TOK], F32, tag="r", name=f"rr{h}")
                    nc.vector.reciprocal(rrow[:], psum_c[DH : DH + 1, :])
                    rb = bcpool.tile([DH, TOK], F32, tag="rb", name=f"rb{h}")
                    nc.gpsimd.partition_broadcast(rb[:], rrow[:])
                    nc.vector.tensor_tensor(
                        ctxT[half, hp, :], psum_c[0:DH, :], rb[:], OP.mult
                    )
            att_pools[1].__exit__(None, None, None)
            att_pools[0].__exit__(None, None, None)

            # ---- proj + residual ----
            st4 = tc.tile_pool(name="ps_mm4", bufs=5, space="PSUM")
            ps_mlp = st4.__enter__()
            x2 = bigs.tile([P, CH, TOK], F32R, tag="x2")
            for g in range(2):
                wt = wpool.tile([P, 4, CH, P], F8, tag="wq", name=f"wpj{g}")
                nc.sync.dma_start(
                    wt[:],
                    wp8[4 * g : 4 * g + 4].rearrange("i p (ch o) -> p i ch o", ch=CH),
                )
                for i in range(4):
                    m = 4 * g + i
                    psum = ps_mlp.tile([P, TOK], F32, tag="acc", name=f"pp_{m}")
                    for j in range(CH // 2):
                        nc.tensor.matmul(
                            psum[:],
                            wt[:, i, 2 * j : 2 * j + 2, :],
                            ctxT[:, 2 * j : 2 * j + 2, :],
                            start=(j == 0),
                            stop=(j == CH // 2 - 1),
                            perf_mode=DR,
                        )
                    attn_sb = stg.tile([P, TOK], F32, tag="stg", name=f"at_{m}")
                    nc.scalar.activation(
                        out=attn_sb[:],
                        in_=psum[:],
                        func=AF.Identity,
                        bias=biaspf_sb[:, m : m + 1],
                        scale=2.0**-9,
                    )
                    nc.gpsimd.tensor_tensor(
                        x2[:, m, :], attn_sb[:], xr_sb[:, m, :], OP.add
                    )  # f32r out: rounded on write for the LN2 stats matmul

            # ---- LN2 (fp32r stats on x2) ----
            with tc.tile_pool(name="ps_row2", bufs=2, space="PSUM") as prow:
                psum_mu2 = prow.tile([1, TOK], F32, tag="row")
                psum_s22 = prow.tile([1, TOK], F32, tag="row")
                for ch in range(CH):
                    nc.tensor.matmul(
                        psum_mu2[:],
                        ones_r[:],
                        x2[:, ch, :],
                        start=(ch == 0),
                        stop=(ch == CH - 1),
                    )
                    sq = work.tile([P, TOK], F32R, tag="t1", name=f"sq2_{ch}")
                    nc.gpsimd.tensor_tensor(
                        sq[:],
                        x2[:, ch, :].bitcast(F32),
                        x2[:, ch, :].bitcast(F32),
                        OP.mult,
                    )
                    nc.tensor.matmul(
                        psum_s22[:],
                        ones_r[:],
                        sq[:],
                        start=(ch == 0),
                        stop=(ch == CH - 1),
                    )
                rstd2_b, c2_b = ln_rows(psum_mu2, psum_s22, "2")

            xn = bigs.tile([P, CH, TOK], BF, tag="xn")
            for ch in range(CH):
                t1 = work.tile([P, TOK], F32, tag="t1", name=f"t2_{ch}")
                nc.gpsimd.tensor_tensor(
                    t1[:], x2[:, ch, :].bitcast(F32), rstd2_b[:], OP.mult
                )
                nc.vector.tensor_tensor(xn[:, ch, :], t1[:], c2_b[:], OP.subtract)

            # ---- MLP (bf16) ----
            gbf = bigs.tile([P, HCH, TOK], BF, tag="g")
            for g in range(CH):
                w1 = wpool.tile([P, 4, CH, P], BF, tag="w1", name=f"w1_{g}")
                nc.sync.dma_start(
                    w1[:], w1bf[g].rearrange("p (i ch o) -> p i ch o", i=4, ch=CH)
                )
                for i in range(4):
                    m = 4 * g + i
                    psum = ps_mlp.tile([P, TOK], F32, tag="acc", name=f"p1_{m}")
                    for ch in range(CH):
                        nc.tensor.matmul(
                            psum[:],
                            w1[:, i, ch, :],
                            xn[:, ch, :],
                            start=(ch == 0),
                            stop=(ch == CH - 1),
                        )
                    nc.scalar.activation(
                        out=gbf[:, m, :],
                        in_=psum[:],
                        func=AF.Gelu,
                        bias=fc1b_sb[:, m : m + 1],
                    )
            for m2 in range(CH):
                w2 = wpool.tile([P, HCH, P], BF, tag="w2", name=f"w2_{m2}")
                nc.sync.dma_start(
                    w2[:], w2bf[m2].rearrange("p (hc o) -> p hc o", hc=HCH)
                )
                psum = ps_mlp.tile([P, TOK], F32, tag="acc", name=f"p2_{m2}")
                for hc in range(HCH):
                    nc.tensor.matmul(
                        psum[:],
                        w2[:, hc, :],
                        gbf[:, hc, :],
                        start=(hc == 0),
                        stop=(hc == HCH - 1),
                    )
                o_sb = stg.tile([P, TOK], F32, tag="stg", name=f"o_{m2}")
                nc.scalar.activation(
                    out=o_sb[:],
                    in_=psum[:],
                    func=AF.Identity,
                    bias=biaspf_sb[:, 8 + m2 : 9 + m2],
                )
                o_f = stg.tile([P, TOK], F32, tag="of", bufs=2, name=f"of_{m2}")
                nc.vector.tensor_add(
                    out=o_f[:], in0=o_sb[:], in1=x2[:, m2, :].bitcast(F32)
                )
                nc.sync.dma_start(outT[m2 * P : (m2 + 1) * P, :], o_f[:])
            st4.__exit__(None, None, None)

    if do_compile:
        nc.compile()
    return nc


_CACHE = {}


def _get_program():
    if "nc" not in _CACHE:
        _CACHE["nc"] = build_program()
    return _CACHE["nc"]


def _prep_inputs(inputs):
    E4 = ml_dtypes.float8_e4m3
    x = np.asarray(inputs["x"], np.float32)
    g1 = np.asarray(inputs["ln1_g"], np.float32)
    b1 = np.asarray(inputs["ln1_b"], np.float32)
    g2 = np.asarray(inputs["ln2_g"], np.float32)
    b2 = np.asarray(inputs["ln2_b"], np.float32)
    qkv_w = np.asarray(inputs["qkv_w"], np.float32) * g1[None, :]
    proj_w = np.asarray(inputs["proj_w"], np.float32)
    fc1_w = np.asarray(inputs["fc1_w"], np.float32) * g2[None, :]
    fc2_w = np.asarray(inputs["fc2_w"], np.float32)

    qkv_bias = np.asarray(inputs["qkv_w"], np.float32) @ b1  # [3D]
    assert np.abs(qkv_bias[2 * D :]).max() == 0.0, "nonzero ln1_b v-bias unsupported"

    def wtile8(w, blocks):
        """w [O, D] -> [nb, P, CH*P] fp8 with [m, p, ch*128+o] = 32*w[m*128+o, ch*128+p]."""
        out = np.empty((len(blocks), P, CH * P), E4)
        for bi, m in enumerate(blocks):
            blk = w[m * P : (m + 1) * P, :] * 32.0  # [o 128, c 1024]
            out[bi] = (
                blk.reshape(P, CH, P).transpose(2, 1, 0).reshape(P, CH * P)
            ).astype(E4)
        return out

    def wtile_bf(w, nb, batch):
        """w [O, D] -> [nb//batch, P, batch*CH*P] bf16 tiles."""
        out = np.empty((nb // batch, P, batch * (w.shape[1] // P) * P), ml_dtypes.bfloat16)
        chn = w.shape[1] // P
        for g in range(nb // batch):
            t = np.empty((P, batch, chn, P), np.float32)
            for i in range(batch):
                m = g * batch + i
                blk = w[m * P : (m + 1) * P, :]  # [o, c]
                t[:, i] = blk.reshape(P, chn, P).transpose(2, 1, 0)
            out[g] = t.reshape(P, -1).astype(ml_dtypes.bfloat16)
        return out

    # V weights token-major: [ph, p, ch*512+vc] = 32*qkv_w'[2D+ph*512+vc, ch*128+p]
    wv = np.empty((2, P, CH * TOK), E4)
    for ph in range(2):
        blk = qkv_w[2 * D + ph * TOK : 2 * D + (ph + 1) * TOK, :] * 32.0  # [vc, c]
        wv[ph] = blk.reshape(TOK, CH, P).transpose(2, 1, 0).reshape(P, CH * TOK).astype(E4)

    bqk = np.zeros((P, 16), np.float32)
    bqk[:, 0:8] = _stripe(16.0 * qkv_bias[0:D])
    bqk[:, 8:16] = _stripe(16.0 * qkv_bias[D : 2 * D])
    bpf = np.zeros((P, 16), np.float32)
    bpf[:, 0:8] = _stripe(inputs["proj_b"])
    bpf[:, 8:16] = _stripe(inputs["fc2_b"])

    shared = {
        "wqk8": wtile8(qkv_w, list(range(16))),
        "wv8": wv,
        "wp8": wtile8(proj_w, list(range(CH))),
        "w1bf": wtile_bf(fc1_w, HCH, 4),
        "w2bf": wtile_bf(fc2_w, CH, 1),
        "biasqk": bqk,
        "biaspf": bpf,
        "fc1b": _stripe(
            np.asarray(inputs["fc1_b"], np.float32)
            + np.asarray(inputs["fc1_w"], np.float32) @ b2
        ),
    }
    in_maps = []
    for c in range(NCORES):
        b, blk = divmod(c, RANKS)
        xblk = x[b, blk * TOK : (blk + 1) * TOK, :]  # [TOK, D]
        xt = round_fp32r(np.ascontiguousarray(xblk.T))
        m = dict(shared)
        m["xT"] = xt
        m["x8T"] = xt.astype(E4)
        in_maps.append(m)
    return in_maps


def _assemble(results):
    out = np.empty((B, N, D), dtype=np.float32)
    for c in range(NCORES):
        b, blk = divmod(c, RANKS)
        out[b, blk * TOK : (blk + 1) * TOK, :] = results[c]["outT"].T
    return out


def run_device(inputs, **kwargs):
    nc = _get_program()
    in_maps = _prep_inputs(inputs)
    res = run_bass_kernel_spmd(nc, in_maps, core_ids=list(range(NCORES)), **kwargs)
    return _assemble(res.results), res


def kernel(**inputs) -> np.ndarray:
    out, _ = run_device(inputs)
    return out


# revision 10
# speedup vs baseline: 1.0091x; 1.0058x over previous
"""Trainium2 Bass kernel v2: fp8 DoubleRow attention + bf16 MLP.

Sharding: sequence-parallel over 8 cores (512 tokens each, batch = core//4).
One 4-rank AllGather carries fp8 K (feature-major) + fp8 V (token-major).

Precision plan (validated in numpy, max_rel ~1.0e-2 vs 2e-2 gate):
  - weights qkv/proj: e4m3 x32 host-scaled; fc1/fc2: bf16 (MLP dominates error)
  - h1/q/k/v/pt/ctx: e4m3 (q,k,v at sigma~16 via 0.5 consume scale)
  - softmax exp: constant shift C=4 (cancels in normalize); split between
    Act (native Exp -> fp8) and DVE (Schraudolph: psum*a+b -> uint8 whose
    bit pattern IS e4m3 2^x; floor-vs-round ambiguity is a constant factor
    that cancels in the softmax normalize)
  - x residual fp32r; LN stats via ones-matmul (fp8 DoubleRow for LN1 on
    host-provided x8, fp32r for LN2)
DoubleRow pair slots: chunk pairs for QKV/ctx/proj; (k, zeros) for scores
(d=64 contraction cannot pair; zero slot makes the 0.5 cyc/row rate legal).
"""
import sys

sys.path.insert(0, "/opt/trn_rl_repo")
import numpy as np
import ml_dtypes
import concourse.bass as bass
import concourse.mybir as mybir
import concourse.tile as tile
from concourse import bacc
from concourse.bass_utils import run_bass_kernel_spmd

B, N, D = 2, 2048, 1024
H, DH = 16, 64
HID = 4096
NCORES = 8
TOK = (B * N) // NCORES  # 512
EPS = 1e-5
SCALE = DH**-0.5
P = 128
CH = D // P  # 8
KC = N // P  # 16
HCH = HID // P  # 32
RANKS = 4
CSH = 4.0  # exp arg shift, cancels in softmax
LN2_ = float(np.log(2.0))
# Schraudolph uint8-as-e4m3: y = psum * SA + SB
SA = 8.0 * (2.0**-11) / LN2_
SB = 56.5 - 8.0 * CSH / LN2_

F32 = mybir.dt.float32
F32R = mybir.dt.float32r
F8 = mybir.dt.float8e4
BF = mybir.dt.bfloat16
U8 = mybir.dt.uint8
AF = mybir.ActivationFunctionType
OP = mybir.AluOpType
DR = mybir.MatmulPerfMode.DoubleRow

REPLICA_GROUPS = [[0, 1, 2, 3], [4, 5, 6, 7]]

KV_K = D * TOK  # bytes of K region (fp8 feature-major [1024, 512])
DHP = DH + 16  # per-head stride in V region: 64 v + 1 ones + 15 pad
# (dual-fp8 LdWeights requires 16B-aligned weight base addresses)
KV_V = TOK * (H * DHP)  # V region [512, 1280]
KV_SZ = KV_K + KV_V


def round_fp32r(x: np.ndarray) -> np.ndarray:
    u = np.ascontiguousarray(x, dtype=np.float32).view(np.uint32)
    u = (u + 0x7FF + ((u >> 12) & 1)) & np.uint32(0xFFFFF000)
    return u.view(np.float32)


def _stripe(v: np.ndarray) -> np.ndarray:
    """[M] -> [P, M//P] with col m, part p = v[m*128+p]."""
    return np.ascontiguousarray(np.asarray(v, np.float32).reshape(-1, P).T)


def build_program(do_compile=True):
    nc = bacc.Bacc("TRN2", target_bir_lowering=False, debug=False, num_devices=NCORES)

    xT = nc.dram_tensor("xT", [D, TOK], F32, kind="ExternalInput").ap()
    x8T = nc.dram_tensor("x8T", [D, TOK], F8, kind="ExternalInput").ap()
    # weight tiles, DMA-contiguous per partition
    wqk8 = nc.dram_tensor("wqk8", [16, P, CH * P], F8, kind="ExternalInput").ap()
    wv8 = nc.dram_tensor("wv8", [2, P, CH * TOK], F8, kind="ExternalInput").ap()
    wp8 = nc.dram_tensor("wp8", [CH, P, CH * P], F8, kind="ExternalInput").ap()
    # fc1 weights as fp8 hi/lo pairs [.., (chunk, hi/lo), out]; fc2 stays bf16
    w1f8 = nc.dram_tensor(
        "w1f8", [CH, P, 4 * 2 * CH * P], F8, kind="ExternalInput"
    ).ap()
    w2bf = nc.dram_tensor("w2bf", [CH, P, HCH * P], BF, kind="ExternalInput").ap()
    biasqk = nc.dram_tensor("biasqk", [P, 16], F32, kind="ExternalInput").ap()
    biaspf = nc.dram_tensor("biaspf", [P, 16], F32, kind="ExternalInput").ap()
    fc1b = nc.dram_tensor("fc1b", [P, HCH], F32, kind="ExternalInput").ap()
    outT = nc.dram_tensor("outT", [D, TOK], F32, kind="ExternalOutput").ap()

    with tile.TileContext(nc) as tc:
        with (
            tc.tile_pool(name="consts", bufs=1) as consts,
            tc.tile_pool(name="bigs", bufs=1) as bigs,
            tc.tile_pool(name="work", bufs=3) as work,
            tc.tile_pool(name="wpool", bufs=2) as wpool,
            tc.tile_pool(name="kvz", bufs=2) as kvz,
            tc.tile_pool(name="pp", bufs=2) as ppool,
            tc.tile_pool(name="rows", bufs=3) as rows,
            tc.tile_pool(name="bc", bufs=2) as bcpool,
            tc.tile_pool(name="stg", bufs=2) as stg,
            tc.tile_pool(name="dram", bufs=1, space="DRAM") as dram,
        ):
            ones8 = consts.tile([P, 1], F8)
            nc.vector.memset(ones8[:].bitcast(U8), 0x38)  # e4m3 1.0
            ones_r = consts.tile([P, 1], F32R)
            nc.vector.memset(ones_r[:].bitcast(F32), 1.0)
            eps_row = consts.tile([1, 1], F32, tag="eps")
            nc.vector.memset(eps_row[:], EPS)
            negc_row = consts.tile([P, 1], F32, tag="negc")
            nc.vector.memset(negc_row[:], -CSH)
            onesv = consts.tile([P, 4, H], F8, tag="onesv")
            nc.vector.memset(onesv[:].bitcast(U8), 0x38)
            biasqk_sb = consts.tile([P, 16], F32, tag="bqk")
            biaspf_sb = consts.tile([P, 16], F32, tag="bpf")
            fc1b_sb = consts.tile([P, HCH], F32, tag="b1")
            nc.sync.dma_start(biasqk_sb[:], biasqk[:])
            nc.sync.dma_start(biaspf_sb[:], biaspf[:])
            nc.sync.dma_start(fc1b_sb[:], fc1b[:])

            kv_in = dram.tile([KV_SZ], F8, tag="kvin")
            kv_out = dram.tile([RANKS * KV_SZ], F8, tag="kvout")
            vk_in = kv_in[0:KV_K].rearrange("(f t) -> f t", t=TOK)
            vv_in = kv_in[KV_K:KV_SZ].rearrange(
                "(t v) -> t v", v=H * DHP
            )
            kv_or = kv_out[:].rearrange("(r x) -> r x", r=RANKS)
            # ones column of the V region: written up-front (no data deps)
            for tc_ in range(RANKS):
                ones_dst = vv_in[tc_ * P : (tc_ + 1) * P, :].rearrange(
                    "p (h c) -> p h c", c=DHP
                )[:, :, DH : DH + 1]
                nc.sync.dma_start(
                    ones_dst, onesv[:, tc_, :].rearrange("p (h c) -> p h c", c=1)
                )

            # ---- LN1 ---- (x8 first: stats depend on it; xr arrives later)
            x8_sb = bigs.tile([P, CH, TOK], F8, tag="x8")
            nc.sync.dma_start(
                x8_sb[:], x8T.rearrange("(ch p) t -> p ch t", p=P)
            )
            xr_sb = bigs.tile([P, CH, TOK], F32, tag="xr")
            nc.sync.dma_start(
                xr_sb[:], xT.rearrange("(ch p) t -> p ch t", p=P)
            )
            sq8 = bigs.tile([P, CH, TOK], F8, tag="h1")  # released before h1

            def ln_rows(psum_mu, psum_s2, name):
                mu = rows.tile([1, TOK], F32, tag="r", name=f"mu{name}")
                nc.vector.tensor_scalar_mul(mu[:], psum_mu[:], 1.0 / D)
                var = rows.tile([1, TOK], F32, tag="r", name=f"va{name}")
                nc.vector.tensor_tensor(var[:], mu[:], mu[:], OP.mult)
                ex2 = rows.tile([1, TOK], F32, tag="r", name=f"e2{name}")
                nc.vector.tensor_scalar_mul(ex2[:], psum_s2[:], 1.0 / D)
                nc.vector.tensor_sub(var[:], ex2[:], var[:])
                rstd = rows.tile([1, TOK], F32, tag="r", name=f"rs{name}")
                nc.scalar.activation(
                    out=rstd[:], in_=var[:], func=AF.Sqrt, bias=eps_row[:]
                )
                nc.vector.reciprocal(rstd[:], rstd[:])
                cpos = rows.tile([1, TOK], F32, tag="r", name=f"cp{name}")
                nc.vector.tensor_tensor(cpos[:], mu[:], rstd[:], OP.mult)
                rstd_b = bcpool.tile([P, TOK], F32, tag="bc", name=f"rb{name}")
                nc.gpsimd.partition_broadcast(rstd_b[:], rstd[:])
                c_b = bcpool.tile([P, TOK], F32, tag="bc", name=f"cb{name}")
                nc.gpsimd.partition_broadcast(c_b[:], cpos[:])
                return rstd_b, c_b

            with tc.tile_pool(name="ps_row1", bufs=2, space="PSUM") as prow:
                psum_mu = prow.tile([1, TOK], F32, tag="row")
                psum_s2 = prow.tile([1, TOK], F32, tag="row")
                for ch in range(CH):
                    eng = nc.vector if ch % 2 == 0 else nc.gpsimd
                    eng.tensor_tensor(
                        sq8[:, ch, :], x8_sb[:, ch, :], x8_sb[:, ch, :], OP.mult
                    )
                for ch in range(CH):
                    nc.tensor.matmul(
                        psum_mu[:],
                        ones8[:],
                        x8_sb[:, ch, :],
                        start=(ch == 0),
                        stop=(ch == CH - 1),
                    )
                for ch in range(CH):
                    nc.tensor.matmul(
                        psum_s2[:],
                        ones8[:],
                        sq8[:, ch, :],
                        start=(ch == 0),
                        stop=(ch == CH - 1),
                    )
                rstd1_b, c1_b = ln_rows(psum_mu, psum_s2, "1")

            h1 = bigs.tile([P, CH, TOK], F8, tag="h1")
            for ch in range(CH):
                eng = nc.vector if ch % 2 == 0 else nc.gpsimd
                t1 = work.tile([P, TOK], F32, tag="t1")
                eng.tensor_tensor(t1[:], xr_sb[:, ch, :], rstd1_b[:], OP.mult)
                eng.tensor_tensor(h1[:, ch, :], t1[:], c1_b[:], OP.subtract)

            # ---- QKV ----
            st2 = tc.tile_pool(name="ps_mm2", bufs=5, space="PSUM")
            ps_acc = st2.__enter__()

            def qkv_block(wt, i, m, consume):
                psum = ps_acc.tile([P, TOK], F32, tag="acc", name=f"ps_{m}")
                for j in range(CH // 2):
                    nc.tensor.matmul(
                        psum[:],
                        wt[:, i, 2 * j : 2 * j + 2, :],
                        h1[:, 2 * j : 2 * j + 2, :],
                        start=(j == 0),
                        stop=(j == CH // 2 - 1),
                        perf_mode=DR,
                    )
                consume(m, psum)

            def k_consume(m, psum):
                k8 = stg.tile([P, TOK], F8, tag="cp", name=f"k8_{m}")
                nc.scalar.activation(
                    out=k8[:],
                    in_=psum[:],
                    func=AF.Identity,
                    bias=biasqk_sb[:, 8 + m : 9 + m],
                    scale=0.5,
                )
                nc.scalar.dma_start(vk_in[m * P : (m + 1) * P, :], k8[:])

            # K blocks (g1 folded into weights on host; cols D..2D of qkv_w)
            wk_t = []
            for g in range(2):
                wt = wpool.tile([P, 4, CH, P], F8, tag="wq", name=f"wk{g}")
                nc.sync.dma_start(
                    wt[:],
                    wqk8[8 + 4 * g : 12 + 4 * g].rearrange("i p (ch o) -> p i ch o", ch=CH),
                )
                wk_t.append(wt)
            for m in range(CH):
                qkv_block(wk_t[m // 4], m % 4, m, k_consume)

            # V (token-major): lhsT = h1 chunk-pair, rhs = wv columns
            for ph in range(2):
                wv_t = wpool.tile([P, CH, TOK], F8, tag="wq", name=f"wv{ph}")
                nc.sync.dma_start(
                    wv_t[:], wv8[ph].rearrange("p (ch v) -> p ch v", ch=CH)
                )
                for tt_ in range(TOK // P):
                    psum = ps_acc.tile([P, TOK], F32, tag="acc", name=f"pv{ph}_{tt_}")
                    for j in range(CH // 2):
                        nc.tensor.matmul(
                            psum[:],
                            h1[:, 2 * j : 2 * j + 2, tt_ * P : (tt_ + 1) * P],
                            wv_t[:, 2 * j : 2 * j + 2, :],
                            start=(j == 0),
                            stop=(j == CH // 2 - 1),
                            perf_mode=DR,
                        )
                    v8 = stg.tile([P, TOK], F8, tag="cp", name=f"v8_{ph}_{tt_}")
                    nc.vector.tensor_scalar_mul(v8[:], psum[:], 0.5)
                    dst = vv_in[
                        tt_ * P : (tt_ + 1) * P,
                        ph * 8 * DHP : (ph + 1) * 8 * DHP,
                    ].rearrange("t (h c) -> t h c", c=DHP)[:, :, 0:DH]
                    nc.gpsimd.dma_start(
                        dst, v8[:].rearrange("t (h d) -> t h d", d=DH)
                    )

            nc.gpsimd.collective_compute(
                "AllGather",
                OP.bypass,
                ins=[kv_in[:].opt()],
                outs=[kv_out[:].opt()],
                replica_groups=REPLICA_GROUPS,
            )

            # Q blocks -> SBUF (chunk 8 duplicates chunk 7 for the hp=7 rhs pair)
            qT = bigs.tile([P, CH + 1, TOK], F8, tag="qT")

            def q_consume(m, psum):
                nc.vector.tensor_scalar(
                    out=qT[:, m, :],
                    in0=psum[:],
                    scalar1=0.5,
                    scalar2=biasqk_sb[:, m : m + 1],
                    op0=OP.mult,
                    op1=OP.add,
                )

            for g in range(2):
                wt = wpool.tile([P, 4, CH, P], F8, tag="wq", name=f"wq{g}")
                nc.sync.dma_start(
                    wt[:],
                    wqk8[4 * g : 4 * g + 4].rearrange("i p (ch o) -> p i ch o", ch=CH),
                )
                for i in range(4):
                    qkv_block(wt, i, 4 * g + i, q_consume)
            nc.sync.dma_start(qT[:, CH, :], qT[:, CH - 1, :])
            st2.__exit__(None, None, None)

            # ---- attention ----
            # kp: [P(2 heads' d), 2, KC, P] fp8; slot0 = K data, slot1 = zeros
            kp_t = []
            for i in range(2):
                t = kvz.tile([P, 2, KC, P], F8, tag="kp", name=f"kp{i}")
                nc.vector.memset(t[:, 1, :, :].bitcast(U8), 0)
                kp_t.append(t)
            # vfull: [P(key-in-chunk), KC, 16*(64+1)] fp8, ones pre-gathered
            vfull = bigs.tile([P, KC, H * DHP], F8, tag="vfull")
            for r in range(RANKS):
                src = (
                    kv_or[r : r + 1, KV_K:KV_SZ]
                    .rearrange("o (t v) -> o t v", v=H * DHP)[0]
                    .rearrange("(tc p) v -> p tc v", p=P)
                )
                nc.sync.dma_start(vfull[:, 4 * r : 4 * r + 4, :], src)

            def load_kp(hp, t):
                src = (
                    kv_or[:, 0:KV_K]
                    .rearrange("r (f t) -> r f t", t=TOK)[
                        :, hp * P : (hp + 1) * P, :
                    ]
                    .rearrange("r p t -> p r t")
                )
                nc.sync.dma_start(t[:, 0, :, :], src)

            ctxT = bigs.tile([P, CH, TOK], F8, tag="x8")  # reuse x8 region
            groups = [(0, 3), (3, 3), (6, 3), (9, 3), (12, 2), (14, 2)]
            pairs_after = {1: [0, 1, 2], 2: [3], 3: [4, 5], 4: [6], 5: [7]}

            att_pools = (
                tc.tile_pool(name="ps_s", bufs=2, space="PSUM"),
                tc.tile_pool(name="ps_ctx", bufs=2, space="PSUM"),
            )
            ps_spool = att_pools[0].__enter__()
            ps_ctx = att_pools[1].__enter__()
            eidx = 0
            for hp in range(H // 2):
                kp = kp_t[hp % 2]
                load_kp(hp, kp)
                for hh in range(2):
                    h = 2 * hp + hh
                    half = slice(hh * DH, hh * DH + DH)
                    qpair = qT[half, hp : hp + 2, :]
                    pt = ppool.tile([P, KC, TOK], F8, tag="pt", name=f"pt{h}")
                    psum_c = ps_ctx.tile([DH + 1, TOK], F32, tag="ctx")
                    for gi, (kc0, nb) in enumerate(groups):
                        ps_s = ps_spool.tile([P, 3 * TOK], F32, tag="s")
                        for j in range(nb):
                            nc.tensor.matmul(
                                ps_s[:, j * TOK : (j + 1) * TOK],
                                kp[half, :, kc0 + j, :],
                                qpair,
                                start=True,
                                stop=True,
                                perf_mode=DR,
                            )
                        dst = pt[:, kc0 : kc0 + nb, :]
                        # alternate Act/DVE so consecutive groups of a head
                        # never serialize on one engine (GPSIMD can't read
                        # PSUM per the BIR verifier); 4:2 toward Act since
                        # DVE also carries reciprocal + normalize
                        if (gi + h) % 2 == 0:
                            nc.scalar.activation(
                                out=dst,
                                in_=ps_s[:, : nb * TOK],
                                func=AF.Exp,
                                bias=negc_row[:],
                                scale=2.0**-11,
                            )
                        else:
                            nc.vector.tensor_scalar(
                                out=dst.bitcast(U8),
                                in0=ps_s[:, : nb * TOK],
                                scalar1=SA,
                                scalar2=SB,
                                op0=OP.mult,
                                op1=OP.add,
                            )
                        eidx += 1
                        for pj in pairs_after.get(gi, []):
                            nc.tensor.matmul(
                                psum_c[:],
                                vfull[
                                    :,
                                    2 * pj : 2 * pj + 2,
                                    h * DHP : h * DHP + DH + 1,
                                ],
                                pt[:, 2 * pj : 2 * pj + 2, :],
                                start=(pj == 0),
                                stop=(pj == KC // 2 - 1),
                                perf_mode=DR,
                            )
                    rrow = rows.tile([1, TOK], F32, tag="r", name=f"rr{h}")
                    nc.vector.reciprocal(rrow[:], psum_c[DH : DH + 1, :])
                    rb = bcpool.tile([DH, TOK], F32, tag="rb", name=f"rb{h}")
                    nc.gpsimd.partition_broadcast(rb[:], rrow[:])
                    nc.vector.tensor_tensor(
                        ctxT[half, hp, :], psum_c[0:DH, :], rb[:], OP.mult
                    )
            att_pools[1].__exit__(None, None, None)
            att_pools[0].__exit__(None, None, None)

            # ---- proj + residual ----
            st4 = tc.tile_pool(name="ps_mm4", bufs=5, space="PSUM")
            ps_mlp = st4.__enter__()
            x2 = bigs.tile([P, CH, TOK], F32R, tag="x2")
            for g in range(2):
                wt = wpool.tile([P, 4, CH, P], F8, tag="wq", name=f"wpj{g}")
                nc.sync.dma_start(
                    wt[:],
                    wp8[4 * g : 4 * g + 4].rearrange("i p (ch o) -> p i ch o", ch=CH),
                )
                for i in range(4):
                    m = 4 * g + i
                    psum = ps_mlp.tile([P, TOK], F32, tag="acc", name=f"pp_{m}")
                    for j in range(CH // 2):
                        nc.tensor.matmul(
                            psum[:],
                            wt[:, i, 2 * j : 2 * j + 2, :],
                            ctxT[:, 2 * j : 2 * j + 2, :],
                            start=(j == 0),
                            stop=(j == CH // 2 - 1),
                            perf_mode=DR,
                        )
                    attn_sb = stg.tile([P, TOK], F32, tag="stg", name=f"at_{m}")
                    nc.scalar.activation(
                        out=attn_sb[:],
                        in_=psum[:],
                        func=AF.Identity,
                        bias=biaspf_sb[:, m : m + 1],
                        scale=2.0**-9,
                    )
                    nc.gpsimd.tensor_tensor(
                        x2[:, m, :], attn_sb[:], xr_sb[:, m, :], OP.add
                    )  # f32r out: rounded on write for the LN2 stats matmul

            # ---- LN2 (fp32r stats on x2) ----
            with tc.tile_pool(name="ps_row2", bufs=2, space="PSUM") as prow:
                psum_mu2 = prow.tile([1, TOK], F32, tag="row")
                psum_s22 = prow.tile([1, TOK], F32, tag="row")
                for ch in range(CH):
                    nc.tensor.matmul(
                        psum_mu2[:],
                        ones_r[:],
                        x2[:, ch, :],
                        start=(ch == 0),
                        stop=(ch == CH - 1),
                    )
                    sq = work.tile([P, TOK], F32R, tag="t1", name=f"sq2_{ch}")
                    nc.gpsimd.tensor_tensor(
                        sq[:],
                        x2[:, ch, :].bitcast(F32),
                        x2[:, ch, :].bitcast(F32),
                        OP.mult,
                    )
                    nc.tensor.matmul(
                        psum_s22[:],
                        ones_r[:],
                        sq[:],
                        start=(ch == 0),
                        stop=(ch == CH - 1),
                    )
                rstd2_b, c2_b = ln_rows(psum_mu2, psum_s22, "2")

            # xn as fp8 hi/lo + duplicated-hi slot: [hi, hi_dup, lo]
            xnf = bigs.tile([P, CH, 3, TOK], F8, tag="xn")
            for ch in range(CH):
                t1 = work.tile([P, TOK], F32, tag="t1", name=f"t2_{ch}")
                nc.gpsimd.tensor_tensor(
                    t1[:], x2[:, ch, :].bitcast(F32), rstd2_b[:], OP.mult
                )
                xn32 = work.tile([P, TOK], F32, tag="xn32", bufs=2, name=f"x32_{ch}")
                nc.vector.tensor_tensor(xn32[:], t1[:], c2_b[:], OP.subtract)
                nc.scalar.activation(
                    out=xnf[:, ch, 0, :], in_=xn32[:], func=AF.Identity
                )
                nc.gpsimd.tensor_copy(
                    out=xnf[:, ch, 1, :], in_=xnf[:, ch, 0, :]
                )
                nc.vector.tensor_tensor(
                    xnf[:, ch, 2, :], xn32[:], xnf[:, ch, 0, :], OP.subtract
                )

            # ---- MLP: fc1 fp8 hi/lo "3-product" DoubleRow, fc2 bf16 ----
            # per chunk pair (c, d): [whi_c,wlo_c]x[xhi,xhidup], same for d,
            # then [whi_c,whi_d]x[xlo_c,xlo_d] (drops the negligible lo*lo)
            gbf = bigs.tile([P, HCH, TOK], BF, tag="g")
            for g in range(CH):
                w1 = wpool.tile([P, 4, 2 * CH, P], F8, tag="w1", name=f"w1_{g}")
                nc.sync.dma_start(
                    w1[:],
                    w1f8[g].rearrange("p (i c o) -> p i c o", i=4, c=2 * CH),
                )
                for i in range(4):
                    m = 4 * g + i
                    psum = ps_mlp.tile([P, TOK], F32, tag="acc", name=f"p1_{m}")
                    for c2 in range(CH // 2):
                        c = 2 * c2
                        nc.tensor.matmul(
                            psum[:],
                            w1[:, i, 2 * c : 2 * c + 2, :],
                            xnf[:, c, 0:2, :],
                            start=(c2 == 0),
                            stop=False,
                            perf_mode=DR,
                        )
                        nc.tensor.matmul(
                            psum[:],
                            w1[:, i, 2 * c + 2 : 2 * c + 4, :],
                            xnf[:, c + 1, 0:2, :],
                            start=False,
                            stop=False,
                            perf_mode=DR,
                        )
                        whi = w1[:, i].rearrange(
                            "p (c two) k -> p c two k", two=2
                        )[:, c : c + 2, 0, :]
                        nc.tensor.matmul(
                            psum[:],
                            whi,
                            xnf[:, c : c + 2, 2, :],
                            start=False,
                            stop=(c2 == CH // 2 - 1),
                            perf_mode=DR,
                        )
                    nc.scalar.activation(
                        out=gbf[:, m, :],
                        in_=psum[:],
                        func=AF.Gelu,
                        bias=fc1b_sb[:, m : m + 1],
                        scale=2.0**-5,
                    )
            for m2 in range(CH):
                w2 = wpool.tile([P, HCH, P], BF, tag="w2", name=f"w2_{m2}")
                nc.sync.dma_start(
                    w2[:], w2bf[m2].rearrange("p (hc o) -> p hc o", hc=HCH)
                )
                psum = ps_mlp.tile([P, TOK], F32, tag="acc", name=f"p2_{m2}")
                for hc in range(HCH):
                    nc.tensor.matmul(
                        psum[:],
                        w2[:, hc, :],
                        gbf[:, hc, :],
                        start=(hc == 0),
                        stop=(hc == HCH - 1),
                    )
                o_sb = stg.tile([P, TOK], F32, tag="stg", name=f"o_{m2}")
                nc.scalar.activation(
                    out=o_sb[:],
                    in_=psum[:],
                    func=AF.Identity,
                    bias=biaspf_sb[:, 8 + m2 : 9 + m2],
                )
                o_f = stg.tile([P, TOK], F32, tag="of", bufs=2, name=f"of_{m2}")
                nc.vector.tensor_add(
                    out=o_f[:], in0=o_sb[:], in1=x2[:, m2, :].bitcast(F32)
                )
                nc.sync.dma_start(outT[m2 * P : (m2 + 1) * P, :], o_f[:])
            st4.__exit__(None, None, None)

    if do_compile:
        nc.compile()
    return nc


_CACHE = {}


def _get_program():
    if "nc" not in _CACHE:
        _CACHE["nc"] = build_program()
    return _CACHE["nc"]


def _prep_inputs(inputs):
    E4 = ml_dtypes.float8_e4m3
    x = np.asarray(inputs["x"], np.float32)
    g1 = np.asarray(inputs["ln1_g"], np.float32)
    b1 = np.asarray(inputs["ln1_b"], np.float32)
    g2 = np.asarray(inputs["ln2_g"], np.float32)
    b2 = np.asarray(inputs["ln2_b"], np.float32)
    qkv_w = np.asarray(inputs["qkv_w"], np.float32) * g1[None, :]
    proj_w = np.asarray(inputs["proj_w"], np.float32)
    fc1_w = np.asarray(inputs["fc1_w"], np.float32) * g2[None, :]
    fc2_w = np.asarray(inputs["fc2_w"], np.float32)

    qkv_bias = np.asarray(inputs["qkv_w"], np.float32) @ b1  # [3D]
    assert np.abs(qkv_bias[2 * D :]).max() == 0.0, "nonzero ln1_b v-bias unsupported"

    def wtile8(w, blocks):
        """w [O, D] -> [nb, P, CH*P] fp8 with [m, p, ch*128+o] = 32*w[m*128+o, ch*128+p]."""
        out = np.empty((len(blocks), P, CH * P), E4)
        for bi, m in enumerate(blocks):
            blk = w[m * P : (m + 1) * P, :] * 32.0  # [o 128, c 1024]
            out[bi] = (
                blk.reshape(P, CH, P).transpose(2, 1, 0).reshape(P, CH * P)
            ).astype(E4)
        return out

    def wtile8_hl(w, nb, batch):
        """w [O, D] (pre-scaled) -> [nb//batch, P, batch*2*chn*P] fp8 hi/lo
        tiles: slot (2c+s) holds hi (s=0) / lo residual (s=1) of chunk c."""
        chn = w.shape[1] // P
        out = np.empty((nb // batch, P, batch * 2 * chn * P), E4)
        for g in range(nb // batch):
            t = np.empty((P, batch, 2 * chn, P), E4)
            for i in range(batch):
                m = g * batch + i
                blk = w[m * P : (m + 1) * P, :]  # [o, c]
                wt = blk.reshape(P, chn, P).transpose(2, 1, 0)  # [p, c, o]
                hi = wt.astype(E4)
                lo = (wt - hi.astype(np.float32)).astype(E4)
                t[:, i, 0::2, :] = hi
                t[:, i, 1::2, :] = lo
            out[g] = t.reshape(P, -1)
        return out

    def wtile_bf(w, nb, batch):
        """w [O, D] -> [nb//batch, P, batch*CH*P] bf16 tiles."""
        out = np.empty((nb // batch, P, batch * (w.shape[1] // P) * P), ml_dtypes.bfloat16)
        chn = w.shape[1] // P
        for g in range(nb // batch):
            t = np.empty((P, batch, chn, P), np.float32)
            for i in range(batch):
                m = g * batch + i
                blk = w[m * P : (m + 1) * P, :]  # [o, c]
                t[:, i] = blk.reshape(P, chn, P).transpose(2, 1, 0)
            out[g] = t.reshape(P, -1).astype(ml_dtypes.bfloat16)
        return out

    # V weights token-major: [ph, p, ch*512+vc] = 32*qkv_w'[2D+ph*512+vc, ch*128+p]
    wv = np.empty((2, P, CH * TOK), E4)
    for ph in range(2):
        blk = qkv_w[2 * D + ph * TOK : 2 * D + (ph + 1) * TOK, :] * 32.0  # [vc, c]
        wv[ph] = blk.reshape(TOK, CH, P).transpose(2, 1, 0).reshape(P, CH * TOK).astype(E4)

    bqk = np.zeros((P, 16), np.float32)
    bqk[:, 0:8] = _stripe(16.0 * qkv_bias[0:D])
    bqk[:, 8:16] = _stripe(16.0 * qkv_bias[D : 2 * D])
    bpf = np.zeros((P, 16), np.float32)
    bpf[:, 0:8] = _stripe(inputs["proj_b"])
    bpf[:, 8:16] = _stripe(inputs["fc2_b"])

    shared = {
        "wqk8": wtile8(qkv_w, list(range(16))),
        "wv8": wv,
        "wp8": wtile8(proj_w, list(range(CH))),
        "w1f8": wtile8_hl(fc1_w * 32.0, HCH, 4),
        "w2bf": wtile_bf(fc2_w, CH, 1),
        "biasqk": bqk,
        "biaspf": bpf,
        "fc1b": _stripe(
            np.asarray(inputs["fc1_b"], np.float32)
            + np.asarray(inputs["fc1_w"], np.float32) @ b2
        ),
    }
    in_maps = []
    for c in range(NCORES):
        b, blk = divmod(c, RANKS)
        xblk = x[b, blk * TOK : (blk + 1) * TOK, :]  # [TOK, D]
        xt = round_fp32r(np.ascontiguousarray(xblk.T))
        m = dict(shared)
        m["xT"] = xt
        m["x8T"] = xt.astype(E4)
        in_maps.append(m)
    return in_maps


def _assemble(results):
    out = np.empty((B, N, D), dtype=np.float32)
    for c in range(NCORES):
        b, blk = divmod(c, RANKS)
        out[b, blk * TOK : (blk + 1) * TOK, :] = results[c]["outT"].T
    return out


def run_device(inputs, **kwargs):
    nc = _get_program()
    in_maps = _prep_inputs(inputs)
    res = run_bass_kernel_spmd(nc, in_maps, core_ids=list(range(NCORES)), **kwargs)
    return _assemble(res.results), res


def kernel(**inputs) -> np.ndarray:
    out, _ = run_device(inputs)
    return out


# revision 11
# speedup vs baseline: 1.0161x; 1.0070x over previous
"""Trainium2 Bass kernel v2: fp8 DoubleRow attention + bf16 MLP.

Sharding: sequence-parallel over 8 cores (512 tokens each, batch = core//4).
One 4-rank AllGather carries fp8 K (feature-major) + fp8 V (token-major).

Precision plan (validated in numpy, max_rel ~1.0e-2 vs 2e-2 gate):
  - weights qkv/proj: e4m3 x32 host-scaled; fc1/fc2: bf16 (MLP dominates error)
  - h1/q/k/v/pt/ctx: e4m3 (q,k,v at sigma~16 via 0.5 consume scale)
  - softmax exp: constant shift C=4 (cancels in normalize); split between
    Act (native Exp -> fp8) and DVE (Schraudolph: psum*a+b -> uint8 whose
    bit pattern IS e4m3 2^x; floor-vs-round ambiguity is a constant factor
    that cancels in the softmax normalize)
  - x residual fp32r; LN stats via ones-matmul (fp8 DoubleRow for LN1 on
    host-provided x8, fp32r for LN2)
DoubleRow pair slots: chunk pairs for QKV/ctx/proj; (k, zeros) for scores
(d=64 contraction cannot pair; zero slot makes the 0.5 cyc/row rate legal).
"""
import sys

sys.path.insert(0, "/opt/trn_rl_repo")
import numpy as np
import ml_dtypes
import concourse.bass as bass
import concourse.mybir as mybir
import concourse.tile as tile
from concourse import bacc
from concourse.bass_utils import run_bass_kernel_spmd

B, N, D = 2, 2048, 1024
H, DH = 16, 64
HID = 4096
NCORES = 8
TOK = (B * N) // NCORES  # 512
EPS = 1e-5
SCALE = DH**-0.5
P = 128
CH = D // P  # 8
KC = N // P  # 16
HCH = HID // P  # 32
RANKS = 4
CSH = 4.0  # exp arg shift, cancels in softmax
LN2_ = float(np.log(2.0))
# Schraudolph uint8-as-e4m3: y = psum * SA + SB
SA = 8.0 * (2.0**-11) / LN2_
SB = 56.5 - 8.0 * CSH / LN2_

F32 = mybir.dt.float32
F32R = mybir.dt.float32r
F8 = mybir.dt.float8e4
BF = mybir.dt.bfloat16
U8 = mybir.dt.uint8
AF = mybir.ActivationFunctionType
OP = mybir.AluOpType
DR = mybir.MatmulPerfMode.DoubleRow

REPLICA_GROUPS = [[0, 1, 2, 3], [4, 5, 6, 7]]

KV_K = D * TOK  # bytes of K region (fp8 feature-major [1024, 512])
DHP = DH + 16  # per-head stride in V region: 64 v + 1 ones + 15 pad
# (dual-fp8 LdWeights requires 16B-aligned weight base addresses)
KV_V = TOK * (H * DHP)  # V region [512, 1280]
KV_SZ = KV_K + KV_V


def round_fp32r(x: np.ndarray) -> np.ndarray:
    u = np.ascontiguousarray(x, dtype=np.float32).view(np.uint32)
    u = (u + 0x7FF + ((u >> 12) & 1)) & np.uint32(0xFFFFF000)
    return u.view(np.float32)


def _stripe(v: np.ndarray) -> np.ndarray:
    """[M] -> [P, M//P] with col m, part p = v[m*128+p]."""
    return np.ascontiguousarray(np.asarray(v, np.float32).reshape(-1, P).T)


def build_program(do_compile=True):
    nc = bacc.Bacc("TRN2", target_bir_lowering=False, debug=False, num_devices=NCORES)

    xT = nc.dram_tensor("xT", [D, TOK], F32, kind="ExternalInput").ap()
    x8T = nc.dram_tensor("x8T", [D, TOK], F8, kind="ExternalInput").ap()
    # weight tiles, DMA-contiguous per partition
    wqk8 = nc.dram_tensor("wqk8", [16, P, CH * P], F8, kind="ExternalInput").ap()
    wv8 = nc.dram_tensor("wv8", [2, P, CH * TOK], F8, kind="ExternalInput").ap()
    wp8 = nc.dram_tensor("wp8", [CH, P, CH * P], F8, kind="ExternalInput").ap()
    # fc1 weights as fp8 hi/lo pairs [.., (chunk, hi/lo), out]; fc2 stays bf16
    w1f8 = nc.dram_tensor(
        "w1f8", [CH, P, 4 * 2 * CH * P], F8, kind="ExternalInput"
    ).ap()
    # fc2 weights fp8 (hi, hi_dup, lo) triples per chunk (dups host-side)
    w2f8 = nc.dram_tensor(
        "w2f8", [CH, P, 3 * HCH * P], F8, kind="ExternalInput"
    ).ap()
    biasqk = nc.dram_tensor("biasqk", [P, 16], F32, kind="ExternalInput").ap()
    biaspf = nc.dram_tensor("biaspf", [P, 16], F32, kind="ExternalInput").ap()
    fc1b = nc.dram_tensor("fc1b", [P, HCH], F32, kind="ExternalInput").ap()
    outT = nc.dram_tensor("outT", [D, TOK], F32, kind="ExternalOutput").ap()

    with tile.TileContext(nc) as tc:
        with (
            tc.tile_pool(name="consts", bufs=1) as consts,
            tc.tile_pool(name="bigs", bufs=1) as bigs,
            tc.tile_pool(name="work", bufs=3) as work,
            tc.tile_pool(name="wpool", bufs=2) as wpool,
            tc.tile_pool(name="kvz", bufs=2) as kvz,
            tc.tile_pool(name="pp", bufs=2) as ppool,
            tc.tile_pool(name="rows", bufs=3) as rows,
            tc.tile_pool(name="bc", bufs=2) as bcpool,
            tc.tile_pool(name="stg", bufs=2) as stg,
            tc.tile_pool(name="dram", bufs=1, space="DRAM") as dram,
        ):
            ones8 = consts.tile([P, 1], F8)
            nc.vector.memset(ones8[:].bitcast(U8), 0x38)  # e4m3 1.0
            ones_r = consts.tile([P, 1], F32R)
            nc.vector.memset(ones_r[:].bitcast(F32), 1.0)
            eps_row = consts.tile([1, 1], F32, tag="eps")
            nc.vector.memset(eps_row[:], EPS)
            negc_row = consts.tile([P, 1], F32, tag="negc")
            nc.vector.memset(negc_row[:], -CSH)
            onesv = consts.tile([P, 4, H], F8, tag="onesv")
            nc.vector.memset(onesv[:].bitcast(U8), 0x38)
            biasqk_sb = consts.tile([P, 16], F32, tag="bqk")
            biaspf_sb = consts.tile([P, 16], F32, tag="bpf")
            fc1b_sb = consts.tile([P, HCH], F32, tag="b1")
            nc.sync.dma_start(biasqk_sb[:], biasqk[:])
            nc.sync.dma_start(biaspf_sb[:], biaspf[:])
            nc.sync.dma_start(fc1b_sb[:], fc1b[:])

            kv_in = dram.tile([KV_SZ], F8, tag="kvin")
            kv_out = dram.tile([RANKS * KV_SZ], F8, tag="kvout")
            vk_in = kv_in[0:KV_K].rearrange("(f t) -> f t", t=TOK)
            vv_in = kv_in[KV_K:KV_SZ].rearrange(
                "(t v) -> t v", v=H * DHP
            )
            kv_or = kv_out[:].rearrange("(r x) -> r x", r=RANKS)
            # ones column of the V region: written up-front (no data deps)
            for tc_ in range(RANKS):
                ones_dst = vv_in[tc_ * P : (tc_ + 1) * P, :].rearrange(
                    "p (h c) -> p h c", c=DHP
                )[:, :, DH : DH + 1]
                nc.sync.dma_start(
                    ones_dst, onesv[:, tc_, :].rearrange("p (h c) -> p h c", c=1)
                )

            # ---- LN1 ---- (x8 first: stats depend on it; xr arrives later)
            x8_sb = bigs.tile([P, CH, TOK], F8, tag="x8")
            nc.sync.dma_start(
                x8_sb[:], x8T.rearrange("(ch p) t -> p ch t", p=P)
            )
            xr_sb = bigs.tile([P, CH, TOK], F32, tag="xr")
            nc.sync.dma_start(
                xr_sb[:], xT.rearrange("(ch p) t -> p ch t", p=P)
            )
            sq8 = bigs.tile([P, CH, TOK], F8, tag="h1")  # released before h1

            def ln_rows(psum_mu, psum_s2, name):
                mu = rows.tile([1, TOK], F32, tag="r", name=f"mu{name}")
                nc.vector.tensor_scalar_mul(mu[:], psum_mu[:], 1.0 / D)
                var = rows.tile([1, TOK], F32, tag="r", name=f"va{name}")
                nc.vector.tensor_tensor(var[:], mu[:], mu[:], OP.mult)
                ex2 = rows.tile([1, TOK], F32, tag="r", name=f"e2{name}")
                nc.vector.tensor_scalar_mul(ex2[:], psum_s2[:], 1.0 / D)
                nc.vector.tensor_sub(var[:], ex2[:], var[:])
                rstd = rows.tile([1, TOK], F32, tag="r", name=f"rs{name}")
                nc.scalar.activation(
                    out=rstd[:], in_=var[:], func=AF.Sqrt, bias=eps_row[:]
                )
                nc.vector.reciprocal(rstd[:], rstd[:])
                cpos = rows.tile([1, TOK], F32, tag="r", name=f"cp{name}")
                nc.vector.tensor_tensor(cpos[:], mu[:], rstd[:], OP.mult)
                rstd_b = bcpool.tile([P, TOK], F32, tag="bc", name=f"rb{name}")
                nc.gpsimd.partition_broadcast(rstd_b[:], rstd[:])
                c_b = bcpool.tile([P, TOK], F32, tag="bc", name=f"cb{name}")
                nc.gpsimd.partition_broadcast(c_b[:], cpos[:])
                return rstd_b, c_b

            with tc.tile_pool(name="ps_row1", bufs=2, space="PSUM") as prow:
                psum_mu = prow.tile([1, TOK], F32, tag="row")
                psum_s2 = prow.tile([1, TOK], F32, tag="row")
                for ch in range(CH):
                    eng = nc.vector if ch % 2 == 0 else nc.gpsimd
                    eng.tensor_tensor(
                        sq8[:, ch, :], x8_sb[:, ch, :], x8_sb[:, ch, :], OP.mult
                    )
                for ch in range(CH):
                    nc.tensor.matmul(
                        psum_mu[:],
                        ones8[:],
                        x8_sb[:, ch, :],
                        start=(ch == 0),
                        stop=(ch == CH - 1),
                    )
                for ch in range(CH):
                    nc.tensor.matmul(
                        psum_s2[:],
                        ones8[:],
                        sq8[:, ch, :],
                        start=(ch == 0),
                        stop=(ch == CH - 1),
                    )
                rstd1_b, c1_b = ln_rows(psum_mu, psum_s2, "1")

            h1 = bigs.tile([P, CH, TOK], F8, tag="h1")
            for ch in range(CH):
                eng = nc.vector if ch % 2 == 0 else nc.gpsimd
                t1 = work.tile([P, TOK], F32, tag="t1")
                eng.tensor_tensor(t1[:], xr_sb[:, ch, :], rstd1_b[:], OP.mult)
                eng.tensor_tensor(h1[:, ch, :], t1[:], c1_b[:], OP.subtract)

            # ---- QKV ----
            st2 = tc.tile_pool(name="ps_mm2", bufs=5, space="PSUM")
            ps_acc = st2.__enter__()

            def qkv_block(wt, i, m, consume):
                psum = ps_acc.tile([P, TOK], F32, tag="acc", name=f"ps_{m}")
                for j in range(CH // 2):
                    nc.tensor.matmul(
                        psum[:],
                        wt[:, i, 2 * j : 2 * j + 2, :],
                        h1[:, 2 * j : 2 * j + 2, :],
                        start=(j == 0),
                        stop=(j == CH // 2 - 1),
                        perf_mode=DR,
                    )
                consume(m, psum)

            def k_consume(m, psum):
                k8 = stg.tile([P, TOK], F8, tag="cp", name=f"k8_{m}")
                nc.scalar.activation(
                    out=k8[:],
                    in_=psum[:],
                    func=AF.Identity,
                    bias=biasqk_sb[:, 8 + m : 9 + m],
                    scale=0.5,
                )
                nc.scalar.dma_start(vk_in[m * P : (m + 1) * P, :], k8[:])

            # K blocks (g1 folded into weights on host; cols D..2D of qkv_w)
            wk_t = []
            for g in range(2):
                wt = wpool.tile([P, 4, CH, P], F8, tag="wq", name=f"wk{g}")
                nc.sync.dma_start(
                    wt[:],
                    wqk8[8 + 4 * g : 12 + 4 * g].rearrange("i p (ch o) -> p i ch o", ch=CH),
                )
                wk_t.append(wt)
            for m in range(CH):
                qkv_block(wk_t[m // 4], m % 4, m, k_consume)

            # V (token-major): lhsT = h1 chunk-pair, rhs = wv columns
            for ph in range(2):
                wv_t = wpool.tile([P, CH, TOK], F8, tag="wq", name=f"wv{ph}")
                nc.sync.dma_start(
                    wv_t[:], wv8[ph].rearrange("p (ch v) -> p ch v", ch=CH)
                )
                for tt_ in range(TOK // P):
                    psum = ps_acc.tile([P, TOK], F32, tag="acc", name=f"pv{ph}_{tt_}")
                    for j in range(CH // 2):
                        nc.tensor.matmul(
                            psum[:],
                            h1[:, 2 * j : 2 * j + 2, tt_ * P : (tt_ + 1) * P],
                            wv_t[:, 2 * j : 2 * j + 2, :],
                            start=(j == 0),
                            stop=(j == CH // 2 - 1),
                            perf_mode=DR,
                        )
                    v8 = stg.tile([P, TOK], F8, tag="cp", name=f"v8_{ph}_{tt_}")
                    nc.vector.tensor_scalar_mul(v8[:], psum[:], 0.5)
                    dst = vv_in[
                        tt_ * P : (tt_ + 1) * P,
                        ph * 8 * DHP : (ph + 1) * 8 * DHP,
                    ].rearrange("t (h c) -> t h c", c=DHP)[:, :, 0:DH]
                    nc.gpsimd.dma_start(
                        dst, v8[:].rearrange("t (h d) -> t h d", d=DH)
                    )

            nc.gpsimd.collective_compute(
                "AllGather",
                OP.bypass,
                ins=[kv_in[:].opt()],
                outs=[kv_out[:].opt()],
                replica_groups=REPLICA_GROUPS,
            )

            # Q blocks -> SBUF (chunk 8 duplicates chunk 7 for the hp=7 rhs pair)
            qT = bigs.tile([P, CH + 1, TOK], F8, tag="qT")

            def q_consume(m, psum):
                nc.vector.tensor_scalar(
                    out=qT[:, m, :],
                    in0=psum[:],
                    scalar1=0.5,
                    scalar2=biasqk_sb[:, m : m + 1],
                    op0=OP.mult,
                    op1=OP.add,
                )

            for g in range(2):
                wt = wpool.tile([P, 4, CH, P], F8, tag="wq", name=f"wq{g}")
                nc.sync.dma_start(
                    wt[:],
                    wqk8[4 * g : 4 * g + 4].rearrange("i p (ch o) -> p i ch o", ch=CH),
                )
                for i in range(4):
                    qkv_block(wt, i, 4 * g + i, q_consume)
            nc.sync.dma_start(qT[:, CH, :], qT[:, CH - 1, :])
            st2.__exit__(None, None, None)

            # ---- attention ----
            # kp: [P(2 heads' d), 2, KC, P] fp8; slot0 = K data, slot1 = zeros
            kp_t = []
            for i in range(2):
                t = kvz.tile([P, 2, KC, P], F8, tag="kp", name=f"kp{i}")
                nc.vector.memset(t[:, 1, :, :].bitcast(U8), 0)
                kp_t.append(t)
            # vfull: [P(key-in-chunk), KC, 16*(64+1)] fp8, ones pre-gathered
            vfull = bigs.tile([P, KC, H * DHP], F8, tag="vfull")
            for r in range(RANKS):
                src = (
                    kv_or[r : r + 1, KV_K:KV_SZ]
                    .rearrange("o (t v) -> o t v", v=H * DHP)[0]
                    .rearrange("(tc p) v -> p tc v", p=P)
                )
                nc.sync.dma_start(vfull[:, 4 * r : 4 * r + 4, :], src)

            def load_kp(hp, t):
                src = (
                    kv_or[:, 0:KV_K]
                    .rearrange("r (f t) -> r f t", t=TOK)[
                        :, hp * P : (hp + 1) * P, :
                    ]
                    .rearrange("r p t -> p r t")
                )
                nc.sync.dma_start(t[:, 0, :, :], src)

            ctxT = bigs.tile([P, CH, TOK], F8, tag="x8")  # reuse x8 region
            groups = [(0, 3), (3, 3), (6, 3), (9, 3), (12, 2), (14, 2)]
            pairs_after = {1: [0, 1, 2], 2: [3], 3: [4, 5], 4: [6], 5: [7]}

            att_pools = (
                tc.tile_pool(name="ps_s", bufs=2, space="PSUM"),
                tc.tile_pool(name="ps_ctx", bufs=2, space="PSUM"),
            )
            ps_spool = att_pools[0].__enter__()
            ps_ctx = att_pools[1].__enter__()
            eidx = 0
            for hp in range(H // 2):
                kp = kp_t[hp % 2]
                load_kp(hp, kp)
                for hh in range(2):
                    h = 2 * hp + hh
                    half = slice(hh * DH, hh * DH + DH)
                    qpair = qT[half, hp : hp + 2, :]
                    pt = ppool.tile([P, KC, TOK], F8, tag="pt", name=f"pt{h}")
                    psum_c = ps_ctx.tile([DH + 1, TOK], F32, tag="ctx")
                    for gi, (kc0, nb) in enumerate(groups):
                        ps_s = ps_spool.tile([P, 3 * TOK], F32, tag="s")
                        for j in range(nb):
                            nc.tensor.matmul(
                                ps_s[:, j * TOK : (j + 1) * TOK],
                                kp[half, :, kc0 + j, :],
                                qpair,
                                start=True,
                                stop=True,
                                perf_mode=DR,
                            )
                        dst = pt[:, kc0 : kc0 + nb, :]
                        # alternate Act/DVE so consecutive groups of a head
                        # never serialize on one engine (GPSIMD can't read
                        # PSUM per the BIR verifier); 4:2 toward Act since
                        # DVE also carries reciprocal + normalize
                        if (gi + h) % 2 == 0:
                            nc.scalar.activation(
                                out=dst,
                                in_=ps_s[:, : nb * TOK],
                                func=AF.Exp,
                                bias=negc_row[:],
                                scale=2.0**-11,
                            )
                        else:
                            nc.vector.tensor_scalar(
                                out=dst.bitcast(U8),
                                in0=ps_s[:, : nb * TOK],
                                scalar1=SA,
                                scalar2=SB,
                                op0=OP.mult,
                                op1=OP.add,
                            )
                        eidx += 1
                        for pj in pairs_after.get(gi, []):
                            nc.tensor.matmul(
                                psum_c[:],
                                vfull[
                                    :,
                                    2 * pj : 2 * pj + 2,
                                    h * DHP : h * DHP + DH + 1,
                                ],
                                pt[:, 2 * pj : 2 * pj + 2, :],
                                start=(pj == 0),
                                stop=(pj == KC // 2 - 1),
                                perf_mode=DR,
                            )
                    rrow = rows.tile([1, TOK], F32, tag="r", name=f"rr{h}")
                    nc.vector.reciprocal(rrow[:], psum_c[DH : DH + 1, :])
                    rb = bcpool.tile([DH, TOK], F32, tag="rb", name=f"rb{h}")
                    nc.gpsimd.partition_broadcast(rb[:], rrow[:])
                    nc.vector.tensor_tensor(
                        ctxT[half, hp, :], psum_c[0:DH, :], rb[:], OP.mult
                    )
            att_pools[1].__exit__(None, None, None)
            att_pools[0].__exit__(None, None, None)

            # ---- proj + residual ----
            st4 = tc.tile_pool(name="ps_mm4", bufs=5, space="PSUM")
            ps_mlp = st4.__enter__()
            x2 = bigs.tile([P, CH, TOK], F32R, tag="x2")
            for g in range(2):
                wt = wpool.tile([P, 4, CH, P], F8, tag="wq", name=f"wpj{g}")
                nc.sync.dma_start(
                    wt[:],
                    wp8[4 * g : 4 * g + 4].rearrange("i p (ch o) -> p i ch o", ch=CH),
                )
                for i in range(4):
                    m = 4 * g + i
                    psum = ps_mlp.tile([P, TOK], F32, tag="acc", name=f"pp_{m}")
                    for j in range(CH // 2):
                        nc.tensor.matmul(
                            psum[:],
                            wt[:, i, 2 * j : 2 * j + 2, :],
                            ctxT[:, 2 * j : 2 * j + 2, :],
                            start=(j == 0),
                            stop=(j == CH // 2 - 1),
                            perf_mode=DR,
                        )
                    attn_sb = stg.tile([P, TOK], F32, tag="stg", name=f"at_{m}")
                    nc.scalar.activation(
                        out=attn_sb[:],
                        in_=psum[:],
                        func=AF.Identity,
                        bias=biaspf_sb[:, m : m + 1],
                        scale=2.0**-9,
                    )
                    nc.gpsimd.tensor_tensor(
                        x2[:, m, :], attn_sb[:], xr_sb[:, m, :], OP.add
                    )  # f32r out: rounded on write for the LN2 stats matmul

            # ---- LN2 (fp32r stats on x2) ----
            with tc.tile_pool(name="ps_row2", bufs=2, space="PSUM") as prow:
                psum_mu2 = prow.tile([1, TOK], F32, tag="row")
                psum_s22 = prow.tile([1, TOK], F32, tag="row")
                for ch in range(CH):
                    nc.tensor.matmul(
                        psum_mu2[:],
                        ones_r[:],
                        x2[:, ch, :],
                        start=(ch == 0),
                        stop=(ch == CH - 1),
                    )
                    sq = work.tile([P, TOK], F32R, tag="t1", name=f"sq2_{ch}")
                    nc.gpsimd.tensor_tensor(
                        sq[:],
                        x2[:, ch, :].bitcast(F32),
                        x2[:, ch, :].bitcast(F32),
                        OP.mult,
                    )
                    nc.tensor.matmul(
                        psum_s22[:],
                        ones_r[:],
                        sq[:],
                        start=(ch == 0),
                        stop=(ch == CH - 1),
                    )
                rstd2_b, c2_b = ln_rows(psum_mu2, psum_s22, "2")

            # xn as fp8 hi/lo + duplicated-hi slot: [hi, hi_dup, lo]
            xnf = bigs.tile([P, CH, 3, TOK], F8, tag="xn")
            for ch in range(CH):
                t1 = work.tile([P, TOK], F32, tag="t1", name=f"t2_{ch}")
                nc.gpsimd.tensor_tensor(
                    t1[:], x2[:, ch, :].bitcast(F32), rstd2_b[:], OP.mult
                )
                xn32 = work.tile([P, TOK], F32, tag="xn32", bufs=2, name=f"x32_{ch}")
                nc.vector.tensor_tensor(xn32[:], t1[:], c2_b[:], OP.subtract)
                nc.scalar.activation(
                    out=xnf[:, ch, 0, :], in_=xn32[:], func=AF.Identity
                )
                nc.gpsimd.tensor_copy(
                    out=xnf[:, ch, 1, :], in_=xnf[:, ch, 0, :]
                )
                nc.vector.tensor_tensor(
                    xnf[:, ch, 2, :], xn32[:], xnf[:, ch, 0, :], OP.subtract
                )

            # ---- MLP: fc1 fp8 hi/lo "3-product" DoubleRow, fc2 bf16 ----
            # per chunk pair (c, d): [whi_c,wlo_c]x[xhi,xhidup], same for d,
            # then [whi_c,whi_d]x[xlo_c,xlo_d] (drops the negligible lo*lo)
            g2 = bigs.tile([P, HCH, 2, TOK], F8, tag="g")
            for g in range(CH):
                w1 = wpool.tile([P, 4, 2 * CH, P], F8, tag="w1", name=f"w1_{g}")
                nc.sync.dma_start(
                    w1[:],
                    w1f8[g].rearrange("p (i c o) -> p i c o", i=4, c=2 * CH),
                )
                for i in range(4):
                    m = 4 * g + i
                    psum = ps_mlp.tile([P, TOK], F32, tag="acc", name=f"p1_{m}")
                    for c2 in range(CH // 2):
                        c = 2 * c2
                        nc.tensor.matmul(
                            psum[:],
                            w1[:, i, 2 * c : 2 * c + 2, :],
                            xnf[:, c, 0:2, :],
                            start=(c2 == 0),
                            stop=False,
                            perf_mode=DR,
                        )
                        nc.tensor.matmul(
                            psum[:],
                            w1[:, i, 2 * c + 2 : 2 * c + 4, :],
                            xnf[:, c + 1, 0:2, :],
                            start=False,
                            stop=False,
                            perf_mode=DR,
                        )
                        whi = w1[:, i].rearrange(
                            "p (c two) k -> p c two k", two=2
                        )[:, c : c + 2, 0, :]
                        nc.tensor.matmul(
                            psum[:],
                            whi,
                            xnf[:, c : c + 2, 2, :],
                            start=False,
                            stop=(c2 == CH // 2 - 1),
                            perf_mode=DR,
                        )
                    g32 = work.tile([P, TOK], F32, tag="xn32", bufs=2, name=f"g32_{m}")
                    nc.scalar.activation(
                        out=g32[:],
                        in_=psum[:],
                        func=AF.Gelu,
                        bias=fc1b_sb[:, m : m + 1],
                        scale=2.0**-5,
                    )
                    nc.gpsimd.tensor_copy(out=g2[:, m, 0, :], in_=g32[:])
                    nc.vector.tensor_tensor(
                        g2[:, m, 1, :], g32[:], g2[:, m, 0, :], OP.subtract
                    )
            for m2 in range(CH):
                psum = ps_mlp.tile([P, TOK], F32, tag="acc", name=f"p2_{m2}")
                for hf in range(2):
                    w2 = wpool.tile(
                        [P, 3 * HCH // 2, P], F8, tag="w2", name=f"w2_{m2}_{hf}"
                    )
                    nc.sync.dma_start(
                        w2[:],
                        w2f8[m2][
                            :, hf * 3 * (HCH // 2) * P : (hf + 1) * 3 * (HCH // 2) * P
                        ].rearrange("p (c o) -> p c o", c=3 * HCH // 2),
                    )
                    w2lo = w2.rearrange("p (c three) k -> p c three k", three=3)
                    for c2 in range(HCH // 4):
                        c = 2 * c2
                        hc = hf * (HCH // 2) + c
                        nc.tensor.matmul(
                            psum[:],
                            w2[:, 3 * c : 3 * c + 2, :],
                            g2[:, hc, 0:2, :],
                            start=(hf == 0 and c2 == 0),
                            stop=False,
                            perf_mode=DR,
                        )
                        nc.tensor.matmul(
                            psum[:],
                            w2[:, 3 * c + 3 : 3 * c + 5, :],
                            g2[:, hc + 1, 0:2, :],
                            start=False,
                            stop=False,
                            perf_mode=DR,
                        )
                        nc.tensor.matmul(
                            psum[:],
                            w2lo[:, c : c + 2, 2, :],
                            g2[:, hc : hc + 2, 0, :],
                            start=False,
                            stop=(hf == 1 and c2 == HCH // 4 - 1),
                            perf_mode=DR,
                        )
                o_sb = stg.tile([P, TOK], F32, tag="stg", name=f"o_{m2}")
                nc.scalar.activation(
                    out=o_sb[:],
                    in_=psum[:],
                    func=AF.Identity,
                    bias=biaspf_sb[:, 8 + m2 : 9 + m2],
                    scale=2.0**-6,
                )
                o_f = stg.tile([P, TOK], F32, tag="of", bufs=2, name=f"of_{m2}")
                nc.vector.tensor_add(
                    out=o_f[:], in0=o_sb[:], in1=x2[:, m2, :].bitcast(F32)
                )
                nc.sync.dma_start(outT[m2 * P : (m2 + 1) * P, :], o_f[:])
            st4.__exit__(None, None, None)

    if do_compile:
        nc.compile()
    return nc


_CACHE = {}


def _get_program():
    if "nc" not in _CACHE:
        _CACHE["nc"] = build_program()
    return _CACHE["nc"]


def _prep_inputs(inputs):
    E4 = ml_dtypes.float8_e4m3
    x = np.asarray(inputs["x"], np.float32)
    g1 = np.asarray(inputs["ln1_g"], np.float32)
    b1 = np.asarray(inputs["ln1_b"], np.float32)
    g2 = np.asarray(inputs["ln2_g"], np.float32)
    b2 = np.asarray(inputs["ln2_b"], np.float32)
    qkv_w = np.asarray(inputs["qkv_w"], np.float32) * g1[None, :]
    proj_w = np.asarray(inputs["proj_w"], np.float32)
    fc1_w = np.asarray(inputs["fc1_w"], np.float32) * g2[None, :]
    fc2_w = np.asarray(inputs["fc2_w"], np.float32)

    qkv_bias = np.asarray(inputs["qkv_w"], np.float32) @ b1  # [3D]
    assert np.abs(qkv_bias[2 * D :]).max() == 0.0, "nonzero ln1_b v-bias unsupported"

    def wtile8(w, blocks):
        """w [O, D] -> [nb, P, CH*P] fp8 with [m, p, ch*128+o] = 32*w[m*128+o, ch*128+p]."""
        out = np.empty((len(blocks), P, CH * P), E4)
        for bi, m in enumerate(blocks):
            blk = w[m * P : (m + 1) * P, :] * 32.0  # [o 128, c 1024]
            out[bi] = (
                blk.reshape(P, CH, P).transpose(2, 1, 0).reshape(P, CH * P)
            ).astype(E4)
        return out

    def wtile8_hl(w, nb, batch):
        """w [O, D] (pre-scaled) -> [nb//batch, P, batch*2*chn*P] fp8 hi/lo
        tiles: slot (2c+s) holds hi (s=0) / lo residual (s=1) of chunk c."""
        chn = w.shape[1] // P
        out = np.empty((nb // batch, P, batch * 2 * chn * P), E4)
        for g in range(nb // batch):
            t = np.empty((P, batch, 2 * chn, P), E4)
            for i in range(batch):
                m = g * batch + i
                blk = w[m * P : (m + 1) * P, :]  # [o, c]
                wt = blk.reshape(P, chn, P).transpose(2, 1, 0)  # [p, c, o]
                hi = wt.astype(E4)
                lo = (wt - hi.astype(np.float32)).astype(E4)
                t[:, i, 0::2, :] = hi
                t[:, i, 1::2, :] = lo
            out[g] = t.reshape(P, -1)
        return out

    def wtile8_t3(w):
        """w [O, D] (pre-scaled) -> [O//P, P, 3*chn*P] fp8 (hi, hi, lo)."""
        chn = w.shape[1] // P
        out = np.empty((w.shape[0] // P, P, 3 * chn * P), E4)
        for m in range(w.shape[0] // P):
            blk = w[m * P : (m + 1) * P, :]
            wt = blk.reshape(P, chn, P).transpose(2, 1, 0)  # [p, c, o]
            hi = wt.astype(E4)
            lo = (wt - hi.astype(np.float32)).astype(E4)
            t = np.empty((P, chn, 3, P), E4)
            t[:, :, 0, :] = hi
            t[:, :, 1, :] = hi
            t[:, :, 2, :] = lo
            out[m] = t.reshape(P, -1)
        return out

    def wtile_bf(w, nb, batch):
        """w [O, D] -> [nb//batch, P, batch*CH*P] bf16 tiles."""
        out = np.empty((nb // batch, P, batch * (w.shape[1] // P) * P), ml_dtypes.bfloat16)
        chn = w.shape[1] // P
        for g in range(nb // batch):
            t = np.empty((P, batch, chn, P), np.float32)
            for i in range(batch):
                m = g * batch + i
                blk = w[m * P : (m + 1) * P, :]  # [o, c]
                t[:, i] = blk.reshape(P, chn, P).transpose(2, 1, 0)
            out[g] = t.reshape(P, -1).astype(ml_dtypes.bfloat16)
        return out

    # V weights token-major: [ph, p, ch*512+vc] = 32*qkv_w'[2D+ph*512+vc, ch*128+p]
    wv = np.empty((2, P, CH * TOK), E4)
    for ph in range(2):
        blk = qkv_w[2 * D + ph * TOK : 2 * D + (ph + 1) * TOK, :] * 32.0  # [vc, c]
        wv[ph] = blk.reshape(TOK, CH, P).transpose(2, 1, 0).reshape(P, CH * TOK).astype(E4)

    bqk = np.zeros((P, 16), np.float32)
    bqk[:, 0:8] = _stripe(16.0 * qkv_bias[0:D])
    bqk[:, 8:16] = _stripe(16.0 * qkv_bias[D : 2 * D])
    bpf = np.zeros((P, 16), np.float32)
    bpf[:, 0:8] = _stripe(inputs["proj_b"])
    bpf[:, 8:16] = _stripe(inputs["fc2_b"])

    shared = {
        "wqk8": wtile8(qkv_w, list(range(16))),
        "wv8": wv,
        "wp8": wtile8(proj_w, list(range(CH))),
        "w1f8": wtile8_hl(fc1_w * 32.0, HCH, 4),
        "w2f8": wtile8_t3(fc2_w * 64.0),
        "biasqk": bqk,
        "biaspf": bpf,
        "fc1b": _stripe(
            np.asarray(inputs["fc1_b"], np.float32)
            + np.asarray(inputs["fc1_w"], np.float32) @ b2
        ),
    }
    in_maps = []
    for c in range(NCORES):
        b, blk = divmod(c, RANKS)
        xblk = x[b, blk * TOK : (blk + 1) * TOK, :]  # [TOK, D]
        xt = round_fp32r(np.ascontiguousarray(xblk.T))
        m = dict(shared)
        m["xT"] = xt
        m["x8T"] = xt.astype(E4)
        in_maps.append(m)
    return in_maps


def _assemble(results):
    out = np.empty((B, N, D), dtype=np.float32)
    for c in range(NCORES):
        b, blk = divmod(c, RANKS)
        out[b, blk * TOK : (blk + 1) * TOK, :] = results[c]["outT"].T
    return out


def run_device(inputs, **kwargs):
    nc = _get_program()
    in_maps = _prep_inputs(inputs)
    res = run_bass_kernel_spmd(nc, in_maps, core_ids=list(range(NCORES)), **kwargs)
    return _assemble(res.results), res


def kernel(**inputs) -> np.ndarray:
    out, _ = run_device(inputs)
    return out


# revision 12
# speedup vs baseline: 1.0427x; 1.0262x over previous
"""Trainium2 Bass kernel v2: fp8 DoubleRow attention + bf16 MLP.

Sharding: sequence-parallel over 8 cores (512 tokens each, batch = core//4).
One 4-rank AllGather carries fp8 K (feature-major) + fp8 V (token-major).

Precision plan (validated in numpy, max_rel ~1.0e-2 vs 2e-2 gate):
  - weights qkv/proj: e4m3 x32 host-scaled; fc1/fc2: bf16 (MLP dominates error)
  - h1/q/k/v/pt/ctx: e4m3 (q,k,v at sigma~16 via 0.5 consume scale)
  - softmax exp: constant shift C=4 (cancels in normalize); split between
    Act (native Exp -> fp8) and DVE (Schraudolph: psum*a+b -> uint8 whose
    bit pattern IS e4m3 2^x; floor-vs-round ambiguity is a constant factor
    that cancels in the softmax normalize)
  - x residual fp32r; LN stats via ones-matmul (fp8 DoubleRow for LN1 on
    host-provided x8, fp32r for LN2)
DoubleRow pair slots: chunk pairs for QKV/ctx/proj; (k, zeros) for scores
(d=64 contraction cannot pair; zero slot makes the 0.5 cyc/row rate legal).
"""
import sys

sys.path.insert(0, "/opt/trn_rl_repo")
import numpy as np
import ml_dtypes
import concourse.bass as bass
import concourse.mybir as mybir
import concourse.tile as tile
from concourse import bacc
from concourse.bass_utils import run_bass_kernel_spmd

B, N, D = 2, 2048, 1024
H, DH = 16, 64
HID = 4096
NCORES = 8
TOK = (B * N) // NCORES  # 512
EPS = 1e-5
SCALE = DH**-0.5
P = 128
CH = D // P  # 8
KC = N // P  # 16
HCH = HID // P  # 32
RANKS = 4
CSH = 4.0  # exp arg shift, cancels in softmax
LN2_ = float(np.log(2.0))
# Schraudolph uint8-as-e4m3: y = psum * SA + SB
SA = 8.0 * (2.0**-11) / LN2_
SB = 56.5 - 8.0 * CSH / LN2_

F32 = mybir.dt.float32
F32R = mybir.dt.float32r
F8 = mybir.dt.float8e4
BF = mybir.dt.bfloat16
U8 = mybir.dt.uint8
AF = mybir.ActivationFunctionType
OP = mybir.AluOpType
DR = mybir.MatmulPerfMode.DoubleRow

REPLICA_GROUPS = [[0, 1, 2, 3], [4, 5, 6, 7]]

KV_K = D * TOK  # bytes of K region (fp8 feature-major [1024, 512])
DHP = DH + 16  # per-head stride in V region: 64 v + 1 ones + 15 pad
# (dual-fp8 LdWeights requires 16B-aligned weight base addresses)
KV_V = TOK * (H * DHP)  # V region [512, 1280]
KV_SZ = KV_K + KV_V


def round_fp32r(x: np.ndarray) -> np.ndarray:
    u = np.ascontiguousarray(x, dtype=np.float32).view(np.uint32)
    u = (u + 0x7FF + ((u >> 12) & 1)) & np.uint32(0xFFFFF000)
    return u.view(np.float32)


def _stripe(v: np.ndarray) -> np.ndarray:
    """[M] -> [P, M//P] with col m, part p = v[m*128+p]."""
    return np.ascontiguousarray(np.asarray(v, np.float32).reshape(-1, P).T)


def build_program(do_compile=True):
    nc = bacc.Bacc("TRN2", target_bir_lowering=False, debug=False, num_devices=NCORES)

    xT = nc.dram_tensor("xT", [D, TOK], F32, kind="ExternalInput").ap()
    x8T = nc.dram_tensor("x8T", [D, TOK], F8, kind="ExternalInput").ap()
    # weight tiles, DMA-contiguous per partition
    wqk8 = nc.dram_tensor("wqk8", [16, P, CH * P], F8, kind="ExternalInput").ap()
    wv8 = nc.dram_tensor("wv8", [2, P, CH * TOK], F8, kind="ExternalInput").ap()
    wp8 = nc.dram_tensor("wp8", [CH, P, CH * P], F8, kind="ExternalInput").ap()
    # fc1 weights as fp8 hi/lo pairs [.., (chunk, hi/lo), out]; fc2 stays bf16
    w1f8 = nc.dram_tensor(
        "w1f8", [CH, P, 4 * 2 * CH * P], F8, kind="ExternalInput"
    ).ap()
    # fc2 weights fp8 (hi, hi_dup, lo) triples per chunk (dups host-side)
    w2f8 = nc.dram_tensor(
        "w2f8", [CH, P, 3 * HCH * P], F8, kind="ExternalInput"
    ).ap()
    biasqk = nc.dram_tensor("biasqk", [P, 16], F32, kind="ExternalInput").ap()
    biaspf = nc.dram_tensor("biaspf", [P, 16], F32, kind="ExternalInput").ap()
    fc1b = nc.dram_tensor("fc1b", [P, HCH], F32, kind="ExternalInput").ap()
    outT = nc.dram_tensor("outT", [D, TOK], F32, kind="ExternalOutput").ap()

    with tile.TileContext(nc) as tc:
        with (
            tc.tile_pool(name="consts", bufs=1) as consts,
            tc.tile_pool(name="bigs", bufs=1) as bigs,
            tc.tile_pool(name="work", bufs=3) as work,
            tc.tile_pool(name="wpool", bufs=2) as wpool,
            tc.tile_pool(name="kvz", bufs=2) as kvz,
            tc.tile_pool(name="pp", bufs=2) as ppool,
            tc.tile_pool(name="rows", bufs=3) as rows,
            tc.tile_pool(name="bc", bufs=2) as bcpool,
            tc.tile_pool(name="stg", bufs=2) as stg,
            tc.tile_pool(name="dram", bufs=1, space="DRAM") as dram,
        ):
            ones8 = consts.tile([P, 1], F8)
            nc.vector.memset(ones8[:].bitcast(U8), 0x38)  # e4m3 1.0
            ones_r = consts.tile([P, 1], F32R)
            nc.vector.memset(ones_r[:].bitcast(F32), 1.0)
            eps_row = consts.tile([1, 1], F32, tag="eps")
            nc.vector.memset(eps_row[:], EPS)
            negc_row = consts.tile([P, 1], F32, tag="negc")
            nc.vector.memset(negc_row[:], -CSH)
            onesv = consts.tile([P, 4, H], F8, tag="onesv")
            nc.vector.memset(onesv[:].bitcast(U8), 0x38)
            biasqk_sb = consts.tile([P, 16], F32, tag="bqk")
            biaspf_sb = consts.tile([P, 16], F32, tag="bpf")
            fc1b_sb = consts.tile([P, HCH], F32, tag="b1")
            nc.sync.dma_start(biasqk_sb[:], biasqk[:])
            nc.sync.dma_start(biaspf_sb[:], biaspf[:])
            nc.sync.dma_start(fc1b_sb[:], fc1b[:])

            kv_in = dram.tile([KV_SZ], F8, tag="kvin")
            kv_out = dram.tile([RANKS * KV_SZ], F8, tag="kvout")
            vk_in = kv_in[0:KV_K].rearrange("(f t) -> f t", t=TOK)
            vv_in = kv_in[KV_K:KV_SZ].rearrange(
                "(t v) -> t v", v=H * DHP
            )
            kv_or = kv_out[:].rearrange("(r x) -> r x", r=RANKS)
            # ones column of the V region: written up-front (no data deps)
            for tc_ in range(RANKS):
                ones_dst = vv_in[tc_ * P : (tc_ + 1) * P, :].rearrange(
                    "p (h c) -> p h c", c=DHP
                )[:, :, DH : DH + 1]
                nc.sync.dma_start(
                    ones_dst, onesv[:, tc_, :].rearrange("p (h c) -> p h c", c=1)
                )

            # ---- LN1 ---- (x8 first: stats depend on it; xr arrives later)
            x8_sb = bigs.tile([P, CH, TOK], F8, tag="x8")
            nc.sync.dma_start(
                x8_sb[:], x8T.rearrange("(ch p) t -> p ch t", p=P)
            )
            xr_sb = bigs.tile([P, CH, TOK], F32, tag="xr")
            nc.sync.dma_start(
                xr_sb[:], xT.rearrange("(ch p) t -> p ch t", p=P)
            )
            sq8 = bigs.tile([P, CH, TOK], F8, tag="h1")  # released before h1

            def ln_rows(psum_mu, psum_s2, name):
                mu = rows.tile([1, TOK], F32, tag="r", name=f"mu{name}")
                nc.vector.tensor_scalar_mul(mu[:], psum_mu[:], 1.0 / D)
                var = rows.tile([1, TOK], F32, tag="r", name=f"va{name}")
                nc.vector.tensor_tensor(var[:], mu[:], mu[:], OP.mult)
                ex2 = rows.tile([1, TOK], F32, tag="r", name=f"e2{name}")
                nc.vector.tensor_scalar_mul(ex2[:], psum_s2[:], 1.0 / D)
                nc.vector.tensor_sub(var[:], ex2[:], var[:])
                rstd = rows.tile([1, TOK], F32, tag="r", name=f"rs{name}")
                nc.scalar.activation(
                    out=rstd[:], in_=var[:], func=AF.Sqrt, bias=eps_row[:]
                )
                nc.vector.reciprocal(rstd[:], rstd[:])
                cpos = rows.tile([1, TOK], F32, tag="r", name=f"cp{name}")
                nc.vector.tensor_tensor(cpos[:], mu[:], rstd[:], OP.mult)
                rstd_b = bcpool.tile([P, TOK], F32, tag="bc", name=f"rb{name}")
                nc.gpsimd.partition_broadcast(rstd_b[:], rstd[:])
                c_b = bcpool.tile([P, TOK], F32, tag="bc", name=f"cb{name}")
                nc.gpsimd.partition_broadcast(c_b[:], cpos[:])
                return rstd_b, c_b

            with tc.tile_pool(name="ps_row1", bufs=2, space="PSUM") as prow:
                psum_mu = prow.tile([1, TOK], F32, tag="row")
                psum_s2 = prow.tile([1, TOK], F32, tag="row")
                for ch in range(CH):
                    eng = nc.vector if ch % 2 == 0 else nc.gpsimd
                    eng.tensor_tensor(
                        sq8[:, ch, :], x8_sb[:, ch, :], x8_sb[:, ch, :], OP.mult
                    )
                for ch in range(CH):
                    nc.tensor.matmul(
                        psum_mu[:],
                        ones8[:],
                        x8_sb[:, ch, :],
                        start=(ch == 0),
                        stop=(ch == CH - 1),
                    )
                for ch in range(CH):
                    nc.tensor.matmul(
                        psum_s2[:],
                        ones8[:],
                        sq8[:, ch, :],
                        start=(ch == 0),
                        stop=(ch == CH - 1),
                    )
                rstd1_b, c1_b = ln_rows(psum_mu, psum_s2, "1")

            h1 = bigs.tile([P, CH, TOK], F8, tag="h1")
            for ch in range(CH):
                eng = nc.vector if ch % 2 == 0 else nc.gpsimd
                t1 = work.tile([P, TOK], F32, tag="t1")
                eng.tensor_tensor(t1[:], xr_sb[:, ch, :], rstd1_b[:], OP.mult)
                eng.tensor_tensor(h1[:, ch, :], t1[:], c1_b[:], OP.subtract)

            # ---- QKV ----
            st2 = tc.tile_pool(name="ps_mm2", bufs=5, space="PSUM")
            ps_acc = st2.__enter__()

            def qkv_block(wt, i, m, consume):
                psum = ps_acc.tile([P, TOK], F32, tag="acc", name=f"ps_{m}")
                for j in range(CH // 2):
                    nc.tensor.matmul(
                        psum[:],
                        wt[:, i, 2 * j : 2 * j + 2, :],
                        h1[:, 2 * j : 2 * j + 2, :],
                        start=(j == 0),
                        stop=(j == CH // 2 - 1),
                        perf_mode=DR,
                    )
                consume(m, psum)

            def k_consume(m, psum):
                k8 = stg.tile([P, TOK], F8, tag="cp", name=f"k8_{m}")
                nc.scalar.activation(
                    out=k8[:],
                    in_=psum[:],
                    func=AF.Identity,
                    bias=biasqk_sb[:, 8 + m : 9 + m],
                    scale=0.5,
                )
                nc.scalar.dma_start(vk_in[m * P : (m + 1) * P, :], k8[:])

            # K blocks (g1 folded into weights on host; cols D..2D of qkv_w)
            wk_t = []
            for g in range(2):
                wt = wpool.tile([P, 4, CH, P], F8, tag="wq", name=f"wk{g}")
                nc.sync.dma_start(
                    wt[:],
                    wqk8[8 + 4 * g : 12 + 4 * g].rearrange("i p (ch o) -> p i ch o", ch=CH),
                )
                wk_t.append(wt)
            for m in range(CH):
                qkv_block(wk_t[m // 4], m % 4, m, k_consume)

            # V (token-major): lhsT = h1 chunk-pair, rhs = wv columns
            for ph in range(2):
                wv_t = wpool.tile([P, CH, TOK], F8, tag="wq", name=f"wv{ph}")
                nc.sync.dma_start(
                    wv_t[:], wv8[ph].rearrange("p (ch v) -> p ch v", ch=CH)
                )
                for tt_ in range(TOK // P):
                    psum = ps_acc.tile([P, TOK], F32, tag="acc", name=f"pv{ph}_{tt_}")
                    for j in range(CH // 2):
                        nc.tensor.matmul(
                            psum[:],
                            h1[:, 2 * j : 2 * j + 2, tt_ * P : (tt_ + 1) * P],
                            wv_t[:, 2 * j : 2 * j + 2, :],
                            start=(j == 0),
                            stop=(j == CH // 2 - 1),
                            perf_mode=DR,
                        )
                    v8 = stg.tile([P, TOK], F8, tag="cp", name=f"v8_{ph}_{tt_}")
                    nc.vector.tensor_scalar_mul(v8[:], psum[:], 0.5)
                    dst = vv_in[
                        tt_ * P : (tt_ + 1) * P,
                        ph * 8 * DHP : (ph + 1) * 8 * DHP,
                    ].rearrange("t (h c) -> t h c", c=DHP)[:, :, 0:DH]
                    nc.gpsimd.dma_start(
                        dst, v8[:].rearrange("t (h d) -> t h d", d=DH)
                    )

            nc.gpsimd.collective_compute(
                "AllGather",
                OP.bypass,
                ins=[kv_in[:].opt()],
                outs=[kv_out[:].opt()],
                replica_groups=REPLICA_GROUPS,
            )

            # Q blocks -> SBUF (chunk 8 duplicates chunk 7 for the hp=7 rhs pair)
            qT = bigs.tile([P, CH + 1, TOK], F8, tag="qT")

            def q_consume(m, psum):
                nc.vector.tensor_scalar(
                    out=qT[:, m, :],
                    in0=psum[:],
                    scalar1=0.5,
                    scalar2=biasqk_sb[:, m : m + 1],
                    op0=OP.mult,
                    op1=OP.add,
                )

            for g in range(2):
                wt = wpool.tile([P, 4, CH, P], F8, tag="wq", name=f"wq{g}")
                nc.sync.dma_start(
                    wt[:],
                    wqk8[4 * g : 4 * g + 4].rearrange("i p (ch o) -> p i ch o", ch=CH),
                )
                for i in range(4):
                    qkv_block(wt, i, 4 * g + i, q_consume)
            nc.sync.dma_start(qT[:, CH, :], qT[:, CH - 1, :])
            st2.__exit__(None, None, None)

            # ---- attention ----
            # kp: [P(2 heads' d), 2, KC, P] fp8; slot0 = K data, slot1 = zeros
            kp_t = []
            for i in range(2):
                t = kvz.tile([P, 2, KC, P], F8, tag="kp", name=f"kp{i}")
                nc.vector.memset(t[:, 1, :, :].bitcast(U8), 0)
                kp_t.append(t)
            # vfull: [P(key-in-chunk), KC, 16*(64+1)] fp8, ones pre-gathered
            vfull = bigs.tile([P, KC, H * DHP], F8, tag="vfull")
            for r in range(RANKS):
                src = (
                    kv_or[r : r + 1, KV_K:KV_SZ]
                    .rearrange("o (t v) -> o t v", v=H * DHP)[0]
                    .rearrange("(tc p) v -> p tc v", p=P)
                )
                nc.sync.dma_start(vfull[:, 4 * r : 4 * r + 4, :], src)

            def load_kp(hp, t):
                src = (
                    kv_or[:, 0:KV_K]
                    .rearrange("r (f t) -> r f t", t=TOK)[
                        :, hp * P : (hp + 1) * P, :
                    ]
                    .rearrange("r p t -> p r t")
                )
                nc.sync.dma_start(t[:, 0, :, :], src)

            ctxT = bigs.tile([P, CH, TOK], F8, tag="x8")  # reuse x8 region
            groups = [(2 * j, 2) for j in range(8)]
            pairs_after = {j: [j] for j in range(8)}

            att_pools = (
                tc.tile_pool(name="ps_s", bufs=2, space="PSUM"),
                tc.tile_pool(name="ps_ctx", bufs=2, space="PSUM"),
            )
            ps_spool = att_pools[0].__enter__()
            ps_ctx = att_pools[1].__enter__()
            eidx = 0
            for hp in range(H // 2):
                kp = kp_t[hp % 2]
                load_kp(hp, kp)
                for hh in range(2):
                    h = 2 * hp + hh
                    half = slice(hh * DH, hh * DH + DH)
                    qpair = qT[half, hp : hp + 2, :]
                    pt = ppool.tile([P, KC, TOK], F8, tag="pt", name=f"pt{h}")
                    psum_c = ps_ctx.tile([DH + 1, TOK], F32, tag="ctx")
                    for gi, (kc0, nb) in enumerate(groups):
                        ps_s = ps_spool.tile([P, 2 * TOK], F32, tag="s", bufs=3)
                        for j in range(nb):
                            nc.tensor.matmul(
                                ps_s[:, j * TOK : (j + 1) * TOK],
                                kp[half, :, kc0 + j, :],
                                qpair,
                                start=True,
                                stop=True,
                                perf_mode=DR,
                            )
                        dst = pt[:, kc0 : kc0 + nb, :]
                        # alternate Act/DVE so consecutive groups of a head
                        # never serialize on one engine (GPSIMD can't read
                        # PSUM per the BIR verifier); 4:2 toward Act since
                        # DVE also carries reciprocal + normalize
                        if (gi + h) % 2 == 0:
                            nc.scalar.activation(
                                out=dst,
                                in_=ps_s[:, : nb * TOK],
                                func=AF.Exp,
                                bias=negc_row[:],
                                scale=2.0**-11,
                            )
                        else:
                            nc.vector.tensor_scalar(
                                out=dst.bitcast(U8),
                                in0=ps_s[:, : nb * TOK],
                                scalar1=SA,
                                scalar2=SB,
                                op0=OP.mult,
                                op1=OP.add,
                            )
                        eidx += 1
                        for pj in pairs_after.get(gi, []):
                            nc.tensor.matmul(
                                psum_c[:],
                                vfull[
                                    :,
                                    2 * pj : 2 * pj + 2,
                                    h * DHP : h * DHP + DH + 1,
                                ],
                                pt[:, 2 * pj : 2 * pj + 2, :],
                                start=(pj == 0),
                                stop=(pj == KC // 2 - 1),
                                perf_mode=DR,
                            )
                    rrow = rows.tile([1, TOK], F32, tag="r", name=f"rr{h}")
                    nc.vector.reciprocal(rrow[:], psum_c[DH : DH + 1, :])
                    rb = bcpool.tile([DH, TOK], F32, tag="rb", name=f"rb{h}")
                    nc.gpsimd.partition_broadcast(rb[:], rrow[:])
                    nc.vector.tensor_tensor(
                        ctxT[half, hp, :], psum_c[0:DH, :], rb[:], OP.mult
                    )
            att_pools[1].__exit__(None, None, None)
            att_pools[0].__exit__(None, None, None)

            # ---- proj + residual ----
            st4 = tc.tile_pool(name="ps_mm4", bufs=5, space="PSUM")
            ps_mlp = st4.__enter__()
            x2 = bigs.tile([P, CH, TOK], F32R, tag="x2")
            for g in range(2):
                wt = wpool.tile([P, 4, CH, P], F8, tag="wq", name=f"wpj{g}")
                nc.sync.dma_start(
                    wt[:],
                    wp8[4 * g : 4 * g + 4].rearrange("i p (ch o) -> p i ch o", ch=CH),
                )
                for i in range(4):
                    m = 4 * g + i
                    psum = ps_mlp.tile([P, TOK], F32, tag="acc", name=f"pp_{m}")
                    for j in range(CH // 2):
                        nc.tensor.matmul(
                            psum[:],
                            wt[:, i, 2 * j : 2 * j + 2, :],
                            ctxT[:, 2 * j : 2 * j + 2, :],
                            start=(j == 0),
                            stop=(j == CH // 2 - 1),
                            perf_mode=DR,
                        )
                    attn_sb = stg.tile([P, TOK], F32, tag="stg", name=f"at_{m}")
                    nc.scalar.activation(
                        out=attn_sb[:],
                        in_=psum[:],
                        func=AF.Identity,
                        bias=biaspf_sb[:, m : m + 1],
                        scale=2.0**-9,
                    )
                    nc.gpsimd.tensor_tensor(
                        x2[:, m, :], attn_sb[:], xr_sb[:, m, :], OP.add
                    )  # f32r out: rounded on write for the LN2 stats matmul

            # ---- LN2 (fp32r stats on x2) ----
            with tc.tile_pool(name="ps_row2", bufs=2, space="PSUM") as prow:
                psum_mu2 = prow.tile([1, TOK], F32, tag="row")
                psum_s22 = prow.tile([1, TOK], F32, tag="row")
                for ch in range(CH):
                    nc.tensor.matmul(
                        psum_mu2[:],
                        ones_r[:],
                        x2[:, ch, :],
                        start=(ch == 0),
                        stop=(ch == CH - 1),
                    )
                    sq = work.tile([P, TOK], F32R, tag="t1", name=f"sq2_{ch}")
                    nc.gpsimd.tensor_tensor(
                        sq[:],
                        x2[:, ch, :].bitcast(F32),
                        x2[:, ch, :].bitcast(F32),
                        OP.mult,
                    )
                    nc.tensor.matmul(
                        psum_s22[:],
                        ones_r[:],
                        sq[:],
                        start=(ch == 0),
                        stop=(ch == CH - 1),
                    )
                rstd2_b, c2_b = ln_rows(psum_mu2, psum_s22, "2")

            # xn as fp8 hi/lo + duplicated-hi slot: [hi, hi_dup, lo]
            xnf = bigs.tile([P, CH, 3, TOK], F8, tag="xn")
            for ch in range(CH):
                t1 = work.tile([P, TOK], F32, tag="t1", name=f"t2_{ch}")
                nc.gpsimd.tensor_tensor(
                    t1[:], x2[:, ch, :].bitcast(F32), rstd2_b[:], OP.mult
                )
                xn32 = work.tile([P, TOK], F32, tag="xn32", bufs=2, name=f"x32_{ch}")
                nc.vector.tensor_tensor(xn32[:], t1[:], c2_b[:], OP.subtract)
                nc.scalar.activation(
                    out=xnf[:, ch, 0, :], in_=xn32[:], func=AF.Identity
                )
                nc.gpsimd.tensor_copy(
                    out=xnf[:, ch, 1, :], in_=xnf[:, ch, 0, :]
                )
                nc.vector.tensor_tensor(
                    xnf[:, ch, 2, :], xn32[:], xnf[:, ch, 0, :], OP.subtract
                )

            # ---- MLP: fc1 fp8 hi/lo "3-product" DoubleRow, fc2 bf16 ----
            # per chunk pair (c, d): [whi_c,wlo_c]x[xhi,xhidup], same for d,
            # then [whi_c,whi_d]x[xlo_c,xlo_d] (drops the negligible lo*lo)
            g2 = bigs.tile([P, HCH, 2, TOK], F8, tag="g")
            for g in range(CH):
                w1 = wpool.tile([P, 4, 2 * CH, P], F8, tag="w1", name=f"w1_{g}")
                nc.sync.dma_start(
                    w1[:],
                    w1f8[g].rearrange("p (i c o) -> p i c o", i=4, c=2 * CH),
                )
                for i in range(4):
                    m = 4 * g + i
                    psum = ps_mlp.tile([P, TOK], F32, tag="acc", name=f"p1_{m}")
                    for c2 in range(CH // 2):
                        c = 2 * c2
                        nc.tensor.matmul(
                            psum[:],
                            w1[:, i, 2 * c : 2 * c + 2, :],
                            xnf[:, c, 0:2, :],
                            start=(c2 == 0),
                            stop=False,
                            perf_mode=DR,
                        )
                        nc.tensor.matmul(
                            psum[:],
                            w1[:, i, 2 * c + 2 : 2 * c + 4, :],
                            xnf[:, c + 1, 0:2, :],
                            start=False,
                            stop=False,
                            perf_mode=DR,
                        )
                        whi = w1[:, i].rearrange(
                            "p (c two) k -> p c two k", two=2
                        )[:, c : c + 2, 0, :]
                        nc.tensor.matmul(
                            psum[:],
                            whi,
                            xnf[:, c : c + 2, 2, :],
                            start=False,
                            stop=(c2 == CH // 2 - 1),
                            perf_mode=DR,
                        )
                    g32 = work.tile([P, TOK], F32, tag="xn32", bufs=2, name=f"g32_{m}")
                    nc.scalar.activation(
                        out=g32[:],
                        in_=psum[:],
                        func=AF.Gelu,
                        bias=fc1b_sb[:, m : m + 1],
                        scale=2.0**-5,
                    )
                    nc.gpsimd.tensor_copy(out=g2[:, m, 0, :], in_=g32[:])
                    nc.vector.tensor_tensor(
                        g2[:, m, 1, :], g32[:], g2[:, m, 0, :], OP.subtract
                    )
            for m2 in range(CH):
                psum = ps_mlp.tile([P, TOK], F32, tag="acc", name=f"p2_{m2}")
                for hf in range(2):
                    w2 = wpool.tile(
                        [P, 3 * HCH // 2, P], F8, tag="w2", name=f"w2_{m2}_{hf}"
                    )
                    nc.sync.dma_start(
                        w2[:],
                        w2f8[m2][
                            :, hf * 3 * (HCH // 2) * P : (hf + 1) * 3 * (HCH // 2) * P
                        ].rearrange("p (c o) -> p c o", c=3 * HCH // 2),
                    )
                    w2lo = w2.rearrange("p (c three) k -> p c three k", three=3)
                    for c2 in range(HCH // 4):
                        c = 2 * c2
                        hc = hf * (HCH // 2) + c
                        nc.tensor.matmul(
                            psum[:],
                            w2[:, 3 * c : 3 * c + 2, :],
                            g2[:, hc, 0:2, :],
                            start=(hf == 0 and c2 == 0),
                            stop=False,
                            perf_mode=DR,
                        )
                        nc.tensor.matmul(
                            psum[:],
                            w2[:, 3 * c + 3 : 3 * c + 5, :],
                            g2[:, hc + 1, 0:2, :],
                            start=False,
                            stop=False,
                            perf_mode=DR,
                        )
                        nc.tensor.matmul(
                            psum[:],
                            w2lo[:, c : c + 2, 2, :],
                            g2[:, hc : hc + 2, 0, :],
                            start=False,
                            stop=(hf == 1 and c2 == HCH // 4 - 1),
                            perf_mode=DR,
                        )
                o_sb = stg.tile([P, TOK], F32, tag="stg", name=f"o_{m2}")
                nc.scalar.activation(
                    out=o_sb[:],
                    in_=psum[:],
                    func=AF.Identity,
                    bias=biaspf_sb[:, 8 + m2 : 9 + m2],
                    scale=2.0**-6,
                )
                o_f = stg.tile([P, TOK], F32, tag="of", bufs=2, name=f"of_{m2}")
                nc.vector.tensor_add(
                    out=o_f[:], in0=o_sb[:], in1=x2[:, m2, :].bitcast(F32)
                )
                nc.sync.dma_start(outT[m2 * P : (m2 + 1) * P, :], o_f[:])
            st4.__exit__(None, None, None)

    if do_compile:
        nc.compile()
    return nc


_CACHE = {}


def _get_program():
    if "nc" not in _CACHE:
        _CACHE["nc"] = build_program()
    return _CACHE["nc"]


def _prep_inputs(inputs):
    E4 = ml_dtypes.float8_e4m3
    x = np.asarray(inputs["x"], np.float32)
    g1 = np.asarray(inputs["ln1_g"], np.float32)
    b1 = np.asarray(inputs["ln1_b"], np.float32)
    g2 = np.asarray(inputs["ln2_g"], np.float32)
    b2 = np.asarray(inputs["ln2_b"], np.float32)
    qkv_w = np.asarray(inputs["qkv_w"], np.float32) * g1[None, :]
    proj_w = np.asarray(inputs["proj_w"], np.float32)
    fc1_w = np.asarray(inputs["fc1_w"], np.float32) * g2[None, :]
    fc2_w = np.asarray(inputs["fc2_w"], np.float32)

    qkv_bias = np.asarray(inputs["qkv_w"], np.float32) @ b1  # [3D]
    assert np.abs(qkv_bias[2 * D :]).max() == 0.0, "nonzero ln1_b v-bias unsupported"

    def wtile8(w, blocks):
        """w [O, D] -> [nb, P, CH*P] fp8 with [m, p, ch*128+o] = 32*w[m*128+o, ch*128+p]."""
        out = np.empty((len(blocks), P, CH * P), E4)
        for bi, m in enumerate(blocks):
            blk = w[m * P : (m + 1) * P, :] * 32.0  # [o 128, c 1024]
            out[bi] = (
                blk.reshape(P, CH, P).transpose(2, 1, 0).reshape(P, CH * P)
            ).astype(E4)
        return out

    def wtile8_hl(w, nb, batch):
        """w [O, D] (pre-scaled) -> [nb//batch, P, batch*2*chn*P] fp8 hi/lo
        tiles: slot (2c+s) holds hi (s=0) / lo residual (s=1) of chunk c."""
        chn = w.shape[1] // P
        out = np.empty((nb // batch, P, batch * 2 * chn * P), E4)
        for g in range(nb // batch):
            t = np.empty((P, batch, 2 * chn, P), E4)
            for i in range(batch):
                m = g * batch + i
                blk = w[m * P : (m + 1) * P, :]  # [o, c]
                wt = blk.reshape(P, chn, P).transpose(2, 1, 0)  # [p, c, o]
                hi = wt.astype(E4)
                lo = (wt - hi.astype(np.float32)).astype(E4)
                t[:, i, 0::2, :] = hi
                t[:, i, 1::2, :] = lo
            out[g] = t.reshape(P, -1)
        return out

    def wtile8_t3(w):
        """w [O, D] (pre-scaled) -> [O//P, P, 3*chn*P] fp8 (hi, hi, lo)."""
        chn = w.shape[1] // P
        out = np.empty((w.shape[0] // P, P, 3 * chn * P), E4)
        for m in range(w.shape[0] // P):
            blk = w[m * P : (m + 1) * P, :]
            wt = blk.reshape(P, chn, P).transpose(2, 1, 0)  # [p, c, o]
            hi = wt.astype(E4)
            lo = (wt - hi.astype(np.float32)).astype(E4)
            t = np.empty((P, chn, 3, P), E4)
            t[:, :, 0, :] = hi
            t[:, :, 1, :] = hi
            t[:, :, 2, :] = lo
            out[m] = t.reshape(P, -1)
        return out

    def wtile_bf(w, nb, batch):
        """w [O, D] -> [nb//batch, P, batch*CH*P] bf16 tiles."""
        out = np.empty((nb // batch, P, batch * (w.shape[1] // P) * P), ml_dtypes.bfloat16)
        chn = w.shape[1] // P
        for g in range(nb // batch):
            t = np.empty((P, batch, chn, P), np.float32)
            for i in range(batch):
                m = g * batch + i
                blk = w[m * P : (m + 1) * P, :]  # [o, c]
                t[:, i] = blk.reshape(P, chn, P).transpose(2, 1, 0)
            out[g] = t.reshape(P, -1).astype(ml_dtypes.bfloat16)
        return out

    # V weights token-major: [ph, p, ch*512+vc] = 32*qkv_w'[2D+ph*512+vc, ch*128+p]
    wv = np.empty((2, P, CH * TOK), E4)
    for ph in range(2):
        blk = qkv_w[2 * D + ph * TOK : 2 * D + (ph + 1) * TOK, :] * 32.0  # [vc, c]
        wv[ph] = blk.reshape(TOK, CH, P).transpose(2, 1, 0).reshape(P, CH * TOK).astype(E4)

    bqk = np.zeros((P, 16), np.float32)
    bqk[:, 0:8] = _stripe(16.0 * qkv_bias[0:D])
    bqk[:, 8:16] = _stripe(16.0 * qkv_bias[D : 2 * D])
    bpf = np.zeros((P, 16), np.float32)
    bpf[:, 0:8] = _stripe(inputs["proj_b"])
    bpf[:, 8:16] = _stripe(inputs["fc2_b"])

    shared = {
        "wqk8": wtile8(qkv_w, list(range(16))),
        "wv8": wv,
        "wp8": wtile8(proj_w, list(range(CH))),
        "w1f8": wtile8_hl(fc1_w * 32.0, HCH, 4),
        "w2f8": wtile8_t3(fc2_w * 64.0),
        "biasqk": bqk,
        "biaspf": bpf,
        "fc1b": _stripe(
            np.asarray(inputs["fc1_b"], np.float32)
            + np.asarray(inputs["fc1_w"], np.float32) @ b2
        ),
    }
    in_maps = []
    for c in range(NCORES):
        b, blk = divmod(c, RANKS)
        xblk = x[b, blk * TOK : (blk + 1) * TOK, :]  # [TOK, D]
        xt = round_fp32r(np.ascontiguousarray(xblk.T))
        m = dict(shared)
        m["xT"] = xt
        m["x8T"] = xt.astype(E4)
        in_maps.append(m)
    return in_maps


def _assemble(results):
    out = np.empty((B, N, D), dtype=np.float32)
    for c in range(NCORES):
        b, blk = divmod(c, RANKS)
        out[b, blk * TOK : (blk + 1) * TOK, :] = results[c]["outT"].T
    return out


def run_device(inputs, **kwargs):
    nc = _get_program()
    in_maps = _prep_inputs(inputs)
    res = run_bass_kernel_spmd(nc, in_maps, core_ids=list(range(NCORES)), **kwargs)
    return _assemble(res.results), res


def kernel(**inputs) -> np.ndarray:
    out, _ = run_device(inputs)
    return out


# revision 13
# speedup vs baseline: 1.0494x; 1.0064x over previous
"""Trainium2 Bass kernel v2: fp8 DoubleRow attention + bf16 MLP.

Sharding: sequence-parallel over 8 cores (512 tokens each, batch = core//4).
One 4-rank AllGather carries fp8 K (feature-major) + fp8 V (token-major).

Precision plan (validated in numpy, max_rel ~1.0e-2 vs 2e-2 gate):
  - weights qkv/proj: e4m3 x32 host-scaled; fc1/fc2: bf16 (MLP dominates error)
  - h1/q/k/v/pt/ctx: e4m3 (q,k,v at sigma~16 via 0.5 consume scale)
  - softmax exp: constant shift C=4 (cancels in normalize); split between
    Act (native Exp -> fp8) and DVE (Schraudolph: psum*a+b -> uint8 whose
    bit pattern IS e4m3 2^x; floor-vs-round ambiguity is a constant factor
    that cancels in the softmax normalize)
  - x residual fp32r; LN stats via ones-matmul (fp8 DoubleRow for LN1 on
    host-provided x8, fp32r for LN2)
DoubleRow pair slots: chunk pairs for QKV/ctx/proj; (k, zeros) for scores
(d=64 contraction cannot pair; zero slot makes the 0.5 cyc/row rate legal).
"""
import sys

sys.path.insert(0, "/opt/trn_rl_repo")
import numpy as np
import ml_dtypes
import concourse.bass as bass
import concourse.mybir as mybir
import concourse.tile as tile
from concourse import bacc
from concourse.bass_utils import run_bass_kernel_spmd

B, N, D = 2, 2048, 1024
H, DH = 16, 64
HID = 4096
NCORES = 8
TOK = (B * N) // NCORES  # 512
EPS = 1e-5
SCALE = DH**-0.5
P = 128
CH = D // P  # 8
KC = N // P  # 16
HCH = HID // P  # 32
RANKS = 4
CSH = 4.0  # exp arg shift, cancels in softmax
LN2_ = float(np.log(2.0))
# Schraudolph uint8-as-e4m3: y = psum * SA + SB
SA = 8.0 * (2.0**-11) / LN2_
SB = 56.5 - 8.0 * CSH / LN2_

F32 = mybir.dt.float32
F32R = mybir.dt.float32r
F8 = mybir.dt.float8e4
BF = mybir.dt.bfloat16
U8 = mybir.dt.uint8
AF = mybir.ActivationFunctionType
OP = mybir.AluOpType
DR = mybir.MatmulPerfMode.DoubleRow

REPLICA_GROUPS = [[0, 1, 2, 3], [4, 5, 6, 7]]

KV_K = D * TOK  # bytes of K region (fp8 feature-major [1024, 512])
DHP = DH + 16  # per-head stride in V region: 64 v + 1 ones + 15 pad
# (dual-fp8 LdWeights requires 16B-aligned weight base addresses)
KV_V = TOK * (H * DHP)  # V region [512, 1280]
KV_SZ = KV_K + KV_V


def round_fp32r(x: np.ndarray) -> np.ndarray:
    u = np.ascontiguousarray(x, dtype=np.float32).view(np.uint32)
    u = (u + 0x7FF + ((u >> 12) & 1)) & np.uint32(0xFFFFF000)
    return u.view(np.float32)


def _stripe(v: np.ndarray) -> np.ndarray:
    """[M] -> [P, M//P] with col m, part p = v[m*128+p]."""
    return np.ascontiguousarray(np.asarray(v, np.float32).reshape(-1, P).T)


def build_program(do_compile=True):
    nc = bacc.Bacc("TRN2", target_bir_lowering=False, debug=False, num_devices=NCORES)

    xT = nc.dram_tensor("xT", [D, TOK], F32, kind="ExternalInput").ap()
    x8T = nc.dram_tensor("x8T", [D, TOK], F8, kind="ExternalInput").ap()
    # weight tiles, DMA-contiguous per partition
    wqk8 = nc.dram_tensor("wqk8", [16, P, CH * P], F8, kind="ExternalInput").ap()
    wv8 = nc.dram_tensor("wv8", [2, P, CH * TOK], F8, kind="ExternalInput").ap()
    wp8 = nc.dram_tensor("wp8", [CH, P, CH * P], F8, kind="ExternalInput").ap()
    # fc1 weights as fp8 hi/lo pairs [.., (chunk, hi/lo), out]; fc2 stays bf16
    w1f8 = nc.dram_tensor(
        "w1f8", [CH, P, 4 * 2 * CH * P], F8, kind="ExternalInput"
    ).ap()
    # fc2 weights fp8 (hi, hi_dup, lo) triples per chunk (dups host-side)
    w2f8 = nc.dram_tensor(
        "w2f8", [CH, P, 3 * HCH * P], F8, kind="ExternalInput"
    ).ap()
    biasqk = nc.dram_tensor("biasqk", [P, 16], F32, kind="ExternalInput").ap()
    biaspf = nc.dram_tensor("biaspf", [P, 16], F32, kind="ExternalInput").ap()
    fc1b = nc.dram_tensor("fc1b", [P, HCH], F32, kind="ExternalInput").ap()
    outT = nc.dram_tensor("outT", [D, TOK], F32, kind="ExternalOutput").ap()

    with tile.TileContext(nc) as tc:
        with (
            tc.tile_pool(name="consts", bufs=1) as consts,
            tc.tile_pool(name="bigs", bufs=1) as bigs,
            tc.tile_pool(name="work", bufs=3) as work,
            tc.tile_pool(name="wpool", bufs=2) as wpool,
            tc.tile_pool(name="kvz", bufs=2) as kvz,
            tc.tile_pool(name="pp", bufs=2) as ppool,
            tc.tile_pool(name="rows", bufs=3) as rows,
            tc.tile_pool(name="bc", bufs=2) as bcpool,
            tc.tile_pool(name="stg", bufs=2) as stg,
            tc.tile_pool(name="dram", bufs=1, space="DRAM") as dram,
        ):
            ones8 = consts.tile([P, 1], F8)
            nc.vector.memset(ones8[:].bitcast(U8), 0x38)  # e4m3 1.0
            ones_r = consts.tile([P, 1], F32R)
            nc.vector.memset(ones_r[:].bitcast(F32), 1.0)
            eps_row = consts.tile([1, 1], F32, tag="eps")
            nc.vector.memset(eps_row[:], EPS)
            negc_row = consts.tile([P, 1], F32, tag="negc")
            nc.vector.memset(negc_row[:], -CSH)
            onesv = consts.tile([P, 4, H], F8, tag="onesv")
            nc.vector.memset(onesv[:].bitcast(U8), 0x38)
            biasqk_sb = consts.tile([P, 16], F32, tag="bqk")
            biaspf_sb = consts.tile([P, 16], F32, tag="bpf")
            fc1b_sb = consts.tile([P, HCH], F32, tag="b1")
            nc.sync.dma_start(biasqk_sb[:], biasqk[:])
            nc.sync.dma_start(biaspf_sb[:], biaspf[:])
            nc.sync.dma_start(fc1b_sb[:], fc1b[:])

            kv_in = dram.tile([KV_SZ], F8, tag="kvin")
            kv_out = dram.tile([RANKS * KV_SZ], F8, tag="kvout")
            vk_in = kv_in[0:KV_K].rearrange("(f t) -> f t", t=TOK)
            vv_in = kv_in[KV_K:KV_SZ].rearrange(
                "(t v) -> t v", v=H * DHP
            )
            kv_or = kv_out[:].rearrange("(r x) -> r x", r=RANKS)
            # ones column of the V region: written up-front (no data deps)
            for tc_ in range(RANKS):
                ones_dst = vv_in[tc_ * P : (tc_ + 1) * P, :].rearrange(
                    "p (h c) -> p h c", c=DHP
                )[:, :, DH : DH + 1]
                nc.sync.dma_start(
                    ones_dst, onesv[:, tc_, :].rearrange("p (h c) -> p h c", c=1)
                )

            # ---- LN1 ---- (x8 first: stats depend on it; xr arrives later)
            x8_sb = bigs.tile([P, CH, TOK], F8, tag="x8")
            nc.sync.dma_start(
                x8_sb[:], x8T.rearrange("(ch p) t -> p ch t", p=P)
            )
            xr_sb = bigs.tile([P, CH, TOK], F32, tag="xr")
            nc.sync.dma_start(
                xr_sb[:], xT.rearrange("(ch p) t -> p ch t", p=P)
            )
            sq8 = bigs.tile([P, CH, TOK], F8, tag="h1")  # released before h1

            def ln_rows(psum_mu, psum_s2, name):
                mu = rows.tile([1, TOK], F32, tag="r", name=f"mu{name}")
                nc.vector.tensor_scalar_mul(mu[:], psum_mu[:], 1.0 / D)
                var = rows.tile([1, TOK], F32, tag="r", name=f"va{name}")
                nc.vector.tensor_tensor(var[:], mu[:], mu[:], OP.mult)
                ex2 = rows.tile([1, TOK], F32, tag="r", name=f"e2{name}")
                nc.vector.tensor_scalar_mul(ex2[:], psum_s2[:], 1.0 / D)
                nc.vector.tensor_sub(var[:], ex2[:], var[:])
                rstd = rows.tile([1, TOK], F32, tag="r", name=f"rs{name}")
                nc.scalar.activation(
                    out=rstd[:], in_=var[:], func=AF.Sqrt, bias=eps_row[:]
                )
                nc.vector.reciprocal(rstd[:], rstd[:])
                cpos = rows.tile([1, TOK], F32, tag="r", name=f"cp{name}")
                nc.vector.tensor_tensor(cpos[:], mu[:], rstd[:], OP.mult)
                rstd_b = bcpool.tile([P, TOK], F32, tag="bc", name=f"rb{name}")
                nc.gpsimd.partition_broadcast(rstd_b[:], rstd[:])
                c_b = bcpool.tile([P, TOK], F32, tag="bc", name=f"cb{name}")
                nc.gpsimd.partition_broadcast(c_b[:], cpos[:])
                return rstd_b, c_b

            with tc.tile_pool(name="ps_row1", bufs=2, space="PSUM") as prow:
                psum_mu = prow.tile([1, TOK], F32, tag="row")
                psum_s2 = prow.tile([1, TOK], F32, tag="row")
                for ch in range(CH):
                    eng = nc.vector if ch % 2 == 0 else nc.gpsimd
                    eng.tensor_tensor(
                        sq8[:, ch, :], x8_sb[:, ch, :], x8_sb[:, ch, :], OP.mult
                    )
                for ch in range(CH):
                    nc.tensor.matmul(
                        psum_mu[:],
                        ones8[:],
                        x8_sb[:, ch, :],
                        start=(ch == 0),
                        stop=(ch == CH - 1),
                    )
                for ch in range(CH):
                    nc.tensor.matmul(
                        psum_s2[:],
                        ones8[:],
                        sq8[:, ch, :],
                        start=(ch == 0),
                        stop=(ch == CH - 1),
                    )
                rstd1_b, c1_b = ln_rows(psum_mu, psum_s2, "1")

            h1 = bigs.tile([P, CH, TOK], F8, tag="h1")
            for ch in range(CH):
                eng = nc.vector if ch % 2 == 0 else nc.gpsimd
                t1 = work.tile([P, TOK], F32, tag="t1")
                eng.tensor_tensor(t1[:], xr_sb[:, ch, :], rstd1_b[:], OP.mult)
                eng.tensor_tensor(h1[:, ch, :], t1[:], c1_b[:], OP.subtract)

            # ---- QKV ----
            st2 = tc.tile_pool(name="ps_mm2", bufs=5, space="PSUM")
            ps_acc = st2.__enter__()

            def qkv_block(wt, i, m, consume):
                psum = ps_acc.tile([P, TOK], F32, tag="acc", name=f"ps_{m}")
                for j in range(CH // 2):
                    nc.tensor.matmul(
                        psum[:],
                        wt[:, i, 2 * j : 2 * j + 2, :],
                        h1[:, 2 * j : 2 * j + 2, :],
                        start=(j == 0),
                        stop=(j == CH // 2 - 1),
                        perf_mode=DR,
                    )
                consume(m, psum)

            def k_consume(m, psum):
                k8 = stg.tile([P, TOK], F8, tag="cp", name=f"k8_{m}")
                nc.scalar.activation(
                    out=k8[:],
                    in_=psum[:],
                    func=AF.Identity,
                    bias=biasqk_sb[:, 8 + m : 9 + m],
                    scale=0.5,
                )
                nc.scalar.dma_start(vk_in[m * P : (m + 1) * P, :], k8[:])

            # K and V interleaved: K consumes on Act, V on DVE run in
            # parallel so the AllGather input completes sooner
            for half in range(2):
                wk = wpool.tile([P, 4, CH, P], F8, tag="wq", name=f"wk{half}")
                nc.sync.dma_start(
                    wk[:],
                    wqk8[8 + 4 * half : 12 + 4 * half].rearrange(
                        "i p (ch o) -> p i ch o", ch=CH
                    ),
                )
                wv_t = wpool.tile([P, CH, TOK], F8, tag="wq", name=f"wv{half}")
                nc.sync.dma_start(
                    wv_t[:], wv8[half].rearrange("p (ch v) -> p ch v", ch=CH)
                )
                for i in range(4):
                    qkv_block(wk, i, 4 * half + i, k_consume)
                    psum = ps_acc.tile(
                        [P, TOK], F32, tag="acc", name=f"pv{half}_{i}"
                    )
                    for j in range(CH // 2):
                        nc.tensor.matmul(
                            psum[:],
                            h1[:, 2 * j : 2 * j + 2, i * P : (i + 1) * P],
                            wv_t[:, 2 * j : 2 * j + 2, :],
                            start=(j == 0),
                            stop=(j == CH // 2 - 1),
                            perf_mode=DR,
                        )
                    v8 = stg.tile([P, TOK], F8, tag="cp", name=f"v8_{half}_{i}")
                    nc.vector.tensor_scalar_mul(v8[:], psum[:], 0.5)
                    dst = vv_in[
                        i * P : (i + 1) * P,
                        half * 8 * DHP : (half + 1) * 8 * DHP,
                    ].rearrange("t (h c) -> t h c", c=DHP)[:, :, 0:DH]
                    nc.gpsimd.dma_start(
                        dst, v8[:].rearrange("t (h d) -> t h d", d=DH)
                    )

            nc.gpsimd.collective_compute(
                "AllGather",
                OP.bypass,
                ins=[kv_in[:].opt()],
                outs=[kv_out[:].opt()],
                replica_groups=REPLICA_GROUPS,
            )

            # Q blocks -> SBUF (chunk 8 duplicates chunk 7 for the hp=7 rhs pair)
            qT = bigs.tile([P, CH + 1, TOK], F8, tag="qT")

            def q_consume(m, psum):
                nc.vector.tensor_scalar(
                    out=qT[:, m, :],
                    in0=psum[:],
                    scalar1=0.5,
                    scalar2=biasqk_sb[:, m : m + 1],
                    op0=OP.mult,
                    op1=OP.add,
                )

            for g in range(2):
                wt = wpool.tile([P, 4, CH, P], F8, tag="wq", name=f"wq{g}")
                nc.sync.dma_start(
                    wt[:],
                    wqk8[4 * g : 4 * g + 4].rearrange("i p (ch o) -> p i ch o", ch=CH),
                )
                for i in range(4):
                    qkv_block(wt, i, 4 * g + i, q_consume)
            nc.sync.dma_start(qT[:, CH, :], qT[:, CH - 1, :])
            st2.__exit__(None, None, None)

            # ---- attention ----
            # kp: [P(2 heads' d), 2, KC, P] fp8; slot0 = K data, slot1 = zeros
            kp_t = []
            for i in range(2):
                t = kvz.tile([P, 2, KC, P], F8, tag="kp", name=f"kp{i}")
                nc.vector.memset(t[:, 1, :, :].bitcast(U8), 0)
                kp_t.append(t)
            # vfull: [P(key-in-chunk), KC, 16*(64+1)] fp8, ones pre-gathered
            vfull = bigs.tile([P, KC, H * DHP], F8, tag="vfull")
            for r in range(RANKS):
                src = (
                    kv_or[r : r + 1, KV_K:KV_SZ]
                    .rearrange("o (t v) -> o t v", v=H * DHP)[0]
                    .rearrange("(tc p) v -> p tc v", p=P)
                )
                nc.sync.dma_start(vfull[:, 4 * r : 4 * r + 4, :], src)

            def load_kp(hp, t):
                src = (
                    kv_or[:, 0:KV_K]
                    .rearrange("r (f t) -> r f t", t=TOK)[
                        :, hp * P : (hp + 1) * P, :
                    ]
                    .rearrange("r p t -> p r t")
                )
                nc.sync.dma_start(t[:, 0, :, :], src)

            ctxT = bigs.tile([P, CH, TOK], F8, tag="x8")  # reuse x8 region
            groups = [(2 * j, 2) for j in range(8)]
            pairs_after = {j: [j] for j in range(8)}

            att_pools = (
                tc.tile_pool(name="ps_s", bufs=2, space="PSUM"),
                tc.tile_pool(name="ps_ctx", bufs=2, space="PSUM"),
            )
            ps_spool = att_pools[0].__enter__()
            ps_ctx = att_pools[1].__enter__()
            eidx = 0
            for hp in range(H // 2):
                kp = kp_t[hp % 2]
                load_kp(hp, kp)
                for hh in range(2):
                    h = 2 * hp + hh
                    half = slice(hh * DH, hh * DH + DH)
                    qpair = qT[half, hp : hp + 2, :]
                    pt = ppool.tile([P, KC, TOK], F8, tag="pt", name=f"pt{h}")
                    psum_c = ps_ctx.tile([DH + 1, TOK], F32, tag="ctx")
                    for gi, (kc0, nb) in enumerate(groups):
                        ps_s = ps_spool.tile([P, 2 * TOK], F32, tag="s", bufs=3)
                        for j in range(nb):
                            nc.tensor.matmul(
                                ps_s[:, j * TOK : (j + 1) * TOK],
                                kp[half, :, kc0 + j, :],
                                qpair,
                                start=True,
                                stop=True,
                                perf_mode=DR,
                            )
                        dst = pt[:, kc0 : kc0 + nb, :]
                        # alternate Act/DVE so consecutive groups of a head
                        # never serialize on one engine (GPSIMD can't read
                        # PSUM per the BIR verifier); 4:2 toward Act since
                        # DVE also carries reciprocal + normalize
                        if (gi + h) % 2 == 0:
                            nc.scalar.activation(
                                out=dst,
                                in_=ps_s[:, : nb * TOK],
                                func=AF.Exp,
                                bias=negc_row[:],
                                scale=2.0**-11,
                            )
                        else:
                            nc.vector.tensor_scalar(
                                out=dst.bitcast(U8),
                                in0=ps_s[:, : nb * TOK],
                                scalar1=SA,
                                scalar2=SB,
                                op0=OP.mult,
                                op1=OP.add,
                            )
                        eidx += 1
                        for pj in pairs_after.get(gi, []):
                            nc.tensor.matmul(
                                psum_c[:],
                                vfull[
                                    :,
                                    2 * pj : 2 * pj + 2,
                                    h * DHP : h * DHP + DH + 1,
                                ],
                                pt[:, 2 * pj : 2 * pj + 2, :],
                                start=(pj == 0),
                                stop=(pj == KC // 2 - 1),
                                perf_mode=DR,
                            )
                    rrow = rows.tile([1, TOK], F32, tag="r", name=f"rr{h}")
                    nc.vector.reciprocal(rrow[:], psum_c[DH : DH + 1, :])
                    rb = bcpool.tile([DH, TOK], F32, tag="rb", name=f"rb{h}")
                    nc.gpsimd.partition_broadcast(rb[:], rrow[:])
                    nc.vector.tensor_tensor(
                        ctxT[half, hp, :], psum_c[0:DH, :], rb[:], OP.mult
                    )
            att_pools[1].__exit__(None, None, None)
            att_pools[0].__exit__(None, None, None)

            # ---- proj + residual ----
            st4 = tc.tile_pool(name="ps_mm4", bufs=5, space="PSUM")
            ps_mlp = st4.__enter__()
            x2 = bigs.tile([P, CH, TOK], F32R, tag="x2")
            for g in range(2):
                wt = wpool.tile([P, 4, CH, P], F8, tag="wq", name=f"wpj{g}")
                nc.sync.dma_start(
                    wt[:],
                    wp8[4 * g : 4 * g + 4].rearrange("i p (ch o) -> p i ch o", ch=CH),
                )
                for i in range(4):
                    m = 4 * g + i
                    psum = ps_mlp.tile([P, TOK], F32, tag="acc", name=f"pp_{m}")
                    for j in range(CH // 2):
                        nc.tensor.matmul(
                            psum[:],
                            wt[:, i, 2 * j : 2 * j + 2, :],
                            ctxT[:, 2 * j : 2 * j + 2, :],
                            start=(j == 0),
                            stop=(j == CH // 2 - 1),
                            perf_mode=DR,
                        )
                    attn_sb = stg.tile([P, TOK], F32, tag="stg", name=f"at_{m}")
                    nc.scalar.activation(
                        out=attn_sb[:],
                        in_=psum[:],
                        func=AF.Identity,
                        bias=biaspf_sb[:, m : m + 1],
                        scale=2.0**-9,
                    )
                    nc.gpsimd.tensor_tensor(
                        x2[:, m, :], attn_sb[:], xr_sb[:, m, :], OP.add
                    )  # f32r out: rounded on write for the LN2 stats matmul

            # ---- LN2 (fp32r stats on x2) ----
            with tc.tile_pool(name="ps_row2", bufs=2, space="PSUM") as prow:
                psum_mu2 = prow.tile([1, TOK], F32, tag="row")
                psum_s22 = prow.tile([1, TOK], F32, tag="row")
                for ch in range(CH):
                    nc.tensor.matmul(
                        psum_mu2[:],
                        ones_r[:],
                        x2[:, ch, :],
                        start=(ch == 0),
                        stop=(ch == CH - 1),
                    )
                    sq = work.tile([P, TOK], F32R, tag="t1", name=f"sq2_{ch}")
                    nc.gpsimd.tensor_tensor(
                        sq[:],
                        x2[:, ch, :].bitcast(F32),
                        x2[:, ch, :].bitcast(F32),
                        OP.mult,
                    )
                    nc.tensor.matmul(
                        psum_s22[:],
                        ones_r[:],
                        sq[:],
                        start=(ch == 0),
                        stop=(ch == CH - 1),
                    )
                rstd2_b, c2_b = ln_rows(psum_mu2, psum_s22, "2")

            # xn as fp8 hi/lo + duplicated-hi slot: [hi, hi_dup, lo]
            xnf = bigs.tile([P, CH, 3, TOK], F8, tag="xn")
            for ch in range(CH):
                t1 = work.tile([P, TOK], F32, tag="t1", name=f"t2_{ch}")
                nc.gpsimd.tensor_tensor(
                    t1[:], x2[:, ch, :].bitcast(F32), rstd2_b[:], OP.mult
                )
                xn32 = work.tile([P, TOK], F32, tag="xn32", bufs=2, name=f"x32_{ch}")
                nc.vector.tensor_tensor(xn32[:], t1[:], c2_b[:], OP.subtract)
                nc.scalar.activation(
                    out=xnf[:, ch, 0, :], in_=xn32[:], func=AF.Identity
                )
                nc.gpsimd.tensor_copy(
                    out=xnf[:, ch, 1, :], in_=xnf[:, ch, 0, :]
                )
                nc.vector.tensor_tensor(
                    xnf[:, ch, 2, :], xn32[:], xnf[:, ch, 0, :], OP.subtract
                )

            # ---- MLP: fc1 fp8 hi/lo "3-product" DoubleRow, fc2 bf16 ----
            # per chunk pair (c, d): [whi_c,wlo_c]x[xhi,xhidup], same for d,
            # then [whi_c,whi_d]x[xlo_c,xlo_d] (drops the negligible lo*lo)
            g2 = bigs.tile([P, HCH, 2, TOK], F8, tag="g")
            for g in range(CH):
                w1 = wpool.tile([P, 4, 2 * CH, P], F8, tag="w1", name=f"w1_{g}")
                nc.sync.dma_start(
                    w1[:],
                    w1f8[g].rearrange("p (i c o) -> p i c o", i=4, c=2 * CH),
                )
                for i in range(4):
                    m = 4 * g + i
                    psum = ps_mlp.tile([P, TOK], F32, tag="acc", name=f"p1_{m}")
                    for c2 in range(CH // 2):
                        c = 2 * c2
                        nc.tensor.matmul(
                            psum[:],
                            w1[:, i, 2 * c : 2 * c + 2, :],
                            xnf[:, c, 0:2, :],
                            start=(c2 == 0),
                            stop=False,
                            perf_mode=DR,
                        )
                        nc.tensor.matmul(
                            psum[:],
                            w1[:, i, 2 * c + 2 : 2 * c + 4, :],
                            xnf[:, c + 1, 0:2, :],
                            start=False,
                            stop=False,
                            perf_mode=DR,
                        )
                        whi = w1[:, i].rearrange(
                            "p (c two) k -> p c two k", two=2
                        )[:, c : c + 2, 0, :]
                        nc.tensor.matmul(
                            psum[:],
                            whi,
                            xnf[:, c : c + 2, 2, :],
                            start=False,
                            stop=(c2 == CH // 2 - 1),
                            perf_mode=DR,
                        )
                    g32 = work.tile([P, TOK], F32, tag="xn32", bufs=2, name=f"g32_{m}")
                    nc.scalar.activation(
                        out=g32[:],
                        in_=psum[:],
                        func=AF.Gelu,
                        bias=fc1b_sb[:, m : m + 1],
                        scale=2.0**-5,
                    )
                    nc.gpsimd.tensor_copy(out=g2[:, m, 0, :], in_=g32[:])
                    nc.vector.tensor_tensor(
                        g2[:, m, 1, :], g32[:], g2[:, m, 0, :], OP.subtract
                    )
            for m2 in range(CH):
                psum = ps_mlp.tile([P, TOK], F32, tag="acc", name=f"p2_{m2}")
                for hf in range(2):
                    w2 = wpool.tile(
                        [P, 3 * HCH // 2, P], F8, tag="w2", name=f"w2_{m2}_{hf}"
                    )
                    nc.sync.dma_start(
                        w2[:],
                        w2f8[m2][
                            :, hf * 3 * (HCH // 2) * P : (hf + 1) * 3 * (HCH // 2) * P
                        ].rearrange("p (c o) -> p c o", c=3 * HCH // 2),
                    )
                    w2lo = w2.rearrange("p (c three) k -> p c three k", three=3)
                    for c2 in range(HCH // 4):
                        c = 2 * c2
                        hc = hf * (HCH // 2) + c
                        nc.tensor.matmul(
                            psum[:],
                            w2[:, 3 * c : 3 * c + 2, :],
                            g2[:, hc, 0:2, :],
                            start=(hf == 0 and c2 == 0),
                            stop=False,
                            perf_mode=DR,
                        )
                        nc.tensor.matmul(
                            psum[:],
                            w2[:, 3 * c + 3 : 3 * c + 5, :],
                            g2[:, hc + 1, 0:2, :],
                            start=False,
                            stop=False,
                            perf_mode=DR,
                        )
                        nc.tensor.matmul(
                            psum[:],
                            w2lo[:, c : c + 2, 2, :],
                            g2[:, hc : hc + 2, 0, :],
                            start=False,
                            stop=(hf == 1 and c2 == HCH // 4 - 1),
                            perf_mode=DR,
                        )
                o_sb = stg.tile([P, TOK], F32, tag="stg", name=f"o_{m2}")
                nc.scalar.activation(
                    out=o_sb[:],
                    in_=psum[:],
                    func=AF.Identity,
                    bias=biaspf_sb[:, 8 + m2 : 9 + m2],
                    scale=2.0**-6,
                )
                o_f = stg.tile([P, TOK], F32, tag="of", bufs=2, name=f"of_{m2}")
                nc.vector.tensor_add(
                    out=o_f[:], in0=o_sb[:], in1=x2[:, m2, :].bitcast(F32)
                )
                nc.sync.dma_start(outT[m2 * P : (m2 + 1) * P, :], o_f[:])
            st4.__exit__(None, None, None)

    if do_compile:
        nc.compile()
    return nc


_CACHE = {}


def _get_program():
    if "nc" not in _CACHE:
        _CACHE["nc"] = build_program()
    return _CACHE["nc"]


def _prep_inputs(inputs):
    E4 = ml_dtypes.float8_e4m3
    x = np.asarray(inputs["x"], np.float32)
    g1 = np.asarray(inputs["ln1_g"], np.float32)
    b1 = np.asarray(inputs["ln1_b"], np.float32)
    g2 = np.asarray(inputs["ln2_g"], np.float32)
    b2 = np.asarray(inputs["ln2_b"], np.float32)
    qkv_w = np.asarray(inputs["qkv_w"], np.float32) * g1[None, :]
    proj_w = np.asarray(inputs["proj_w"], np.float32)
    fc1_w = np.asarray(inputs["fc1_w"], np.float32) * g2[None, :]
    fc2_w = np.asarray(inputs["fc2_w"], np.float32)

    qkv_bias = np.asarray(inputs["qkv_w"], np.float32) @ b1  # [3D]
    assert np.abs(qkv_bias[2 * D :]).max() == 0.0, "nonzero ln1_b v-bias unsupported"

    def wtile8(w, blocks):
        """w [O, D] -> [nb, P, CH*P] fp8 with [m, p, ch*128+o] = 32*w[m*128+o, ch*128+p]."""
        out = np.empty((len(blocks), P, CH * P), E4)
        for bi, m in enumerate(blocks):
            blk = w[m * P : (m + 1) * P, :] * 32.0  # [o 128, c 1024]
            out[bi] = (
                blk.reshape(P, CH, P).transpose(2, 1, 0).reshape(P, CH * P)
            ).astype(E4)
        return out

    def wtile8_hl(w, nb, batch):
        """w [O, D] (pre-scaled) -> [nb//batch, P, batch*2*chn*P] fp8 hi/lo
        tiles: slot (2c+s) holds hi (s=0) / lo residual (s=1) of chunk c."""
        chn = w.shape[1] // P
        out = np.empty((nb // batch, P, batch * 2 * chn * P), E4)
        for g in range(nb // batch):
            t = np.empty((P, batch, 2 * chn, P), E4)
            for i in range(batch):
                m = g * batch + i
                blk = w[m * P : (m + 1) * P, :]  # [o, c]
                wt = blk.reshape(P, chn, P).transpose(2, 1, 0)  # [p, c, o]
                hi = wt.astype(E4)
                lo = (wt - hi.astype(np.float32)).astype(E4)
                t[:, i, 0::2, :] = hi
                t[:, i, 1::2, :] = lo
            out[g] = t.reshape(P, -1)
        return out

    def wtile8_t3(w):
        """w [O, D] (pre-scaled) -> [O//P, P, 3*chn*P] fp8 (hi, hi, lo)."""
        chn = w.shape[1] // P
        out = np.empty((w.shape[0] // P, P, 3 * chn * P), E4)
        for m in range(w.shape[0] // P):
            blk = w[m * P : (m + 1) * P, :]
            wt = blk.reshape(P, chn, P).transpose(2, 1, 0)  # [p, c, o]
            hi = wt.astype(E4)
            lo = (wt - hi.astype(np.float32)).astype(E4)
            t = np.empty((P, chn, 3, P), E4)
            t[:, :, 0, :] = hi
            t[:, :, 1, :] = hi
            t[:, :, 2, :] = lo
            out[m] = t.reshape(P, -1)
        return out

    def wtile_bf(w, nb, batch):
        """w [O, D] -> [nb//batch, P, batch*CH*P] bf16 tiles."""
        out = np.empty((nb // batch, P, batch * (w.shape[1] // P) * P), ml_dtypes.bfloat16)
        chn = w.shape[1] // P
        for g in range(nb // batch):
            t = np.empty((P, batch, chn, P), np.float32)
            for i in range(batch):
                m = g * batch + i
                blk = w[m * P : (m + 1) * P, :]  # [o, c]
                t[:, i] = blk.reshape(P, chn, P).transpose(2, 1, 0)
            out[g] = t.reshape(P, -1).astype(ml_dtypes.bfloat16)
        return out

    # V weights token-major: [ph, p, ch*512+vc] = 32*qkv_w'[2D+ph*512+vc, ch*128+p]
    wv = np.empty((2, P, CH * TOK), E4)
    for ph in range(2):
        blk = qkv_w[2 * D + ph * TOK : 2 * D + (ph + 1) * TOK, :] * 32.0  # [vc, c]
        wv[ph] = blk.reshape(TOK, CH, P).transpose(2, 1, 0).reshape(P, CH * TOK).astype(E4)

    bqk = np.zeros((P, 16), np.float32)
    bqk[:, 0:8] = _stripe(16.0 * qkv_bias[0:D])
    bqk[:, 8:16] = _stripe(16.0 * qkv_bias[D : 2 * D])
    bpf = np.zeros((P, 16), np.float32)
    bpf[:, 0:8] = _stripe(inputs["proj_b"])
    bpf[:, 8:16] = _stripe(inputs["fc2_b"])

    shared = {
        "wqk8": wtile8(qkv_w, list(range(16))),
        "wv8": wv,
        "wp8": wtile8(proj_w, list(range(CH))),
        "w1f8": wtile8_hl(fc1_w * 32.0, HCH, 4),
        "w2f8": wtile8_t3(fc2_w * 64.0),
        "biasqk": bqk,
        "biaspf": bpf,
        "fc1b": _stripe(
            np.asarray(inputs["fc1_b"], np.float32)
            + np.asarray(inputs["fc1_w"], np.float32) @ b2
        ),
    }
    in_maps = []
    for c in range(NCORES):
        b, blk = divmod(c, RANKS)
        xblk = x[b, blk * TOK : (blk + 1) * TOK, :]  # [TOK, D]
        xt = round_fp32r(np.ascontiguousarray(xblk.T))
        m = dict(shared)
        m["xT"] = xt
        m["x8T"] = xt.astype(E4)
        in_maps.append(m)
    return in_maps


def _assemble(results):
    out = np.empty((B, N, D), dtype=np.float32)
    for c in range(NCORES):
        b, blk = divmod(c, RANKS)
        out[b, blk * TOK : (blk + 1) * TOK, :] = results[c]["outT"].T
    return out


def run_device(inputs, **kwargs):
    nc = _get_program()
    in_maps = _prep_inputs(inputs)
    res = run_bass_kernel_spmd(nc, in_maps, core_ids=list(range(NCORES)), **kwargs)
    return _assemble(res.results), res


def kernel(**inputs) -> np.ndarray:
    out, _ = run_device(inputs)
    return out


# revision 14
# speedup vs baseline: 1.0589x; 1.0091x over previous
"""Trainium2 Bass kernel v2: fp8 DoubleRow attention + bf16 MLP.

Sharding: sequence-parallel over 8 cores (512 tokens each, batch = core//4).
One 4-rank AllGather carries fp8 K (feature-major) + fp8 V (token-major).

Precision plan (validated in numpy, max_rel ~1.0e-2 vs 2e-2 gate):
  - weights qkv/proj: e4m3 x32 host-scaled; fc1/fc2: bf16 (MLP dominates error)
  - h1/q/k/v/pt/ctx: e4m3 (q,k,v at sigma~16 via 0.5 consume scale)
  - softmax exp: constant shift C=4 (cancels in normalize); split between
    Act (native Exp -> fp8) and DVE (Schraudolph: psum*a+b -> uint8 whose
    bit pattern IS e4m3 2^x; floor-vs-round ambiguity is a constant factor
    that cancels in the softmax normalize)
  - x residual fp32r; LN stats via ones-matmul (fp8 DoubleRow for LN1 on
    host-provided x8, fp32r for LN2)
DoubleRow pair slots: chunk pairs for QKV/ctx/proj; (k, zeros) for scores
(d=64 contraction cannot pair; zero slot makes the 0.5 cyc/row rate legal).
"""
import sys

sys.path.insert(0, "/opt/trn_rl_repo")
import numpy as np
import ml_dtypes
import concourse.bass as bass
import concourse.mybir as mybir
import concourse.tile as tile
from concourse import bacc
from concourse.bass_utils import run_bass_kernel_spmd

B, N, D = 2, 2048, 1024
H, DH = 16, 64
HID = 4096
NCORES = 8
TOK = (B * N) // NCORES  # 512
EPS = 1e-5
SCALE = DH**-0.5
P = 128
CH = D // P  # 8
KC = N // P  # 16
HCH = HID // P  # 32
RANKS = 4
CSH = 4.0  # exp arg shift, cancels in softmax
LN2_ = float(np.log(2.0))
# Schraudolph uint8-as-e4m3: y = psum * SA + SB
SA = 8.0 * (2.0**-11) / LN2_
SB = 56.5 - 8.0 * CSH / LN2_

F32 = mybir.dt.float32
F32R = mybir.dt.float32r
F8 = mybir.dt.float8e4
BF = mybir.dt.bfloat16
U8 = mybir.dt.uint8
AF = mybir.ActivationFunctionType
OP = mybir.AluOpType
DR = mybir.MatmulPerfMode.DoubleRow

REPLICA_GROUPS = [[0, 1, 2, 3], [4, 5, 6, 7]]

KV_K = D * TOK  # bytes of K region (fp8 feature-major [1024, 512])
DHP = DH + 16  # per-head stride in V region: 64 v + 1 ones + 15 pad
# (dual-fp8 LdWeights requires 16B-aligned weight base addresses)
KV_V = TOK * (H * DHP)  # V region [512, 1280]
KV_SZ = KV_K + KV_V


def round_fp32r(x: np.ndarray) -> np.ndarray:
    u = np.ascontiguousarray(x, dtype=np.float32).view(np.uint32)
    u = (u + 0x7FF + ((u >> 12) & 1)) & np.uint32(0xFFFFF000)
    return u.view(np.float32)


def _stripe(v: np.ndarray) -> np.ndarray:
    """[M] -> [P, M//P] with col m, part p = v[m*128+p]."""
    return np.ascontiguousarray(np.asarray(v, np.float32).reshape(-1, P).T)


def build_program(do_compile=True):
    nc = bacc.Bacc("TRN2", target_bir_lowering=False, debug=False, num_devices=NCORES)

    xT = nc.dram_tensor("xT", [D, TOK], F32, kind="ExternalInput").ap()
    x8T = nc.dram_tensor("x8T", [D, TOK], F8, kind="ExternalInput").ap()
    # weight tiles, DMA-contiguous per partition
    wqk8 = nc.dram_tensor("wqk8", [16, P, CH * P], F8, kind="ExternalInput").ap()
    wv8 = nc.dram_tensor("wv8", [2, P, CH * TOK], F8, kind="ExternalInput").ap()
    wp8 = nc.dram_tensor("wp8", [CH, P, CH * P], F8, kind="ExternalInput").ap()
    # fc1 weights as fp8 hi/lo pairs [.., (chunk, hi/lo), out]; fc2 stays bf16
    w1f8 = nc.dram_tensor(
        "w1f8", [CH, P, 4 * 2 * CH * P], F8, kind="ExternalInput"
    ).ap()
    # fc2 weights fp8 (hi, hi_dup, lo) triples per chunk (dups host-side)
    w2f8 = nc.dram_tensor(
        "w2f8", [CH, P, 3 * HCH * P], F8, kind="ExternalInput"
    ).ap()
    biasqk = nc.dram_tensor("biasqk", [P, 16], F32, kind="ExternalInput").ap()
    biaspf = nc.dram_tensor("biaspf", [P, 16], F32, kind="ExternalInput").ap()
    fc1b = nc.dram_tensor("fc1b", [P, HCH], F32, kind="ExternalInput").ap()
    outT = nc.dram_tensor("outT", [D, TOK], F32, kind="ExternalOutput").ap()

    with tile.TileContext(nc) as tc:
        with (
            tc.tile_pool(name="consts", bufs=1) as consts,
            tc.tile_pool(name="bigs", bufs=1) as bigs,
            tc.tile_pool(name="work", bufs=3) as work,
            tc.tile_pool(name="wpool", bufs=2) as wpool,
            tc.tile_pool(name="kvz", bufs=2) as kvz,
            tc.tile_pool(name="pp", bufs=2) as ppool,
            tc.tile_pool(name="rows", bufs=3) as rows,
            tc.tile_pool(name="bc", bufs=2) as bcpool,
            tc.tile_pool(name="stg", bufs=2) as stg,
            tc.tile_pool(name="dram", bufs=1, space="DRAM") as dram,
        ):
            ones8 = consts.tile([P, 1], F8)
            nc.vector.memset(ones8[:].bitcast(U8), 0x38)  # e4m3 1.0
            ones_r = consts.tile([P, 1], F32R)
            nc.vector.memset(ones_r[:].bitcast(F32), 1.0)
            eps_row = consts.tile([1, 1], F32, tag="eps")
            nc.vector.memset(eps_row[:], EPS)
            negc_row = consts.tile([P, 1], F32, tag="negc")
            nc.vector.memset(negc_row[:], -CSH)
            onesv = consts.tile([P, 4, H], F8, tag="onesv")
            nc.vector.memset(onesv[:].bitcast(U8), 0x38)
            biasqk_sb = consts.tile([P, 16], F32, tag="bqk")
            biaspf_sb = consts.tile([P, 16], F32, tag="bpf")
            fc1b_sb = consts.tile([P, HCH], F32, tag="b1")
            nc.sync.dma_start(biasqk_sb[:], biasqk[:])
            nc.sync.dma_start(biaspf_sb[:], biaspf[:])
            nc.sync.dma_start(fc1b_sb[:], fc1b[:])

            kv_in = dram.tile([KV_SZ], F8, tag="kvin")
            kv_out = dram.tile([RANKS * KV_SZ], F8, tag="kvout")
            vk_in = kv_in[0:KV_K].rearrange("(f t) -> f t", t=TOK)
            vv_in = kv_in[KV_K:KV_SZ].rearrange(
                "(t v) -> t v", v=H * DHP
            )
            kv_or = kv_out[:].rearrange("(r x) -> r x", r=RANKS)
            # ones column of the V region: written up-front (no data deps)
            for tc_ in range(RANKS):
                ones_dst = vv_in[tc_ * P : (tc_ + 1) * P, :].rearrange(
                    "p (h c) -> p h c", c=DHP
                )[:, :, DH : DH + 1]
                nc.sync.dma_start(
                    ones_dst, onesv[:, tc_, :].rearrange("p (h c) -> p h c", c=1)
                )

            # ---- LN1 ---- (x8 first: stats depend on it; xr arrives later)
            x8_sb = bigs.tile([P, CH, TOK], F8, tag="x8")
            nc.sync.dma_start(
                x8_sb[:], x8T.rearrange("(ch p) t -> p ch t", p=P)
            )
            xr_sb = bigs.tile([P, CH, TOK], F32, tag="xr")
            nc.sync.dma_start(
                xr_sb[:], xT.rearrange("(ch p) t -> p ch t", p=P)
            )
            sq8 = bigs.tile([P, CH, TOK], F8, tag="h1")  # released before h1

            def ln_rows(psum_mu, psum_s2, name):
                mu = rows.tile([1, TOK], F32, tag="r", name=f"mu{name}")
                nc.vector.tensor_scalar_mul(mu[:], psum_mu[:], 1.0 / D)
                var = rows.tile([1, TOK], F32, tag="r", name=f"va{name}")
                nc.vector.tensor_tensor(var[:], mu[:], mu[:], OP.mult)
                ex2 = rows.tile([1, TOK], F32, tag="r", name=f"e2{name}")
                nc.vector.tensor_scalar_mul(ex2[:], psum_s2[:], 1.0 / D)
                nc.vector.tensor_sub(var[:], ex2[:], var[:])
                rstd = rows.tile([1, TOK], F32, tag="r", name=f"rs{name}")
                nc.scalar.activation(
                    out=rstd[:], in_=var[:], func=AF.Sqrt, bias=eps_row[:]
                )
                nc.vector.reciprocal(rstd[:], rstd[:])
                cpos = rows.tile([1, TOK], F32, tag="r", name=f"cp{name}")
                nc.vector.tensor_tensor(cpos[:], mu[:], rstd[:], OP.mult)
                rstd_b = bcpool.tile([P, TOK], F32, tag="bc", name=f"rb{name}")
                nc.gpsimd.partition_broadcast(rstd_b[:], rstd[:])
                c_b = bcpool.tile([P, TOK], F32, tag="bc", name=f"cb{name}")
                nc.gpsimd.partition_broadcast(c_b[:], cpos[:])
                return rstd_b, c_b

            with tc.tile_pool(name="ps_row1", bufs=2, space="PSUM") as prow:
                psum_mu = prow.tile([1, TOK], F32, tag="row")
                psum_s2 = prow.tile([1, TOK], F32, tag="row")
                for ch in range(CH):
                    eng = nc.vector if ch % 2 == 0 else nc.gpsimd
                    eng.tensor_tensor(
                        sq8[:, ch, :], x8_sb[:, ch, :], x8_sb[:, ch, :], OP.mult
                    )
                for ch in range(CH):
                    nc.tensor.matmul(
                        psum_mu[:],
                        ones8[:],
                        x8_sb[:, ch, :],
                        start=(ch == 0),
                        stop=(ch == CH - 1),
                    )
                for ch in range(CH):
                    nc.tensor.matmul(
                        psum_s2[:],
                        ones8[:],
                        sq8[:, ch, :],
                        start=(ch == 0),
                        stop=(ch == CH - 1),
                    )
                rstd1_b, c1_b = ln_rows(psum_mu, psum_s2, "1")

            h1 = bigs.tile([P, CH, TOK], F8, tag="h1")
            for ch in range(CH):
                eng = nc.vector if ch % 2 == 0 else nc.gpsimd
                t1 = work.tile([P, TOK], F32, tag="t1")
                eng.tensor_tensor(t1[:], xr_sb[:, ch, :], rstd1_b[:], OP.mult)
                eng.tensor_tensor(h1[:, ch, :], t1[:], c1_b[:], OP.subtract)

            # ---- QKV ----
            st2 = tc.tile_pool(name="ps_mm2", bufs=5, space="PSUM")
            ps_acc = st2.__enter__()

            def qkv_block(wt, i, m, consume):
                psum = ps_acc.tile([P, TOK], F32, tag="acc", name=f"ps_{m}")
                for j in range(CH // 2):
                    nc.tensor.matmul(
                        psum[:],
                        wt[:, i, 2 * j : 2 * j + 2, :],
                        h1[:, 2 * j : 2 * j + 2, :],
                        start=(j == 0),
                        stop=(j == CH // 2 - 1),
                        perf_mode=DR,
                    )
                consume(m, psum)

            def k_consume(m, psum):
                k8 = stg.tile([P, TOK], F8, tag="cp", name=f"k8_{m}")
                nc.scalar.activation(
                    out=k8[:],
                    in_=psum[:],
                    func=AF.Identity,
                    bias=biasqk_sb[:, 8 + m : 9 + m],
                    scale=0.5,
                )
                nc.scalar.dma_start(vk_in[m * P : (m + 1) * P, :], k8[:])

            # K and V interleaved: K consumes on Act, V on DVE run in
            # parallel so the AllGather input completes sooner
            for half in range(2):
                wk = wpool.tile([P, 4, CH, P], F8, tag="wq", name=f"wk{half}")
                nc.sync.dma_start(
                    wk[:],
                    wqk8[8 + 4 * half : 12 + 4 * half].rearrange(
                        "i p (ch o) -> p i ch o", ch=CH
                    ),
                )
                wv_t = wpool.tile([P, CH, TOK], F8, tag="wq", name=f"wv{half}")
                nc.sync.dma_start(
                    wv_t[:], wv8[half].rearrange("p (ch v) -> p ch v", ch=CH)
                )
                for i in range(4):
                    qkv_block(wk, i, 4 * half + i, k_consume)
                    psum = ps_acc.tile(
                        [P, TOK], F32, tag="acc", name=f"pv{half}_{i}"
                    )
                    for j in range(CH // 2):
                        nc.tensor.matmul(
                            psum[:],
                            h1[:, 2 * j : 2 * j + 2, i * P : (i + 1) * P],
                            wv_t[:, 2 * j : 2 * j + 2, :],
                            start=(j == 0),
                            stop=(j == CH // 2 - 1),
                            perf_mode=DR,
                        )
                    v8 = stg.tile([P, TOK], F8, tag="cp", name=f"v8_{half}_{i}")
                    nc.vector.tensor_scalar_mul(v8[:], psum[:], 0.5)
                    dst = vv_in[
                        i * P : (i + 1) * P,
                        half * 8 * DHP : (half + 1) * 8 * DHP,
                    ].rearrange("t (h c) -> t h c", c=DHP)[:, :, 0:DH]
                    nc.gpsimd.dma_start(
                        dst, v8[:].rearrange("t (h d) -> t h d", d=DH)
                    )

            nc.gpsimd.collective_compute(
                "AllGather",
                OP.bypass,
                ins=[kv_in[:].opt()],
                outs=[kv_out[:].opt()],
                replica_groups=REPLICA_GROUPS,
            )

            # Q blocks -> SBUF (chunk 8 duplicates chunk 7 for the hp=7 rhs pair)
            qT = bigs.tile([P, CH + 1, TOK], F8, tag="qT")

            def q_consume(m, psum):
                nc.vector.tensor_scalar(
                    out=qT[:, m, :],
                    in0=psum[:],
                    scalar1=0.5,
                    scalar2=biasqk_sb[:, m : m + 1],
                    op0=OP.mult,
                    op1=OP.add,
                )

            for g in range(2):
                wt = wpool.tile([P, 4, CH, P], F8, tag="wq", name=f"wq{g}")
                nc.sync.dma_start(
                    wt[:],
                    wqk8[4 * g : 4 * g + 4].rearrange("i p (ch o) -> p i ch o", ch=CH),
                )
                for i in range(4):
                    qkv_block(wt, i, 4 * g + i, q_consume)
            nc.sync.dma_start(qT[:, CH, :], qT[:, CH - 1, :])
            st2.__exit__(None, None, None)

            # ---- attention ----
            # kp: [P(2 heads' d), 2, KC, P] fp8; slot0 = K data, slot1 = zeros
            kp_t = []
            for i in range(2):
                t = kvz.tile([P, 2, KC, P], F8, tag="kp", name=f"kp{i}")
                nc.vector.memset(t[:, 1, :, :].bitcast(U8), 0)
                kp_t.append(t)
            # vfull: [P(key-in-chunk), KC, 16*(64+1)] fp8, ones pre-gathered
            vfull = bigs.tile([P, KC, H * DHP], F8, tag="vfull")
            for r in range(RANKS):
                src = (
                    kv_or[r : r + 1, KV_K:KV_SZ]
                    .rearrange("o (t v) -> o t v", v=H * DHP)[0]
                    .rearrange("(tc p) v -> p tc v", p=P)
                )
                nc.sync.dma_start(vfull[:, 4 * r : 4 * r + 4, :], src)

            def load_kp(hp, t):
                src = (
                    kv_or[:, 0:KV_K]
                    .rearrange("r (f t) -> r f t", t=TOK)[
                        :, hp * P : (hp + 1) * P, :
                    ]
                    .rearrange("r p t -> p r t")
                )
                nc.sync.dma_start(t[:, 0, :, :], src)

            ctxT = bigs.tile([P, CH, TOK], F8, tag="x8")  # reuse x8 region
            groups = [(2 * j, 2) for j in range(8)]
            pairs_after = {j: [j] for j in range(8)}

            att_pools = (
                tc.tile_pool(name="ps_s", bufs=2, space="PSUM"),
                tc.tile_pool(name="ps_ctx", bufs=2, space="PSUM"),
            )
            ps_spool = att_pools[0].__enter__()
            ps_ctx = att_pools[1].__enter__()
            eidx = 0
            for hp in range(H // 2):
                kp = kp_t[hp % 2]
                load_kp(hp, kp)
                for hh in range(2):
                    h = 2 * hp + hh
                    half = slice(hh * DH, hh * DH + DH)
                    qpair = qT[half, hp : hp + 2, :]
                    pt = ppool.tile([P, KC, TOK], F8, tag="pt", name=f"pt{h}")
                    psum_c = ps_ctx.tile([DH + 1, TOK], F32, tag="ctx")
                    for gi, (kc0, nb) in enumerate(groups):
                        ps_s = ps_spool.tile([P, 2 * TOK], F32, tag="s", bufs=3)
                        for j in range(nb):
                            nc.tensor.matmul(
                                ps_s[:, j * TOK : (j + 1) * TOK],
                                kp[half, :, kc0 + j, :],
                                qpair,
                                start=True,
                                stop=True,
                                perf_mode=DR,
                            )
                        dst = pt[:, kc0 : kc0 + nb, :]
                        # alternate Act/DVE so consecutive groups of a head
                        # never serialize on one engine (GPSIMD can't read
                        # PSUM per the BIR verifier); 4:2 toward Act since
                        # DVE also carries reciprocal + normalize
                        if (gi + h) % 2 == 0 or (h * 8 + gi) % 21 == 3:
                            nc.scalar.activation(
                                out=dst,
                                in_=ps_s[:, : nb * TOK],
                                func=AF.Exp,
                                bias=negc_row[:],
                                scale=2.0**-11,
                            )
                        else:
                            nc.vector.tensor_scalar(
                                out=dst.bitcast(U8),
                                in0=ps_s[:, : nb * TOK],
                                scalar1=SA,
                                scalar2=SB,
                                op0=OP.mult,
                                op1=OP.add,
                            )
                        eidx += 1
                        for pj in pairs_after.get(gi, []):
                            nc.tensor.matmul(
                                psum_c[:],
                                vfull[
                                    :,
                                    2 * pj : 2 * pj + 2,
                                    h * DHP : h * DHP + DH + 1,
                                ],
                                pt[:, 2 * pj : 2 * pj + 2, :],
                                start=(pj == 0),
                                stop=(pj == KC // 2 - 1),
                                perf_mode=DR,
                            )
                    rrow = rows.tile([1, TOK], F32, tag="r", name=f"rr{h}")
                    nc.vector.reciprocal(rrow[:], psum_c[DH : DH + 1, :])
                    rb = bcpool.tile([DH, TOK], F32, tag="rb", name=f"rb{h}")
                    nc.gpsimd.partition_broadcast(rb[:], rrow[:])
                    nc.vector.tensor_tensor(
                        ctxT[half, hp, :], psum_c[0:DH, :], rb[:], OP.mult
                    )
            att_pools[1].__exit__(None, None, None)
            att_pools[0].__exit__(None, None, None)

            # ---- proj + residual ----
            st4 = tc.tile_pool(name="ps_mm4", bufs=5, space="PSUM")
            ps_mlp = st4.__enter__()
            x2 = bigs.tile([P, CH, TOK], F32R, tag="x2")
            for g in range(2):
                wt = wpool.tile([P, 4, CH, P], F8, tag="wq", name=f"wpj{g}")
                nc.sync.dma_start(
                    wt[:],
                    wp8[4 * g : 4 * g + 4].rearrange("i p (ch o) -> p i ch o", ch=CH),
                )
                for i in range(4):
                    m = 4 * g + i
                    psum = ps_mlp.tile([P, TOK], F32, tag="acc", name=f"pp_{m}")
                    for j in range(CH // 2):
                        nc.tensor.matmul(
                            psum[:],
                            wt[:, i, 2 * j : 2 * j + 2, :],
                            ctxT[:, 2 * j : 2 * j + 2, :],
                            start=(j == 0),
                            stop=(j == CH // 2 - 1),
                            perf_mode=DR,
                        )
                    attn_sb = stg.tile([P, TOK], F32, tag="stg", name=f"at_{m}")
                    nc.scalar.activation(
                        out=attn_sb[:],
                        in_=psum[:],
                        func=AF.Identity,
                        bias=biaspf_sb[:, m : m + 1],
                        scale=2.0**-9,
                    )
                    nc.gpsimd.tensor_tensor(
                        x2[:, m, :], attn_sb[:], xr_sb[:, m, :], OP.add
                    )  # f32r out: rounded on write for the LN2 stats matmul

            # ---- LN2 (fp32r stats on x2) ----
            with tc.tile_pool(name="ps_row2", bufs=2, space="PSUM") as prow:
                psum_mu2 = prow.tile([1, TOK], F32, tag="row")
                psum_s22 = prow.tile([1, TOK], F32, tag="row")
                for ch in range(CH):
                    nc.tensor.matmul(
                        psum_mu2[:],
                        ones_r[:],
                        x2[:, ch, :],
                        start=(ch == 0),
                        stop=(ch == CH - 1),
                    )
                    sq = work.tile([P, TOK], F32R, tag="t1", name=f"sq2_{ch}")
                    nc.gpsimd.tensor_tensor(
                        sq[:],
                        x2[:, ch, :].bitcast(F32),
                        x2[:, ch, :].bitcast(F32),
                        OP.mult,
                    )
                    nc.tensor.matmul(
                        psum_s22[:],
                        ones_r[:],
                        sq[:],
                        start=(ch == 0),
                        stop=(ch == CH - 1),
                    )
                rstd2_b, c2_b = ln_rows(psum_mu2, psum_s22, "2")

            # xn as fp8 hi/lo + duplicated-hi slot: [hi, hi_dup, lo]
            xnf = bigs.tile([P, CH, 3, TOK], F8, tag="xn")
            for ch in range(CH):
                t1 = work.tile([P, TOK], F32, tag="t1", name=f"t2_{ch}")
                nc.gpsimd.tensor_tensor(
                    t1[:], x2[:, ch, :].bitcast(F32), rstd2_b[:], OP.mult
                )
                xn32 = work.tile([P, TOK], F32, tag="xn32", bufs=2, name=f"x32_{ch}")
                nc.vector.tensor_tensor(xn32[:], t1[:], c2_b[:], OP.subtract)
                nc.scalar.activation(
                    out=xnf[:, ch, 0, :], in_=xn32[:], func=AF.Identity
                )
                nc.gpsimd.tensor_copy(
                    out=xnf[:, ch, 1, :], in_=xnf[:, ch, 0, :]
                )
                nc.vector.tensor_tensor(
                    xnf[:, ch, 2, :], xn32[:], xnf[:, ch, 0, :], OP.subtract
                )

            # ---- MLP: fc1 fp8 hi/lo "3-product" DoubleRow, fc2 bf16 ----
            # per chunk pair (c, d): [whi_c,wlo_c]x[xhi,xhidup], same for d,
            # then [whi_c,whi_d]x[xlo_c,xlo_d] (drops the negligible lo*lo)
            g2 = bigs.tile([P, HCH, 2, TOK], F8, tag="g")
            for g in range(CH):
                w1 = wpool.tile([P, 4, 2 * CH, P], F8, tag="w1", name=f"w1_{g}")
                nc.sync.dma_start(
                    w1[:],
                    w1f8[g].rearrange("p (i c o) -> p i c o", i=4, c=2 * CH),
                )
                for i in range(4):
                    m = 4 * g + i
                    psum = ps_mlp.tile([P, TOK], F32, tag="acc", name=f"p1_{m}")
                    for c2 in range(CH // 2):
                        c = 2 * c2
                        nc.tensor.matmul(
                            psum[:],
                            w1[:, i, 2 * c : 2 * c + 2, :],
                            xnf[:, c, 0:2, :],
                            start=(c2 == 0),
                            stop=False,
                            perf_mode=DR,
                        )
                        nc.tensor.matmul(
                            psum[:],
                            w1[:, i, 2 * c + 2 : 2 * c + 4, :],
                            xnf[:, c + 1, 0:2, :],
                            start=False,
                            stop=False,
                            perf_mode=DR,
                        )
                        whi = w1[:, i].rearrange(
                            "p (c two) k -> p c two k", two=2
                        )[:, c : c + 2, 0, :]
                        nc.tensor.matmul(
                            psum[:],
                            whi,
                            xnf[:, c : c + 2, 2, :],
                            start=False,
                            stop=(c2 == CH // 2 - 1),
                            perf_mode=DR,
                        )
                    g32 = work.tile([P, TOK], F32, tag="xn32", bufs=2, name=f"g32_{m}")
                    nc.scalar.activation(
                        out=g32[:],
                        in_=psum[:],
                        func=AF.Gelu,
                        bias=fc1b_sb[:, m : m + 1],
                        scale=2.0**-5,
                    )
                    nc.gpsimd.tensor_copy(out=g2[:, m, 0, :], in_=g32[:])
                    nc.vector.tensor_tensor(
                        g2[:, m, 1, :], g32[:], g2[:, m, 0, :], OP.subtract
                    )
            for m2 in range(CH):
                psum = ps_mlp.tile([P, TOK], F32, tag="acc", name=f"p2_{m2}")
                for hf in range(2):
                    w2 = wpool.tile(
                        [P, 3 * HCH // 2, P], F8, tag="w2", name=f"w2_{m2}_{hf}"
                    )
                    nc.sync.dma_start(
                        w2[:],
                        w2f8[m2][
                            :, hf * 3 * (HCH // 2) * P : (hf + 1) * 3 * (HCH // 2) * P
                        ].rearrange("p (c o) -> p c o", c=3 * HCH // 2),
                    )
                    w2lo = w2.rearrange("p (c three) k -> p c three k", three=3)
                    for c2 in range(HCH // 4):
                        c = 2 * c2
                        hc = hf * (HCH // 2) + c
                        nc.tensor.matmul(
                            psum[:],
                            w2[:, 3 * c : 3 * c + 2, :],
                            g2[:, hc, 0:2, :],
                            start=(hf == 0 and c2 == 0),
                            stop=False,
                            perf_mode=DR,
                        )
                        nc.tensor.matmul(
                            psum[:],
                            w2[:, 3 * c + 3 : 3 * c + 5, :],
                            g2[:, hc + 1, 0:2, :],
                            start=False,
                            stop=False,
                            perf_mode=DR,
                        )
                        nc.tensor.matmul(
                            psum[:],
                            w2lo[:, c : c + 2, 2, :],
                            g2[:, hc : hc + 2, 0, :],
                            start=False,
                            stop=(hf == 1 and c2 == HCH // 4 - 1),
                            perf_mode=DR,
                        )
                o_sb = stg.tile([P, TOK], F32, tag="stg", name=f"o_{m2}")
                nc.scalar.activation(
                    out=o_sb[:],
                    in_=psum[:],
                    func=AF.Identity,
                    bias=biaspf_sb[:, 8 + m2 : 9 + m2],
                    scale=2.0**-6,
                )
                o_f = stg.tile([P, TOK], F32, tag="of", bufs=2, name=f"of_{m2}")
                nc.vector.tensor_add(
                    out=o_f[:], in0=o_sb[:], in1=x2[:, m2, :].bitcast(F32)
                )
                nc.sync.dma_start(outT[m2 * P : (m2 + 1) * P, :], o_f[:])
            st4.__exit__(None, None, None)

    if do_compile:
        nc.compile()
    return nc


_CACHE = {}


def _get_program():
    if "nc" not in _CACHE:
        _CACHE["nc"] = build_program()
    return _CACHE["nc"]


def _prep_inputs(inputs):
    E4 = ml_dtypes.float8_e4m3
    x = np.asarray(inputs["x"], np.float32)
    g1 = np.asarray(inputs["ln1_g"], np.float32)
    b1 = np.asarray(inputs["ln1_b"], np.float32)
    g2 = np.asarray(inputs["ln2_g"], np.float32)
    b2 = np.asarray(inputs["ln2_b"], np.float32)
    qkv_w = np.asarray(inputs["qkv_w"], np.float32) * g1[None, :]
    proj_w = np.asarray(inputs["proj_w"], np.float32)
    fc1_w = np.asarray(inputs["fc1_w"], np.float32) * g2[None, :]
    fc2_w = np.asarray(inputs["fc2_w"], np.float32)

    qkv_bias = np.asarray(inputs["qkv_w"], np.float32) @ b1  # [3D]
    assert np.abs(qkv_bias[2 * D :]).max() == 0.0, "nonzero ln1_b v-bias unsupported"

    def wtile8(w, blocks):
        """w [O, D] -> [nb, P, CH*P] fp8 with [m, p, ch*128+o] = 32*w[m*128+o, ch*128+p]."""
        out = np.empty((len(blocks), P, CH * P), E4)
        for bi, m in enumerate(blocks):
            blk = w[m * P : (m + 1) * P, :] * 32.0  # [o 128, c 1024]
            out[bi] = (
                blk.reshape(P, CH, P).transpose(2, 1, 0).reshape(P, CH * P)
            ).astype(E4)
        return out

    def wtile8_hl(w, nb, batch):
        """w [O, D] (pre-scaled) -> [nb//batch, P, batch*2*chn*P] fp8 hi/lo
        tiles: slot (2c+s) holds hi (s=0) / lo residual (s=1) of chunk c."""
        chn = w.shape[1] // P
        out = np.empty((nb // batch, P, batch * 2 * chn * P), E4)
        for g in range(nb // batch):
            t = np.empty((P, batch, 2 * chn, P), E4)
            for i in range(batch):
                m = g * batch + i
                blk = w[m * P : (m + 1) * P, :]  # [o, c]
                wt = blk.reshape(P, chn, P).transpose(2, 1, 0)  # [p, c, o]
                hi = wt.astype(E4)
                lo = (wt - hi.astype(np.float32)).astype(E4)
                t[:, i, 0::2, :] = hi
                t[:, i, 1::2, :] = lo
            out[g] = t.reshape(P, -1)
        return out

    def wtile8_t3(w):
        """w [O, D] (pre-scaled) -> [O//P, P, 3*chn*P] fp8 (hi, hi, lo)."""
        chn = w.shape[1] // P
        out = np.empty((w.shape[0] // P, P, 3 * chn * P), E4)
        for m in range(w.shape[0] // P):
            blk = w[m * P : (m + 1) * P, :]
            wt = blk.reshape(P, chn, P).transpose(2, 1, 0)  # [p, c, o]
            hi = wt.astype(E4)
            lo = (wt - hi.astype(np.float32)).astype(E4)
            t = np.empty((P, chn, 3, P), E4)
            t[:, :, 0, :] = hi
            t[:, :, 1, :] = hi
            t[:, :, 2, :] = lo
            out[m] = t.reshape(P, -1)
        return out

    def wtile_bf(w, nb, batch):
        """w [O, D] -> [nb//batch, P, batch*CH*P] bf16 tiles."""
        out = np.empty((nb // batch, P, batch * (w.shape[1] // P) * P), ml_dtypes.bfloat16)
        chn = w.shape[1] // P
        for g in range(nb // batch):
            t = np.empty((P, batch, chn, P), np.float32)
            for i in range(batch):
                m = g * batch + i
                blk = w[m * P : (m + 1) * P, :]  # [o, c]
                t[:, i] = blk.reshape(P, chn, P).transpose(2, 1, 0)
            out[g] = t.reshape(P, -1).astype(ml_dtypes.bfloat16)
        return out

    # V weights token-major: [ph, p, ch*512+vc] = 32*qkv_w'[2D+ph*512+vc, ch*128+p]
    wv = np.empty((2, P, CH * TOK), E4)
    for ph in range(2):
        blk = qkv_w[2 * D + ph * TOK : 2 * D + (ph + 1) * TOK, :] * 32.0  # [vc, c]
        wv[ph] = blk.reshape(TOK, CH, P).transpose(2, 1, 0).reshape(P, CH * TOK).astype(E4)

    bqk = np.zeros((P, 16), np.float32)
    bqk[:, 0:8] = _stripe(16.0 * qkv_bias[0:D])
    bqk[:, 8:16] = _stripe(16.0 * qkv_bias[D : 2 * D])
    bpf = np.zeros((P, 16), np.float32)
    bpf[:, 0:8] = _stripe(inputs["proj_b"])
    bpf[:, 8:16] = _stripe(inputs["fc2_b"])

    shared = {
        "wqk8": wtile8(qkv_w, list(range(16))),
        "wv8": wv,
        "wp8": wtile8(proj_w, list(range(CH))),
        "w1f8": wtile8_hl(fc1_w * 32.0, HCH, 4),
        "w2f8": wtile8_t3(fc2_w * 64.0),
        "biasqk": bqk,
        "biaspf": bpf,
        "fc1b": _stripe(
            np.asarray(inputs["fc1_b"], np.float32)
            + np.asarray(inputs["fc1_w"], np.float32) @ b2
        ),
    }
    in_maps = []
    for c in range(NCORES):
        b, blk = divmod(c, RANKS)
        xblk = x[b, blk * TOK : (blk + 1) * TOK, :]  # [TOK, D]
        xt = round_fp32r(np.ascontiguousarray(xblk.T))
        m = dict(shared)
        m["xT"] = xt
        m["x8T"] = xt.astype(E4)
        in_maps.append(m)
    return in_maps


def _assemble(results):
    out = np.empty((B, N, D), dtype=np.float32)
    for c in range(NCORES):
        b, blk = divmod(c, RANKS)
        out[b, blk * TOK : (blk + 1) * TOK, :] = results[c]["outT"].T
    return out


def run_device(inputs, **kwargs):
    nc = _get_program()
    in_maps = _prep_inputs(inputs)
    res = run_bass_kernel_spmd(nc, in_maps, core_ids=list(range(NCORES)), **kwargs)
    return _assemble(res.results), res


def kernel(**inputs) -> np.ndarray:
    out, _ = run_device(inputs)
    return out


# revision 15
# speedup vs baseline: 1.0623x; 1.0032x over previous
"""Trainium2 Bass kernel v2: fp8 DoubleRow attention + bf16 MLP.

Sharding: sequence-parallel over 8 cores (512 tokens each, batch = core//4).
One 4-rank AllGather carries fp8 K (feature-major) + fp8 V (token-major).

Precision plan (validated in numpy, max_rel ~1.0e-2 vs 2e-2 gate):
  - weights qkv/proj: e4m3 x32 host-scaled; fc1/fc2: bf16 (MLP dominates error)
  - h1/q/k/v/pt/ctx: e4m3 (q,k,v at sigma~16 via 0.5 consume scale)
  - softmax exp: constant shift C=4 (cancels in normalize); split between
    Act (native Exp -> fp8) and DVE (Schraudolph: psum*a+b -> uint8 whose
    bit pattern IS e4m3 2^x; floor-vs-round ambiguity is a constant factor
    that cancels in the softmax normalize)
  - x residual fp32r; LN stats via ones-matmul (fp8 DoubleRow for LN1 on
    host-provided x8, fp32r for LN2)
DoubleRow pair slots: chunk pairs for QKV/ctx/proj; (k, zeros) for scores
(d=64 contraction cannot pair; zero slot makes the 0.5 cyc/row rate legal).
"""
import sys

sys.path.insert(0, "/opt/trn_rl_repo")
import numpy as np
import ml_dtypes
import concourse.bass as bass
import concourse.mybir as mybir
import concourse.tile as tile
from concourse import bacc
from concourse.bass_utils import run_bass_kernel_spmd

B, N, D = 2, 2048, 1024
H, DH = 16, 64
HID = 4096
NCORES = 8
TOK = (B * N) // NCORES  # 512
EPS = 1e-5
SCALE = DH**-0.5
P = 128
CH = D // P  # 8
KC = N // P  # 16
HCH = HID // P  # 32
RANKS = 4
CSH = 4.0  # exp arg shift, cancels in softmax
LN2_ = float(np.log(2.0))
# Schraudolph uint8-as-e4m3: y = psum * SA + SB
SA = 8.0 * (2.0**-11) / LN2_
SB = 56.5 - 8.0 * CSH / LN2_

F32 = mybir.dt.float32
F32R = mybir.dt.float32r
F8 = mybir.dt.float8e4
BF = mybir.dt.bfloat16
U8 = mybir.dt.uint8
AF = mybir.ActivationFunctionType
OP = mybir.AluOpType
DR = mybir.MatmulPerfMode.DoubleRow

REPLICA_GROUPS = [[0, 1, 2, 3], [4, 5, 6, 7]]

KV_K = D * TOK  # bytes of K region (fp8 feature-major [1024, 512])
DHP = DH + 16  # per-head stride in V region: 64 v + 1 ones + 15 pad
# (dual-fp8 LdWeights requires 16B-aligned weight base addresses)
KV_V = TOK * (H * DHP)  # V region [512, 1280]
KV_SZ = KV_K + KV_V


def round_fp32r(x: np.ndarray) -> np.ndarray:
    u = np.ascontiguousarray(x, dtype=np.float32).view(np.uint32)
    u = (u + 0x7FF + ((u >> 12) & 1)) & np.uint32(0xFFFFF000)
    return u.view(np.float32)


def _stripe(v: np.ndarray) -> np.ndarray:
    """[M] -> [P, M//P] with col m, part p = v[m*128+p]."""
    return np.ascontiguousarray(np.asarray(v, np.float32).reshape(-1, P).T)


def build_program(do_compile=True):
    nc = bacc.Bacc("TRN2", target_bir_lowering=False, debug=False, num_devices=NCORES)

    xT = nc.dram_tensor("xT", [D, TOK], F32, kind="ExternalInput").ap()
    x8T = nc.dram_tensor("x8T", [D, TOK], F8, kind="ExternalInput").ap()
    # weight tiles, DMA-contiguous per partition
    wqk8 = nc.dram_tensor("wqk8", [16, P, CH * P], F8, kind="ExternalInput").ap()
    wv8 = nc.dram_tensor("wv8", [2, P, CH * TOK], F8, kind="ExternalInput").ap()
    wp8 = nc.dram_tensor("wp8", [CH, P, CH * P], F8, kind="ExternalInput").ap()
    # fc1 weights as fp8 hi/lo pairs [.., (chunk, hi/lo), out]; fc2 stays bf16
    w1f8 = nc.dram_tensor(
        "w1f8", [CH, P, 4 * 2 * CH * P], F8, kind="ExternalInput"
    ).ap()
    # fc2 weights fp8 (hi, hi_dup, lo) triples per chunk (dups host-side)
    w2f8 = nc.dram_tensor(
        "w2f8", [CH, P, 3 * HCH * P], F8, kind="ExternalInput"
    ).ap()
    biasqk = nc.dram_tensor("biasqk", [P, 16], F32, kind="ExternalInput").ap()
    biaspf = nc.dram_tensor("biaspf", [P, 16], F32, kind="ExternalInput").ap()
    fc1b = nc.dram_tensor("fc1b", [P, HCH], F32, kind="ExternalInput").ap()
    outT = nc.dram_tensor("outT", [D, TOK], F32, kind="ExternalOutput").ap()

    with tile.TileContext(nc) as tc:
        with (
            tc.tile_pool(name="consts", bufs=1) as consts,
            tc.tile_pool(name="bigs", bufs=1) as bigs,
            tc.tile_pool(name="work", bufs=3) as work,
            tc.tile_pool(name="wpool", bufs=2) as wpool,
            tc.tile_pool(name="kvz", bufs=2) as kvz,
            tc.tile_pool(name="pp", bufs=2) as ppool,
            tc.tile_pool(name="rows", bufs=3) as rows,
            tc.tile_pool(name="bc", bufs=2) as bcpool,
            tc.tile_pool(name="stg", bufs=2) as stg,
            tc.tile_pool(name="dram", bufs=1, space="DRAM") as dram,
        ):
            ones8 = consts.tile([P, 1], F8)
            nc.vector.memset(ones8[:].bitcast(U8), 0x38)  # e4m3 1.0
            ones_r = consts.tile([P, 1], F32R)
            nc.vector.memset(ones_r[:].bitcast(F32), 1.0)
            eps_row = consts.tile([1, 1], F32, tag="eps")
            nc.vector.memset(eps_row[:], EPS)
            negc_row = consts.tile([P, 1], F32, tag="negc")
            nc.vector.memset(negc_row[:], -CSH)
            onesv = consts.tile([P, 4, H], F8, tag="onesv")
            nc.vector.memset(onesv[:].bitcast(U8), 0x38)
            biasqk_sb = consts.tile([P, 16], F32, tag="bqk")
            biaspf_sb = consts.tile([P, 16], F32, tag="bpf")
            fc1b_sb = consts.tile([P, HCH], F32, tag="b1")
            nc.sync.dma_start(biasqk_sb[:], biasqk[:])
            nc.sync.dma_start(biaspf_sb[:], biaspf[:])
            nc.sync.dma_start(fc1b_sb[:], fc1b[:])

            kv_in = dram.tile([KV_SZ], F8, tag="kvin")
            kv_out = dram.tile([RANKS * KV_SZ], F8, tag="kvout")
            vk_in = kv_in[0:KV_K].rearrange("(f t) -> f t", t=TOK)
            vv_in = kv_in[KV_K:KV_SZ].rearrange(
                "(t v) -> t v", v=H * DHP
            )
            kv_or = kv_out[:].rearrange("(r x) -> r x", r=RANKS)
            # ones column of the V region: written up-front (no data deps)
            for tc_ in range(RANKS):
                ones_dst = vv_in[tc_ * P : (tc_ + 1) * P, :].rearrange(
                    "p (h c) -> p h c", c=DHP
                )[:, :, DH : DH + 1]
                nc.sync.dma_start(
                    ones_dst, onesv[:, tc_, :].rearrange("p (h c) -> p h c", c=1)
                )

            # ---- LN1 ---- (x8 first: stats depend on it; xr arrives later)
            x8_sb = bigs.tile([P, CH, TOK], F8, tag="x8")
            nc.sync.dma_start(
                x8_sb[:], x8T.rearrange("(ch p) t -> p ch t", p=P)
            )
            xr_sb = bigs.tile([P, CH, TOK], F32, tag="xr")
            nc.sync.dma_start(
                xr_sb[:], xT.rearrange("(ch p) t -> p ch t", p=P)
            )
            sq8 = bigs.tile([P, CH, TOK], F8, tag="h1")  # released before h1

            def ln_rows(psum_mu, psum_s2, name):
                mu = rows.tile([1, TOK], F32, tag="r", name=f"mu{name}")
                nc.vector.tensor_scalar_mul(mu[:], psum_mu[:], 1.0 / D)
                var = rows.tile([1, TOK], F32, tag="r", name=f"va{name}")
                nc.vector.tensor_tensor(var[:], mu[:], mu[:], OP.mult)
                ex2 = rows.tile([1, TOK], F32, tag="r", name=f"e2{name}")
                nc.vector.tensor_scalar_mul(ex2[:], psum_s2[:], 1.0 / D)
                nc.vector.tensor_sub(var[:], ex2[:], var[:])
                rstd = rows.tile([1, TOK], F32, tag="r", name=f"rs{name}")
                nc.scalar.activation(
                    out=rstd[:], in_=var[:], func=AF.Sqrt, bias=eps_row[:]
                )
                nc.vector.reciprocal(rstd[:], rstd[:])
                cpos = rows.tile([1, TOK], F32, tag="r", name=f"cp{name}")
                nc.vector.tensor_tensor(cpos[:], mu[:], rstd[:], OP.mult)
                rstd_b = bcpool.tile([P, TOK], F32, tag="bc", name=f"rb{name}")
                nc.gpsimd.partition_broadcast(rstd_b[:], rstd[:])
                c_b = bcpool.tile([P, TOK], F32, tag="bc", name=f"cb{name}")
                nc.gpsimd.partition_broadcast(c_b[:], cpos[:])
                return rstd_b, c_b

            with tc.tile_pool(name="ps_row1", bufs=2, space="PSUM") as prow:
                psum_mu = prow.tile([1, TOK], F32, tag="row")
                psum_s2 = prow.tile([1, TOK], F32, tag="row")
                for ch in range(CH):
                    eng = nc.vector if ch % 2 == 0 else nc.gpsimd
                    eng.tensor_tensor(
                        sq8[:, ch, :], x8_sb[:, ch, :], x8_sb[:, ch, :], OP.mult
                    )
                for ch in range(CH):
                    nc.tensor.matmul(
                        psum_mu[:],
                        ones8[:],
                        x8_sb[:, ch, :],
                        start=(ch == 0),
                        stop=(ch == CH - 1),
                    )
                for ch in range(CH):
                    nc.tensor.matmul(
                        psum_s2[:],
                        ones8[:],
                        sq8[:, ch, :],
                        start=(ch == 0),
                        stop=(ch == CH - 1),
                    )
                rstd1_b, c1_b = ln_rows(psum_mu, psum_s2, "1")

            h1 = bigs.tile([P, CH, TOK], F8, tag="h1")
            for ch in range(CH):
                eng = nc.vector if ch % 2 == 0 else nc.gpsimd
                t1 = work.tile([P, TOK], F32, tag="t1")
                eng.tensor_tensor(t1[:], xr_sb[:, ch, :], rstd1_b[:], OP.mult)
                eng.tensor_tensor(h1[:, ch, :], t1[:], c1_b[:], OP.subtract)

            # ---- QKV ----
            st2 = tc.tile_pool(name="ps_mm2", bufs=5, space="PSUM")
            ps_acc = st2.__enter__()

            def qkv_block(wt, i, m, consume):
                psum = ps_acc.tile([P, TOK], F32, tag="acc", name=f"ps_{m}")
                for j in range(CH // 2):
                    nc.tensor.matmul(
                        psum[:],
                        wt[:, i, 2 * j : 2 * j + 2, :],
                        h1[:, 2 * j : 2 * j + 2, :],
                        start=(j == 0),
                        stop=(j == CH // 2 - 1),
                        perf_mode=DR,
                    )
                consume(m, psum)

            def k_consume(m, psum):
                k8 = stg.tile([P, TOK], F8, tag="cp", name=f"k8_{m}")
                nc.scalar.activation(
                    out=k8[:],
                    in_=psum[:],
                    func=AF.Identity,
                    bias=biasqk_sb[:, 8 + m : 9 + m],
                    scale=0.5,
                )
                nc.scalar.dma_start(vk_in[m * P : (m + 1) * P, :], k8[:])

            # K and V interleaved: K consumes on Act, V on DVE run in
            # parallel so the AllGather input completes sooner
            for half in range(2):
                wk = wpool.tile([P, 4, CH, P], F8, tag="wq", name=f"wk{half}")
                nc.sync.dma_start(
                    wk[:],
                    wqk8[8 + 4 * half : 12 + 4 * half].rearrange(
                        "i p (ch o) -> p i ch o", ch=CH
                    ),
                )
                wv_t = wpool.tile([P, CH, TOK], F8, tag="wq", name=f"wv{half}")
                nc.sync.dma_start(
                    wv_t[:], wv8[half].rearrange("p (ch v) -> p ch v", ch=CH)
                )
                for i in range(4):
                    qkv_block(wk, i, 4 * half + i, k_consume)
                    psum = ps_acc.tile(
                        [P, TOK], F32, tag="acc", name=f"pv{half}_{i}"
                    )
                    for j in range(CH // 2):
                        nc.tensor.matmul(
                            psum[:],
                            h1[:, 2 * j : 2 * j + 2, i * P : (i + 1) * P],
                            wv_t[:, 2 * j : 2 * j + 2, :],
                            start=(j == 0),
                            stop=(j == CH // 2 - 1),
                            perf_mode=DR,
                        )
                    v8 = stg.tile([P, TOK], F8, tag="cp", name=f"v8_{half}_{i}")
                    nc.vector.tensor_scalar_mul(v8[:], psum[:], 0.5)
                    dst = vv_in[
                        i * P : (i + 1) * P,
                        half * 8 * DHP : (half + 1) * 8 * DHP,
                    ].rearrange("t (h c) -> t h c", c=DHP)[:, :, 0:DH]
                    nc.gpsimd.dma_start(
                        dst, v8[:].rearrange("t (h d) -> t h d", d=DH)
                    )

            nc.gpsimd.collective_compute(
                "AllGather",
                OP.bypass,
                ins=[kv_in[:].opt()],
                outs=[kv_out[:].opt()],
                replica_groups=REPLICA_GROUPS,
            )

            # Q blocks -> SBUF (chunk 8 duplicates chunk 7 for the hp=7 rhs pair)
            qT = bigs.tile([P, CH + 1, TOK], F8, tag="qT")

            def q_consume(m, psum):
                nc.vector.tensor_scalar(
                    out=qT[:, m, :],
                    in0=psum[:],
                    scalar1=0.5,
                    scalar2=biasqk_sb[:, m : m + 1],
                    op0=OP.mult,
                    op1=OP.add,
                )

            for g in range(2):
                wt = wpool.tile([P, 4, CH, P], F8, tag="wq", name=f"wq{g}")
                nc.sync.dma_start(
                    wt[:],
                    wqk8[4 * g : 4 * g + 4].rearrange("i p (ch o) -> p i ch o", ch=CH),
                )
                for i in range(4):
                    qkv_block(wt, i, 4 * g + i, q_consume)
            nc.sync.dma_start(qT[:, CH, :], qT[:, CH - 1, :])
            st2.__exit__(None, None, None)

            # ---- attention ----
            # kp: [P(2 heads' d), 2, KC, P] fp8; slot0 = K data, slot1 = zeros
            kp_t = []
            for i in range(2):
                t = kvz.tile([P, 2, KC, P], F8, tag="kp", name=f"kp{i}")
                nc.vector.memset(t[:, 1, :, :].bitcast(U8), 0)
                kp_t.append(t)
            # vfull: [P(key-in-chunk), KC, 16*(64+1)] fp8, ones pre-gathered
            vfull = bigs.tile([P, KC, H * DHP], F8, tag="vfull")
            for r in range(RANKS):
                src = (
                    kv_or[r : r + 1, KV_K:KV_SZ]
                    .rearrange("o (t v) -> o t v", v=H * DHP)[0]
                    .rearrange("(tc p) v -> p tc v", p=P)
                )
                nc.sync.dma_start(vfull[:, 4 * r : 4 * r + 4, :], src)

            def load_kp(hp, t):
                src = (
                    kv_or[:, 0:KV_K]
                    .rearrange("r (f t) -> r f t", t=TOK)[
                        :, hp * P : (hp + 1) * P, :
                    ]
                    .rearrange("r p t -> p r t")
                )
                nc.sync.dma_start(t[:, 0, :, :], src)

            ctxT = bigs.tile([P, CH, TOK], F8, tag="x8")  # reuse x8 region
            groups = [(2 * j, 2) for j in range(8)]
            pairs_after = {j: [j] for j in range(8)}

            att_pools = (
                tc.tile_pool(name="ps_s", bufs=2, space="PSUM"),
                tc.tile_pool(name="ps_ctx", bufs=2, space="PSUM"),
            )
            ps_spool = att_pools[0].__enter__()
            ps_ctx = att_pools[1].__enter__()
            eidx = 0
            for hp in range(H // 2):
                kp = kp_t[hp % 2]
                load_kp(hp, kp)
                for hh in range(2):
                    h = 2 * hp + hh
                    half = slice(hh * DH, hh * DH + DH)
                    qpair = qT[half, hp : hp + 2, :]
                    pt = ppool.tile([P, KC, TOK], F8, tag="pt", name=f"pt{h}")
                    psum_c = ps_ctx.tile([DH + 1, TOK], F32, tag="ctx")
                    for gi, (kc0, nb) in enumerate(groups):
                        ps_s = ps_spool.tile([P, 2 * TOK], F32, tag="s", bufs=3)
                        for j in range(nb):
                            nc.tensor.matmul(
                                ps_s[:, j * TOK : (j + 1) * TOK],
                                kp[half, :, kc0 + j, :],
                                qpair,
                                start=True,
                                stop=True,
                                perf_mode=DR,
                            )
                        dst = pt[:, kc0 : kc0 + nb, :]
                        # alternate Act/DVE so consecutive groups of a head
                        # never serialize on one engine (GPSIMD can't read
                        # PSUM per the BIR verifier); 4:2 toward Act since
                        # DVE also carries reciprocal + normalize
                        if (gi + h) % 2 == 0 or (h * 8 + gi) % 14 == 3:
                            nc.scalar.activation(
                                out=dst,
                                in_=ps_s[:, : nb * TOK],
                                func=AF.Exp,
                                bias=negc_row[:],
                                scale=2.0**-11,
                            )
                        else:
                            nc.vector.tensor_scalar(
                                out=dst.bitcast(U8),
                                in0=ps_s[:, : nb * TOK],
                                scalar1=SA,
                                scalar2=SB,
                                op0=OP.mult,
                                op1=OP.add,
                            )
                        eidx += 1
                        for pj in pairs_after.get(gi, []):
                            nc.tensor.matmul(
                                psum_c[:],
                                vfull[
                                    :,
                                    2 * pj : 2 * pj + 2,
                                    h * DHP : h * DHP + DH + 1,
                                ],
                                pt[:, 2 * pj : 2 * pj + 2, :],
                                start=(pj == 0),
                                stop=(pj == KC // 2 - 1),
                                perf_mode=DR,
                            )
                    rrow = rows.tile([1, TOK], F32, tag="r", name=f"rr{h}")
                    nc.vector.reciprocal(rrow[:], psum_c[DH : DH + 1, :])
                    rb = bcpool.tile([DH, TOK], F32, tag="rb", name=f"rb{h}")
                    nc.gpsimd.partition_broadcast(rb[:], rrow[:])
                    nc.vector.tensor_tensor(
                        ctxT[half, hp, :], psum_c[0:DH, :], rb[:], OP.mult
                    )
            att_pools[1].__exit__(None, None, None)
            att_pools[0].__exit__(None, None, None)

            # ---- proj + residual ----
            st4 = tc.tile_pool(name="ps_mm4", bufs=5, space="PSUM")
            ps_mlp = st4.__enter__()
            x2 = bigs.tile([P, CH, TOK], F32R, tag="x2")
            for g in range(2):
                wt = wpool.tile([P, 4, CH, P], F8, tag="wq", name=f"wpj{g}")
                nc.sync.dma_start(
                    wt[:],
                    wp8[4 * g : 4 * g + 4].rearrange("i p (ch o) -> p i ch o", ch=CH),
                )
                for i in range(4):
                    m = 4 * g + i
                    psum = ps_mlp.tile([P, TOK], F32, tag="acc", name=f"pp_{m}")
                    for j in range(CH // 2):
                        nc.tensor.matmul(
                            psum[:],
                            wt[:, i, 2 * j : 2 * j + 2, :],
                            ctxT[:, 2 * j : 2 * j + 2, :],
                            start=(j == 0),
                            stop=(j == CH // 2 - 1),
                            perf_mode=DR,
                        )
                    attn_sb = stg.tile([P, TOK], F32, tag="stg", name=f"at_{m}")
                    nc.scalar.activation(
                        out=attn_sb[:],
                        in_=psum[:],
                        func=AF.Identity,
                        bias=biaspf_sb[:, m : m + 1],
                        scale=2.0**-9,
                    )
                    nc.gpsimd.tensor_tensor(
                        x2[:, m, :], attn_sb[:], xr_sb[:, m, :], OP.add
                    )  # f32r out: rounded on write for the LN2 stats matmul

            # ---- LN2 (fp32r stats on x2) ----
            with tc.tile_pool(name="ps_row2", bufs=2, space="PSUM") as prow:
                psum_mu2 = prow.tile([1, TOK], F32, tag="row")
                psum_s22 = prow.tile([1, TOK], F32, tag="row")
                for ch in range(CH):
                    nc.tensor.matmul(
                        psum_mu2[:],
                        ones_r[:],
                        x2[:, ch, :],
                        start=(ch == 0),
                        stop=(ch == CH - 1),
                    )
                    sq = work.tile([P, TOK], F32R, tag="t1", name=f"sq2_{ch}")
                    nc.gpsimd.tensor_tensor(
                        sq[:],
                        x2[:, ch, :].bitcast(F32),
                        x2[:, ch, :].bitcast(F32),
                        OP.mult,
                    )
                    nc.tensor.matmul(
                        psum_s22[:],
                        ones_r[:],
                        sq[:],
                        start=(ch == 0),
                        stop=(ch == CH - 1),
                    )
                rstd2_b, c2_b = ln_rows(psum_mu2, psum_s22, "2")

            # xn as fp8 hi/lo + duplicated-hi slot: [hi, hi_dup, lo]
            xnf = bigs.tile([P, CH, 3, TOK], F8, tag="xn")
            for ch in range(CH):
                t1 = work.tile([P, TOK], F32, tag="t1", name=f"t2_{ch}")
                nc.gpsimd.tensor_tensor(
                    t1[:], x2[:, ch, :].bitcast(F32), rstd2_b[:], OP.mult
                )
                xn32 = work.tile([P, TOK], F32, tag="xn32", bufs=2, name=f"x32_{ch}")
                nc.vector.tensor_tensor(xn32[:], t1[:], c2_b[:], OP.subtract)
                nc.scalar.activation(
                    out=xnf[:, ch, 0, :], in_=xn32[:], func=AF.Identity
                )
                nc.gpsimd.tensor_copy(
                    out=xnf[:, ch, 1, :], in_=xnf[:, ch, 0, :]
                )
                nc.vector.tensor_tensor(
                    xnf[:, ch, 2, :], xn32[:], xnf[:, ch, 0, :], OP.subtract
                )

            # ---- MLP: fc1 fp8 hi/lo "3-product" DoubleRow, fc2 bf16 ----
            # per chunk pair (c, d): [whi_c,wlo_c]x[xhi,xhidup], same for d,
            # then [whi_c,whi_d]x[xlo_c,xlo_d] (drops the negligible lo*lo)
            g2 = bigs.tile([P, HCH, 2, TOK], F8, tag="g")
            for g in range(CH):
                w1 = wpool.tile([P, 4, 2 * CH, P], F8, tag="w1", name=f"w1_{g}")
                nc.sync.dma_start(
                    w1[:],
                    w1f8[g].rearrange("p (i c o) -> p i c o", i=4, c=2 * CH),
                )
                for i in range(4):
                    m = 4 * g + i
                    psum = ps_mlp.tile([P, TOK], F32, tag="acc", name=f"p1_{m}")
                    for c2 in range(CH // 2):
                        c = 2 * c2
                        nc.tensor.matmul(
                            psum[:],
                            w1[:, i, 2 * c : 2 * c + 2, :],
                            xnf[:, c, 0:2, :],
                            start=(c2 == 0),
                            stop=False,
                            perf_mode=DR,
                        )
                        nc.tensor.matmul(
                            psum[:],
                            w1[:, i, 2 * c + 2 : 2 * c + 4, :],
                            xnf[:, c + 1, 0:2, :],
                            start=False,
                            stop=False,
                            perf_mode=DR,
                        )
                        whi = w1[:, i].rearrange(
                            "p (c two) k -> p c two k", two=2
                        )[:, c : c + 2, 0, :]
                        nc.tensor.matmul(
                            psum[:],
                            whi,
                            xnf[:, c : c + 2, 2, :],
                            start=False,
                            stop=(c2 == CH // 2 - 1),
                            perf_mode=DR,
                        )
                    g32 = work.tile([P, TOK], F32, tag="xn32", bufs=2, name=f"g32_{m}")
                    nc.scalar.activation(
                        out=g32[:],
                        in_=psum[:],
                        func=AF.Gelu,
                        bias=fc1b_sb[:, m : m + 1],
                        scale=2.0**-5,
                    )
                    nc.gpsimd.tensor_copy(out=g2[:, m, 0, :], in_=g32[:])
                    nc.vector.tensor_tensor(
                        g2[:, m, 1, :], g32[:], g2[:, m, 0, :], OP.subtract
                    )
            for m2 in range(CH):
                psum = ps_mlp.tile([P, TOK], F32, tag="acc", name=f"p2_{m2}")
                for hf in range(2):
                    w2 = wpool.tile(
                        [P, 3 * HCH // 2, P], F8, tag="w2", name=f"w2_{m2}_{hf}"
                    )
                    nc.sync.dma_start(
                        w2[:],
                        w2f8[m2][
                            :, hf * 3 * (HCH // 2) * P : (hf + 1) * 3 * (HCH // 2) * P
                        ].rearrange("p (c o) -> p c o", c=3 * HCH // 2),
                    )
                    w2lo = w2.rearrange("p (c three) k -> p c three k", three=3)
                    for c2 in range(HCH // 4):
                        c = 2 * c2
                        hc = hf * (HCH // 2) + c
                        nc.tensor.matmul(
                            psum[:],
                            w2[:, 3 * c : 3 * c + 2, :],
                            g2[:, hc, 0:2, :],
                            start=(hf == 0 and c2 == 0),
                            stop=False,
                            perf_mode=DR,
                        )
                        nc.tensor.matmul(
                            psum[:],
                            w2[:, 3 * c + 3 : 3 * c + 5, :],
                            g2[:, hc + 1, 0:2, :],
                            start=False,
                            stop=False,
                            perf_mode=DR,
                        )
                        nc.tensor.matmul(
                            psum[:],
                            w2lo[:, c : c + 2, 2, :],
                            g2[:, hc : hc + 2, 0, :],
                            start=False,
                            stop=(hf == 1 and c2 == HCH // 4 - 1),
                            perf_mode=DR,
                        )
                o_sb = stg.tile([P, TOK], F32, tag="stg", name=f"o_{m2}")
                nc.scalar.activation(
                    out=o_sb[:],
                    in_=psum[:],
                    func=AF.Identity,
                    bias=biaspf_sb[:, 8 + m2 : 9 + m2],
                    scale=2.0**-6,
                )
                o_f = stg.tile([P, TOK], F32, tag="of", bufs=2, name=f"of_{m2}")
                nc.vector.tensor_add(
                    out=o_f[:], in0=o_sb[:], in1=x2[:, m2, :].bitcast(F32)
                )
                nc.sync.dma_start(outT[m2 * P : (m2 + 1) * P, :], o_f[:])
            st4.__exit__(None, None, None)

    if do_compile:
        nc.compile()
    return nc


_CACHE = {}


def _get_program():
    if "nc" not in _CACHE:
        _CACHE["nc"] = build_program()
    return _CACHE["nc"]


def _prep_inputs(inputs):
    E4 = ml_dtypes.float8_e4m3
    x = np.asarray(inputs["x"], np.float32)
    g1 = np.asarray(inputs["ln1_g"], np.float32)
    b1 = np.asarray(inputs["ln1_b"], np.float32)
    g2 = np.asarray(inputs["ln2_g"], np.float32)
    b2 = np.asarray(inputs["ln2_b"], np.float32)
    qkv_w = np.asarray(inputs["qkv_w"], np.float32) * g1[None, :]
    proj_w = np.asarray(inputs["proj_w"], np.float32)
    fc1_w = np.asarray(inputs["fc1_w"], np.float32) * g2[None, :]
    fc2_w = np.asarray(inputs["fc2_w"], np.float32)

    qkv_bias = np.asarray(inputs["qkv_w"], np.float32) @ b1  # [3D]
    assert np.abs(qkv_bias[2 * D :]).max() == 0.0, "nonzero ln1_b v-bias unsupported"

    def wtile8(w, blocks):
        """w [O, D] -> [nb, P, CH*P] fp8 with [m, p, ch*128+o] = 32*w[m*128+o, ch*128+p]."""
        out = np.empty((len(blocks), P, CH * P), E4)
        for bi, m in enumerate(blocks):
            blk = w[m * P : (m + 1) * P, :] * 32.0  # [o 128, c 1024]
            out[bi] = (
                blk.reshape(P, CH, P).transpose(2, 1, 0).reshape(P, CH * P)
            ).astype(E4)
        return out

    def wtile8_hl(w, nb, batch):
        """w [O, D] (pre-scaled) -> [nb//batch, P, batch*2*chn*P] fp8 hi/lo
        tiles: slot (2c+s) holds hi (s=0) / lo residual (s=1) of chunk c."""
        chn = w.shape[1] // P
        out = np.empty((nb // batch, P, batch * 2 * chn * P), E4)
        for g in range(nb // batch):
            t = np.empty((P, batch, 2 * chn, P), E4)
            for i in range(batch):
                m = g * batch + i
                blk = w[m * P : (m + 1) * P, :]  # [o, c]
                wt = blk.reshape(P, chn, P).transpose(2, 1, 0)  # [p, c, o]
                hi = wt.astype(E4)
                lo = (wt - hi.astype(np.float32)).astype(E4)
                t[:, i, 0::2, :] = hi
                t[:, i, 1::2, :] = lo
            out[g] = t.reshape(P, -1)
        return out

    def wtile8_t3(w):
        """w [O, D] (pre-scaled) -> [O//P, P, 3*chn*P] fp8 (hi, hi, lo)."""
        chn = w.shape[1] // P
        out = np.empty((w.shape[0] // P, P, 3 * chn * P), E4)
        for m in range(w.shape[0] // P):
            blk = w[m * P : (m + 1) * P, :]
            wt = blk.reshape(P, chn, P).transpose(2, 1, 0)  # [p, c, o]
            hi = wt.astype(E4)
            lo = (wt - hi.astype(np.float32)).astype(E4)
            t = np.empty((P, chn, 3, P), E4)
            t[:, :, 0, :] = hi
            t[:, :, 1, :] = hi
            t[:, :, 2, :] = lo
            out[m] = t.reshape(P, -1)
        return out

    def wtile_bf(w, nb, batch):
        """w [O, D] -> [nb//batch, P, batch*CH*P] bf16 tiles."""
        out = np.empty((nb // batch, P, batch * (w.shape[1] // P) * P), ml_dtypes.bfloat16)
        chn = w.shape[1] // P
        for g in range(nb // batch):
            t = np.empty((P, batch, chn, P), np.float32)
            for i in range(batch):
                m = g * batch + i
                blk = w[m * P : (m + 1) * P, :]  # [o, c]
                t[:, i] = blk.reshape(P, chn, P).transpose(2, 1, 0)
            out[g] = t.reshape(P, -1).astype(ml_dtypes.bfloat16)
        return out

    # V weights token-major: [ph, p, ch*512+vc] = 32*qkv_w'[2D+ph*512+vc, ch*128+p]
    wv = np.empty((2, P, CH * TOK), E4)
    for ph in range(2):
        blk = qkv_w[2 * D + ph * TOK : 2 * D + (ph + 1) * TOK, :] * 32.0  # [vc, c]
        wv[ph] = blk.reshape(TOK, CH, P).transpose(2, 1, 0).reshape(P, CH * TOK).astype(E4)

    bqk = np.zeros((P, 16), np.float32)
    bqk[:, 0:8] = _stripe(16.0 * qkv_bias[0:D])
    bqk[:, 8:16] = _stripe(16.0 * qkv_bias[D : 2 * D])
    bpf = np.zeros((P, 16), np.float32)
    bpf[:, 0:8] = _stripe(inputs["proj_b"])
    bpf[:, 8:16] = _stripe(inputs["fc2_b"])

    shared = {
        "wqk8": wtile8(qkv_w, list(range(16))),
        "wv8": wv,
        "wp8": wtile8(proj_w, list(range(CH))),
        "w1f8": wtile8_hl(fc1_w * 32.0, HCH, 4),
        "w2f8": wtile8_t3(fc2_w * 64.0),
        "biasqk": bqk,
        "biaspf": bpf,
        "fc1b": _stripe(
            np.asarray(inputs["fc1_b"], np.float32)
            + np.asarray(inputs["fc1_w"], np.float32) @ b2
        ),
    }
    in_maps = []
    for c in range(NCORES):
        b, blk = divmod(c, RANKS)
        xblk = x[b, blk * TOK : (blk + 1) * TOK, :]  # [TOK, D]
        xt = round_fp32r(np.ascontiguousarray(xblk.T))
        m = dict(shared)
        m["xT"] = xt
        m["x8T"] = xt.astype(E4)
        in_maps.append(m)
    return in_maps


def _assemble(results):
    out = np.empty((B, N, D), dtype=np.float32)
    for c in range(NCORES):
        b, blk = divmod(c, RANKS)
        out[b, blk * TOK : (blk + 1) * TOK, :] = results[c]["outT"].T
    return out


def run_device(inputs, **kwargs):
    nc = _get_program()
    in_maps = _prep_inputs(inputs)
    res = run_bass_kernel_spmd(nc, in_maps, core_ids=list(range(NCORES)), **kwargs)
    return _assemble(res.results), res


def kernel(**inputs) -> np.ndarray:
    out, _ = run_device(inputs)
    return out


# revision 16
# speedup vs baseline: 1.0630x; 1.0006x over previous
"""Trainium2 Bass kernel v2: fp8 DoubleRow attention + bf16 MLP.

Sharding: sequence-parallel over 8 cores (512 tokens each, batch = core//4).
One 4-rank AllGather carries fp8 K (feature-major) + fp8 V (token-major).

Precision plan (validated in numpy, max_rel ~1.0e-2 vs 2e-2 gate):
  - weights qkv/proj: e4m3 x32 host-scaled; fc1/fc2: bf16 (MLP dominates error)
  - h1/q/k/v/pt/ctx: e4m3 (q,k,v at sigma~16 via 0.5 consume scale)
  - softmax exp: constant shift C=4 (cancels in normalize); split between
    Act (native Exp -> fp8) and DVE (Schraudolph: psum*a+b -> uint8 whose
    bit pattern IS e4m3 2^x; floor-vs-round ambiguity is a constant factor
    that cancels in the softmax normalize)
  - x residual fp32r; LN stats via ones-matmul (fp8 DoubleRow for LN1 on
    host-provided x8, fp32r for LN2)
DoubleRow pair slots: chunk pairs for QKV/ctx/proj; (k, zeros) for scores
(d=64 contraction cannot pair; zero slot makes the 0.5 cyc/row rate legal).
"""
import sys

sys.path.insert(0, "/opt/trn_rl_repo")
import numpy as np
import ml_dtypes
import concourse.bass as bass
import concourse.mybir as mybir
import concourse.tile as tile
from concourse import bacc
from concourse.bass_utils import run_bass_kernel_spmd

B, N, D = 2, 2048, 1024
H, DH = 16, 64
HID = 4096
NCORES = 8
TOK = (B * N) // NCORES  # 512
EPS = 1e-5
SCALE = DH**-0.5
P = 128
CH = D // P  # 8
KC = N // P  # 16
HCH = HID // P  # 32
RANKS = 4
CSH = 4.0  # exp arg shift, cancels in softmax
LN2_ = float(np.log(2.0))
# Schraudolph uint8-as-e4m3: y = psum * SA + SB
SA = 8.0 * (2.0**-11) / LN2_
SB = 56.5 - 8.0 * CSH / LN2_

F32 = mybir.dt.float32
F32R = mybir.dt.float32r
F8 = mybir.dt.float8e4
BF = mybir.dt.bfloat16
U8 = mybir.dt.uint8
AF = mybir.ActivationFunctionType
OP = mybir.AluOpType
DR = mybir.MatmulPerfMode.DoubleRow

REPLICA_GROUPS = [[0, 1, 2, 3], [4, 5, 6, 7]]

KV_K = D * TOK  # bytes of K region (fp8 feature-major [1024, 512])
DHP = DH + 16  # per-head stride in V region: 64 v + 1 ones + 15 pad
# (dual-fp8 LdWeights requires 16B-aligned weight base addresses)
KV_V = TOK * (H * DHP)  # V region [512, 1280]
KV_SZ = KV_K + KV_V


def round_fp32r(x: np.ndarray) -> np.ndarray:
    u = np.ascontiguousarray(x, dtype=np.float32).view(np.uint32)
    u = (u + 0x7FF + ((u >> 12) & 1)) & np.uint32(0xFFFFF000)
    return u.view(np.float32)


def _stripe(v: np.ndarray) -> np.ndarray:
    """[M] -> [P, M//P] with col m, part p = v[m*128+p]."""
    return np.ascontiguousarray(np.asarray(v, np.float32).reshape(-1, P).T)


def build_program(do_compile=True):
    nc = bacc.Bacc("TRN2", target_bir_lowering=False, debug=False, num_devices=NCORES)

    xT = nc.dram_tensor("xT", [D, TOK], F32, kind="ExternalInput").ap()
    x8T = nc.dram_tensor("x8T", [D, TOK], F8, kind="ExternalInput").ap()
    # weight tiles, DMA-contiguous per partition
    wqk8 = nc.dram_tensor("wqk8", [16, P, CH * P], F8, kind="ExternalInput").ap()
    wv8 = nc.dram_tensor("wv8", [2, P, CH * TOK], F8, kind="ExternalInput").ap()
    wp8 = nc.dram_tensor("wp8", [CH, P, CH * P], F8, kind="ExternalInput").ap()
    # fc1 weights as fp8 hi/lo pairs [.., (chunk, hi/lo), out]; fc2 stays bf16
    w1f8 = nc.dram_tensor(
        "w1f8", [CH, P, 4 * 2 * CH * P], F8, kind="ExternalInput"
    ).ap()
    # fc2 weights fp8 (hi, hi_dup, lo) triples per chunk (dups host-side)
    w2f8 = nc.dram_tensor(
        "w2f8", [CH, P, 3 * HCH * P], F8, kind="ExternalInput"
    ).ap()
    biasqk = nc.dram_tensor("biasqk", [P, 16], F32, kind="ExternalInput").ap()
    biaspf = nc.dram_tensor("biaspf", [P, 16], F32, kind="ExternalInput").ap()
    fc1b = nc.dram_tensor("fc1b", [P, HCH], F32, kind="ExternalInput").ap()
    outT = nc.dram_tensor("outT", [D, TOK], F32, kind="ExternalOutput").ap()

    with tile.TileContext(nc) as tc:
        with (
            tc.tile_pool(name="consts", bufs=1) as consts,
            tc.tile_pool(name="bigs", bufs=1) as bigs,
            tc.tile_pool(name="work", bufs=3) as work,
            tc.tile_pool(name="wpool", bufs=2) as wpool,
            tc.tile_pool(name="kvz", bufs=2) as kvz,
            tc.tile_pool(name="pp", bufs=2) as ppool,
            tc.tile_pool(name="rows", bufs=3) as rows,
            tc.tile_pool(name="bc", bufs=2) as bcpool,
            tc.tile_pool(name="stg", bufs=2) as stg,
            tc.tile_pool(name="dram", bufs=1, space="DRAM") as dram,
        ):
            ones8 = consts.tile([P, 1], F8)
            nc.vector.memset(ones8[:].bitcast(U8), 0x38)  # e4m3 1.0
            ones_r = consts.tile([P, 1], F32R)
            nc.vector.memset(ones_r[:].bitcast(F32), 1.0)
            eps_row = consts.tile([1, 1], F32, tag="eps")
            nc.vector.memset(eps_row[:], EPS)
            negc_row = consts.tile([P, 1], F32, tag="negc")
            nc.vector.memset(negc_row[:], -CSH)
            onesv = consts.tile([P, 4, H], F8, tag="onesv")
            nc.vector.memset(onesv[:].bitcast(U8), 0x38)
            biasqk_sb = consts.tile([P, 16], F32, tag="bqk")
            biaspf_sb = consts.tile([P, 16], F32, tag="bpf")
            fc1b_sb = consts.tile([P, HCH], F32, tag="b1")
            nc.sync.dma_start(biasqk_sb[:], biasqk[:])
            nc.sync.dma_start(biaspf_sb[:], biaspf[:])
            nc.sync.dma_start(fc1b_sb[:], fc1b[:])

            kv_in = dram.tile([KV_SZ], F8, tag="kvin")
            kv_out = dram.tile([RANKS * KV_SZ], F8, tag="kvout")
            vk_in = kv_in[0:KV_K].rearrange("(f t) -> f t", t=TOK)
            vv_in = kv_in[KV_K:KV_SZ].rearrange(
                "(t v) -> t v", v=H * DHP
            )
            kv_or = kv_out[:].rearrange("(r x) -> r x", r=RANKS)
            # ones column of the V region: written up-front (no data deps)
            for tc_ in range(RANKS):
                ones_dst = vv_in[tc_ * P : (tc_ + 1) * P, :].rearrange(
                    "p (h c) -> p h c", c=DHP
                )[:, :, DH : DH + 1]
                nc.sync.dma_start(
                    ones_dst, onesv[:, tc_, :].rearrange("p (h c) -> p h c", c=1)
                )

            # ---- LN1 ---- (x8 first: stats depend on it; xr arrives later)
            x8_sb = bigs.tile([P, CH, TOK], F8, tag="x8")
            nc.sync.dma_start(
                x8_sb[:], x8T.rearrange("(ch p) t -> p ch t", p=P)
            )
            xr_sb = bigs.tile([P, CH, TOK], F32, tag="xr")
            nc.sync.dma_start(
                xr_sb[:], xT.rearrange("(ch p) t -> p ch t", p=P)
            )
            sq8 = bigs.tile([P, CH, TOK], F8, tag="h1")  # released before h1

            def ln_rows(psum_mu, psum_s2, name):
                mu = rows.tile([1, TOK], F32, tag="r", name=f"mu{name}")
                nc.vector.tensor_scalar_mul(mu[:], psum_mu[:], 1.0 / D)
                var = rows.tile([1, TOK], F32, tag="r", name=f"va{name}")
                nc.vector.tensor_tensor(var[:], mu[:], mu[:], OP.mult)
                ex2 = rows.tile([1, TOK], F32, tag="r", name=f"e2{name}")
                nc.vector.tensor_scalar_mul(ex2[:], psum_s2[:], 1.0 / D)
                nc.vector.tensor_sub(var[:], ex2[:], var[:])
                rstd = rows.tile([1, TOK], F32, tag="r", name=f"rs{name}")
                nc.scalar.activation(
                    out=rstd[:], in_=var[:], func=AF.Sqrt, bias=eps_row[:]
                )
                nc.vector.reciprocal(rstd[:], rstd[:])
                cpos = rows.tile([1, TOK], F32, tag="r", name=f"cp{name}")
                nc.vector.tensor_tensor(cpos[:], mu[:], rstd[:], OP.mult)
                rstd_b = bcpool.tile([P, TOK], F32, tag="bc", name=f"rb{name}")
                nc.gpsimd.partition_broadcast(rstd_b[:], rstd[:])
                c_b = bcpool.tile([P, TOK], F32, tag="bc", name=f"cb{name}")
                nc.gpsimd.partition_broadcast(c_b[:], cpos[:])
                return rstd_b, c_b

            with tc.tile_pool(name="ps_row1", bufs=2, space="PSUM") as prow:
                psum_mu = prow.tile([1, TOK], F32, tag="row")
                psum_s2 = prow.tile([1, TOK], F32, tag="row")
                for ch in range(CH):
                    eng = nc.vector if ch % 2 == 0 else nc.gpsimd
                    eng.tensor_tensor(
                        sq8[:, ch, :], x8_sb[:, ch, :], x8_sb[:, ch, :], OP.mult
                    )
                for ch in range(CH):
                    nc.tensor.matmul(
                        psum_mu[:],
                        ones8[:],
                        x8_sb[:, ch, :],
                        start=(ch == 0),
                        stop=(ch == CH - 1),
                    )
                for ch in range(CH):
                    nc.tensor.matmul(
                        psum_s2[:],
                        ones8[:],
                        sq8[:, ch, :],
                        start=(ch == 0),
                        stop=(ch == CH - 1),
                    )
                rstd1_b, c1_b = ln_rows(psum_mu, psum_s2, "1")

            h1 = bigs.tile([P, CH, TOK], F8, tag="h1")
            for ch in range(CH):
                eng = nc.vector if ch % 2 == 0 else nc.gpsimd
                t1 = work.tile([P, TOK], F32, tag="t1")
                eng.tensor_tensor(t1[:], xr_sb[:, ch, :], rstd1_b[:], OP.mult)
                eng.tensor_tensor(h1[:, ch, :], t1[:], c1_b[:], OP.subtract)

            # ---- QKV ----
            st2 = tc.tile_pool(name="ps_mm2", bufs=5, space="PSUM")
            ps_acc = st2.__enter__()

            def qkv_block(wt, i, m, consume):
                psum = ps_acc.tile([P, TOK], F32, tag="acc", name=f"ps_{m}")
                for j in range(CH // 2):
                    nc.tensor.matmul(
                        psum[:],
                        wt[:, i, 2 * j : 2 * j + 2, :],
                        h1[:, 2 * j : 2 * j + 2, :],
                        start=(j == 0),
                        stop=(j == CH // 2 - 1),
                        perf_mode=DR,
                    )
                consume(m, psum)

            def k_consume(m, psum):
                k8 = stg.tile([P, TOK], F8, tag="cp", name=f"k8_{m}")
                nc.scalar.activation(
                    out=k8[:],
                    in_=psum[:],
                    func=AF.Identity,
                    bias=biasqk_sb[:, 8 + m : 9 + m],
                    scale=0.5,
                )
                nc.scalar.dma_start(vk_in[m * P : (m + 1) * P, :], k8[:])

            # K and V interleaved: K consumes on Act, V on DVE run in
            # parallel so the AllGather input completes sooner
            for half in range(2):
                wk = wpool.tile([P, 4, CH, P], F8, tag="wq", name=f"wk{half}")
                nc.sync.dma_start(
                    wk[:],
                    wqk8[8 + 4 * half : 12 + 4 * half].rearrange(
                        "i p (ch o) -> p i ch o", ch=CH
                    ),
                )
                wv_t = wpool.tile([P, CH, TOK], F8, tag="wq", name=f"wv{half}")
                nc.sync.dma_start(
                    wv_t[:], wv8[half].rearrange("p (ch v) -> p ch v", ch=CH)
                )
                for i in range(4):
                    qkv_block(wk, i, 4 * half + i, k_consume)
                    psum = ps_acc.tile(
                        [P, TOK], F32, tag="acc", name=f"pv{half}_{i}"
                    )
                    for j in range(CH // 2):
                        nc.tensor.matmul(
                            psum[:],
                            h1[:, 2 * j : 2 * j + 2, i * P : (i + 1) * P],
                            wv_t[:, 2 * j : 2 * j + 2, :],
                            start=(j == 0),
                            stop=(j == CH // 2 - 1),
                            perf_mode=DR,
                        )
                    v8 = stg.tile([P, TOK], F8, tag="cp", name=f"v8_{half}_{i}")
                    nc.vector.tensor_scalar_mul(v8[:], psum[:], 0.5)
                    dst = vv_in[
                        i * P : (i + 1) * P,
                        half * 8 * DHP : (half + 1) * 8 * DHP,
                    ].rearrange("t (h c) -> t h c", c=DHP)[:, :, 0:DH]
                    nc.gpsimd.dma_start(
                        dst, v8[:].rearrange("t (h d) -> t h d", d=DH)
                    )

            nc.gpsimd.collective_compute(
                "AllGather",
                OP.bypass,
                ins=[kv_in[:].opt()],
                outs=[kv_out[:].opt()],
                replica_groups=REPLICA_GROUPS,
            )

            # Q blocks -> SBUF (chunk 8 duplicates chunk 7 for the hp=7 rhs pair)
            qT = bigs.tile([P, CH + 1, TOK], F8, tag="qT")

            def q_consume(m, psum):
                nc.vector.tensor_scalar(
                    out=qT[:, m, :],
                    in0=psum[:],
                    scalar1=0.5,
                    scalar2=biasqk_sb[:, m : m + 1],
                    op0=OP.mult,
                    op1=OP.add,
                )

            for g in range(2):
                wt = wpool.tile([P, 4, CH, P], F8, tag="wq", name=f"wq{g}")
                nc.sync.dma_start(
                    wt[:],
                    wqk8[4 * g : 4 * g + 4].rearrange("i p (ch o) -> p i ch o", ch=CH),
                )
                for i in range(4):
                    qkv_block(wt, i, 4 * g + i, q_consume)
            nc.sync.dma_start(qT[:, CH, :], qT[:, CH - 1, :])
            st2.__exit__(None, None, None)

            # ---- attention ----
            # kp: [P(2 heads' d), 2, KC, P] fp8; slot0 = K data, slot1 = zeros
            kp_t = []
            for i in range(2):
                t = kvz.tile([P, 2, KC, P], F8, tag="kp", name=f"kp{i}")
                nc.vector.memset(t[:, 1, :, :].bitcast(U8), 0)
                kp_t.append(t)
            # vfull: [P(key-in-chunk), KC, 16*(64+1)] fp8, ones pre-gathered
            vfull = bigs.tile([P, KC, H * DHP], F8, tag="vfull")
            for r in range(RANKS):
                src = (
                    kv_or[r : r + 1, KV_K:KV_SZ]
                    .rearrange("o (t v) -> o t v", v=H * DHP)[0]
                    .rearrange("(tc p) v -> p tc v", p=P)
                )
                nc.sync.dma_start(vfull[:, 4 * r : 4 * r + 4, :], src)

            def load_kp(hp, t):
                src = (
                    kv_or[:, 0:KV_K]
                    .rearrange("r (f t) -> r f t", t=TOK)[
                        :, hp * P : (hp + 1) * P, :
                    ]
                    .rearrange("r p t -> p r t")
                )
                nc.sync.dma_start(t[:, 0, :, :], src)

            ctxT = bigs.tile([P, CH, TOK], F8, tag="x8")  # reuse x8 region
            groups = [(2 * j, 2) for j in range(8)]
            pairs_after = {j: [j] for j in range(8)}

            att_pools = (
                tc.tile_pool(name="ps_s", bufs=2, space="PSUM"),
                tc.tile_pool(name="ps_ctx", bufs=2, space="PSUM"),
            )
            ps_spool = att_pools[0].__enter__()
            ps_ctx = att_pools[1].__enter__()
            eidx = 0
            for hp in range(H // 2):
                kp = kp_t[hp % 2]
                load_kp(hp, kp)
                for hh in range(2):
                    h = 2 * hp + hh
                    half = slice(hh * DH, hh * DH + DH)
                    qpair = qT[half, hp : hp + 2, :]
                    pt = ppool.tile([P, KC, TOK], F8, tag="pt", name=f"pt{h}")
                    psum_c = ps_ctx.tile([DH + 1, TOK], F32, tag="ctx")
                    for gi, (kc0, nb) in enumerate(groups):
                        ps_s = ps_spool.tile([P, 2 * TOK], F32, tag="s", bufs=3)
                        for j in range(nb):
                            nc.tensor.matmul(
                                ps_s[:, j * TOK : (j + 1) * TOK],
                                kp[half, :, kc0 + j, :],
                                qpair,
                                start=True,
                                stop=True,
                                perf_mode=DR,
                            )
                        dst = pt[:, kc0 : kc0 + nb, :]
                        # alternate Act/DVE so consecutive groups of a head
                        # never serialize on one engine (GPSIMD can't read
                        # PSUM per the BIR verifier); 4:2 toward Act since
                        # DVE also carries reciprocal + normalize
                        if (gi + h) % 2 == 0 or (h * 8 + gi) % 12 == 3:
                            nc.scalar.activation(
                                out=dst,
                                in_=ps_s[:, : nb * TOK],
                                func=AF.Exp,
                                bias=negc_row[:],
                                scale=2.0**-11,
                            )
                        else:
                            nc.vector.tensor_scalar(
                                out=dst.bitcast(U8),
                                in0=ps_s[:, : nb * TOK],
                                scalar1=SA,
                                scalar2=SB,
                                op0=OP.mult,
                                op1=OP.add,
                            )
                        eidx += 1
                        for pj in pairs_after.get(gi, []):
                            nc.tensor.matmul(
                                psum_c[:],
                                vfull[
                                    :,
                                    2 * pj : 2 * pj + 2,
                                    h * DHP : h * DHP + DH + 1,
                                ],
                                pt[:, 2 * pj : 2 * pj + 2, :],
                                start=(pj == 0),
                                stop=(pj == KC // 2 - 1),
                                perf_mode=DR,
                            )
                    rrow = rows.tile([1, TOK], F32, tag="r", name=f"rr{h}")
                    nc.vector.reciprocal(rrow[:], psum_c[DH : DH + 1, :])
                    rb = bcpool.tile([DH, TOK], F32, tag="rb", name=f"rb{h}")
                    nc.gpsimd.partition_broadcast(rb[:], rrow[:])
                    nc.vector.tensor_tensor(
                        ctxT[half, hp, :], psum_c[0:DH, :], rb[:], OP.mult
                    )
            att_pools[1].__exit__(None, None, None)
            att_pools[0].__exit__(None, None, None)

            # ---- proj + residual ----
            st4 = tc.tile_pool(name="ps_mm4", bufs=5, space="PSUM")
            ps_mlp = st4.__enter__()
            x2 = bigs.tile([P, CH, TOK], F32R, tag="x2")
            for g in range(2):
                wt = wpool.tile([P, 4, CH, P], F8, tag="wq", name=f"wpj{g}")
                nc.sync.dma_start(
                    wt[:],
                    wp8[4 * g : 4 * g + 4].rearrange("i p (ch o) -> p i ch o", ch=CH),
                )
                for i in range(4):
                    m = 4 * g + i
                    psum = ps_mlp.tile([P, TOK], F32, tag="acc", name=f"pp_{m}")
                    for j in range(CH // 2):
                        nc.tensor.matmul(
                            psum[:],
                            wt[:, i, 2 * j : 2 * j + 2, :],
                            ctxT[:, 2 * j : 2 * j + 2, :],
                            start=(j == 0),
                            stop=(j == CH // 2 - 1),
                            perf_mode=DR,
                        )
                    attn_sb = stg.tile([P, TOK], F32, tag="stg", name=f"at_{m}")
                    nc.scalar.activation(
                        out=attn_sb[:],
                        in_=psum[:],
                        func=AF.Identity,
                        bias=biaspf_sb[:, m : m + 1],
                        scale=2.0**-9,
                    )
                    nc.gpsimd.tensor_tensor(
                        x2[:, m, :], attn_sb[:], xr_sb[:, m, :], OP.add
                    )  # f32r out: rounded on write for the LN2 stats matmul

            # ---- LN2 (fp32r stats on x2) ----
            with tc.tile_pool(name="ps_row2", bufs=2, space="PSUM") as prow:
                psum_mu2 = prow.tile([1, TOK], F32, tag="row")
                psum_s22 = prow.tile([1, TOK], F32, tag="row")
                for ch in range(CH):
                    nc.tensor.matmul(
                        psum_mu2[:],
                        ones_r[:],
                        x2[:, ch, :],
                        start=(ch == 0),
                        stop=(ch == CH - 1),
                    )
                    sq = work.tile([P, TOK], F32R, tag="t1", name=f"sq2_{ch}")
                    nc.gpsimd.tensor_tensor(
                        sq[:],
                        x2[:, ch, :].bitcast(F32),
                        x2[:, ch, :].bitcast(F32),
                        OP.mult,
                    )
                    nc.tensor.matmul(
                        psum_s22[:],
                        ones_r[:],
                        sq[:],
                        start=(ch == 0),
                        stop=(ch == CH - 1),
                    )
                rstd2_b, c2_b = ln_rows(psum_mu2, psum_s22, "2")

            # xn as fp8 hi/lo + duplicated-hi slot: [hi, hi_dup, lo]
            xnf = bigs.tile([P, CH, 3, TOK], F8, tag="xn")
            for ch in range(CH):
                t1 = work.tile([P, TOK], F32, tag="t1", name=f"t2_{ch}")
                nc.gpsimd.tensor_tensor(
                    t1[:], x2[:, ch, :].bitcast(F32), rstd2_b[:], OP.mult
                )
                xn32 = work.tile([P, TOK], F32, tag="xn32", bufs=2, name=f"x32_{ch}")
                nc.vector.tensor_tensor(xn32[:], t1[:], c2_b[:], OP.subtract)
                nc.scalar.activation(
                    out=xnf[:, ch, 0, :], in_=xn32[:], func=AF.Identity
                )
                nc.gpsimd.tensor_copy(
                    out=xnf[:, ch, 1, :], in_=xnf[:, ch, 0, :]
                )
                nc.vector.tensor_tensor(
                    xnf[:, ch, 2, :], xn32[:], xnf[:, ch, 0, :], OP.subtract
                )

            # ---- MLP: fc1 fp8 hi/lo "3-product" DoubleRow, fc2 bf16 ----
            # per chunk pair (c, d): [whi_c,wlo_c]x[xhi,xhidup], same for d,
            # then [whi_c,whi_d]x[xlo_c,xlo_d] (drops the negligible lo*lo)
            g2 = bigs.tile([P, HCH, 2, TOK], F8, tag="g")
            for g in range(CH):
                w1 = wpool.tile([P, 4, 2 * CH, P], F8, tag="w1", name=f"w1_{g}")
                nc.sync.dma_start(
                    w1[:],
                    w1f8[g].rearrange("p (i c o) -> p i c o", i=4, c=2 * CH),
                )
                for i in range(4):
                    m = 4 * g + i
                    psum = ps_mlp.tile([P, TOK], F32, tag="acc", name=f"p1_{m}")
                    for c2 in range(CH // 2):
                        c = 2 * c2
                        nc.tensor.matmul(
                            psum[:],
                            w1[:, i, 2 * c : 2 * c + 2, :],
                            xnf[:, c, 0:2, :],
                            start=(c2 == 0),
                            stop=False,
                            perf_mode=DR,
                        )
                        nc.tensor.matmul(
                            psum[:],
                            w1[:, i, 2 * c + 2 : 2 * c + 4, :],
                            xnf[:, c + 1, 0:2, :],
                            start=False,
                            stop=False,
                            perf_mode=DR,
                        )
                        whi = w1[:, i].rearrange(
                            "p (c two) k -> p c two k", two=2
                        )[:, c : c + 2, 0, :]
                        nc.tensor.matmul(
                            psum[:],
                            whi,
                            xnf[:, c : c + 2, 2, :],
                            start=False,
                            stop=(c2 == CH // 2 - 1),
                            perf_mode=DR,
                        )
                    g32 = work.tile([P, TOK], F32, tag="xn32", bufs=2, name=f"g32_{m}")
                    nc.scalar.activation(
                        out=g32[:],
                        in_=psum[:],
                        func=AF.Gelu,
                        bias=fc1b_sb[:, m : m + 1],
                        scale=2.0**-5,
                    )
                    nc.gpsimd.tensor_copy(out=g2[:, m, 0, :], in_=g32[:])
                    nc.vector.tensor_tensor(
                        g2[:, m, 1, :], g32[:], g2[:, m, 0, :], OP.subtract
                    )
            for m2 in range(CH):
                psum = ps_mlp.tile([P, TOK], F32, tag="acc", name=f"p2_{m2}")
                for hf in range(2):
                    w2 = wpool.tile(
                        [P, 3 * HCH // 2, P], F8, tag="w2", name=f"w2_{m2}_{hf}"
                    )
                    nc.sync.dma_start(
                        w2[:],
                        w2f8[m2][
                            :, hf * 3 * (HCH // 2) * P : (hf + 1) * 3 * (HCH // 2) * P
                        ].rearrange("p (c o) -> p c o", c=3 * HCH // 2),
                    )
                    w2lo = w2.rearrange("p (c three) k -> p c three k", three=3)
                    for c2 in range(HCH // 4):
                        c = 2 * c2
                        hc = hf * (HCH // 2) + c
                        nc.tensor.matmul(
                            psum[:],
                            w2[:, 3 * c : 3 * c + 2, :],
                            g2[:, hc, 0:2, :],
                            start=(hf == 0 and c2 == 0),
                            stop=False,
                            perf_mode=DR,
                        )
                        nc.tensor.matmul(
                            psum[:],
                            w2[:, 3 * c + 3 : 3 * c + 5, :],
                            g2[:, hc + 1, 0:2, :],
                            start=False,
                            stop=False,
                            perf_mode=DR,
                        )
                        nc.tensor.matmul(
                            psum[:],
                            w2lo[:, c : c + 2, 2, :],
                            g2[:, hc : hc + 2, 0, :],
                            start=False,
                            stop=(hf == 1 and c2 == HCH // 4 - 1),
                            perf_mode=DR,
                        )
                o_sb = stg.tile([P, TOK], F32, tag="stg", name=f"o_{m2}")
                nc.scalar.activation(
                    out=o_sb[:],
                    in_=psum[:],
                    func=AF.Identity,
                    bias=biaspf_sb[:, 8 + m2 : 9 + m2],
                    scale=2.0**-6,
                )
                o_f = stg.tile([P, TOK], F32, tag="of", bufs=2, name=f"of_{m2}")
                nc.vector.tensor_add(
                    out=o_f[:], in0=o_sb[:], in1=x2[:, m2, :].bitcast(F32)
                )
                nc.sync.dma_start(outT[m2 * P : (m2 + 1) * P, :], o_f[:])
            st4.__exit__(None, None, None)

    if do_compile:
        nc.compile()
    return nc


_CACHE = {}


def _get_program():
    if "nc" not in _CACHE:
        _CACHE["nc"] = build_program()
    return _CACHE["nc"]


def _prep_inputs(inputs):
    E4 = ml_dtypes.float8_e4m3
    x = np.asarray(inputs["x"], np.float32)
    g1 = np.asarray(inputs["ln1_g"], np.float32)
    b1 = np.asarray(inputs["ln1_b"], np.float32)
    g2 = np.asarray(inputs["ln2_g"], np.float32)
    b2 = np.asarray(inputs["ln2_b"], np.float32)
    qkv_w = np.asarray(inputs["qkv_w"], np.float32) * g1[None, :]
    proj_w = np.asarray(inputs["proj_w"], np.float32)
    fc1_w = np.asarray(inputs["fc1_w"], np.float32) * g2[None, :]
    fc2_w = np.asarray(inputs["fc2_w"], np.float32)

    qkv_bias = np.asarray(inputs["qkv_w"], np.float32) @ b1  # [3D]
    assert np.abs(qkv_bias[2 * D :]).max() == 0.0, "nonzero ln1_b v-bias unsupported"

    def wtile8(w, blocks):
        """w [O, D] -> [nb, P, CH*P] fp8 with [m, p, ch*128+o] = 32*w[m*128+o, ch*128+p]."""
        out = np.empty((len(blocks), P, CH * P), E4)
        for bi, m in enumerate(blocks):
            blk = w[m * P : (m + 1) * P, :] * 32.0  # [o 128, c 1024]
            out[bi] = (
                blk.reshape(P, CH, P).transpose(2, 1, 0).reshape(P, CH * P)
            ).astype(E4)
        return out

    def wtile8_hl(w, nb, batch):
        """w [O, D] (pre-scaled) -> [nb//batch, P, batch*2*chn*P] fp8 hi/lo
        tiles: slot (2c+s) holds hi (s=0) / lo residual (s=1) of chunk c."""
        chn = w.shape[1] // P
        out = np.empty((nb // batch, P, batch * 2 * chn * P), E4)
        for g in range(nb // batch):
            t = np.empty((P, batch, 2 * chn, P), E4)
            for i in range(batch):
                m = g * batch + i
                blk = w[m * P : (m + 1) * P, :]  # [o, c]
                wt = blk.reshape(P, chn, P).transpose(2, 1, 0)  # [p, c, o]
                hi = wt.astype(E4)
                lo = (wt - hi.astype(np.float32)).astype(E4)
                t[:, i, 0::2, :] = hi
                t[:, i, 1::2, :] = lo
            out[g] = t.reshape(P, -1)
        return out

    def wtile8_t3(w):
        """w [O, D] (pre-scaled) -> [O//P, P, 3*chn*P] fp8 (hi, hi, lo)."""
        chn = w.shape[1] // P
        out = np.empty((w.shape[0] // P, P, 3 * chn * P), E4)
        for m in range(w.shape[0] // P):
            blk = w[m * P : (m + 1) * P, :]
            wt = blk.reshape(P, chn, P).transpose(2, 1, 0)  # [p, c, o]
            hi = wt.astype(E4)
            lo = (wt - hi.astype(np.float32)).astype(E4)
            t = np.empty((P, chn, 3, P), E4)
            t[:, :, 0, :] = hi
            t[:, :, 1, :] = hi
            t[:, :, 2, :] = lo
            out[m] = t.reshape(P, -1)
        return out

    def wtile_bf(w, nb, batch):
        """w [O, D] -> [nb//batch, P, batch*CH*P] bf16 tiles."""
        out = np.empty((nb // batch, P, batch * (w.shape[1] // P) * P), ml_dtypes.bfloat16)
        chn = w.shape[1] // P
        for g in range(nb // batch):
            t = np.empty((P, batch, chn, P), np.float32)
            for i in range(batch):
                m = g * batch + i
                blk = w[m * P : (m + 1) * P, :]  # [o, c]
                t[:, i] = blk.reshape(P, chn, P).transpose(2, 1, 0)
            out[g] = t.reshape(P, -1).astype(ml_dtypes.bfloat16)
        return out

    # V weights token-major: [ph, p, ch*512+vc] = 32*qkv_w'[2D+ph*512+vc, ch*128+p]
    wv = np.empty((2, P, CH * TOK), E4)
    for ph in range(2):
        blk = qkv_w[2 * D + ph * TOK : 2 * D + (ph + 1) * TOK, :] * 32.0  # [vc, c]
        wv[ph] = blk.reshape(TOK, CH, P).transpose(2, 1, 0).reshape(P, CH * TOK).astype(E4)

    bqk = np.zeros((P, 16), np.float32)
    bqk[:, 0:8] = _stripe(16.0 * qkv_bias[0:D])
    bqk[:, 8:16] = _stripe(16.0 * qkv_bias[D : 2 * D])
    bpf = np.zeros((P, 16), np.float32)
    bpf[:, 0:8] = _stripe(inputs["proj_b"])
    bpf[:, 8:16] = _stripe(inputs["fc2_b"])

    shared = {
        "wqk8": wtile8(qkv_w, list(range(16))),
        "wv8": wv,
        "wp8": wtile8(proj_w, list(range(CH))),
        "w1f8": wtile8_hl(fc1_w * 32.0, HCH, 4),
        "w2f8": wtile8_t3(fc2_w * 64.0),
        "biasqk": bqk,
        "biaspf": bpf,
        "fc1b": _stripe(
            np.asarray(inputs["fc1_b"], np.float32)
            + np.asarray(inputs["fc1_w"], np.float32) @ b2
        ),
    }
    in_maps = []
    for c in range(NCORES):
        b, blk = divmod(c, RANKS)
        xblk = x[b, blk * TOK : (blk + 1) * TOK, :]  # [TOK, D]
        xt = round_fp32r(np.ascontiguousarray(xblk.T))
        m = dict(shared)
        m["xT"] = xt
        m["x8T"] = xt.astype(E4)
        in_maps.append(m)
    return in_maps


def _assemble(results):
    out = np.empty((B, N, D), dtype=np.float32)
    for c in range(NCORES):
        b, blk = divmod(c, RANKS)
        out[b, blk * TOK : (blk + 1) * TOK, :] = results[c]["outT"].T
    return out


def run_device(inputs, **kwargs):
    nc = _get_program()
    in_maps = _prep_inputs(inputs)
    res = run_bass_kernel_spmd(nc, in_maps, core_ids=list(range(NCORES)), **kwargs)
    return _assemble(res.results), res


def kernel(**inputs) -> np.ndarray:
    out, _ = run_device(inputs)
    return out
